# revision 1
# baseline (speedup 1.0000x reference)
"""Wilson-clover Dirac operator D_WC on Trainium2, 8-core SPMD.

Self-contained: hardcodes LAT=(32,16,16,16), shards the T axis across 8
cores with host-side halo slices (t0-1..t0+4), computes everything
site-locally on-device with DVE elementwise math in an SoA layout:

  plane[p, col]: p = z*8 + (y>>1),  col = plane_idx*W + (s-base)*32
                 + (y&1)*16 + x          (s = local T-slice, W = ns*32)

Shifts: T = column offset (free), X = 2-piece AP copy, Z = 2-piece
partition-offset DMA, Y = 1 copy + 2 DMAs.
"""
import numpy as np

# ----------------------------------------------------------------------
# constants (physics)
# ----------------------------------------------------------------------
MASS, C_SW = 0.5, 1.0
_i = 1j
G1 = np.array([[0,0,0,_i],[0,0,_i,0],[0,-_i,0,0],[-_i,0,0,0]], dtype=np.complex64)
G2 = np.array([[0,0,0,-1],[0,0,1,0],[0,1,0,0],[-1,0,0,0]], dtype=np.complex64)
G3 = np.array([[0,0,_i,0],[0,0,0,-_i],[-_i,0,0,0],[0,_i,0,0]], dtype=np.complex64)
G4 = np.array([[0,0,1,0],[0,0,0,1],[1,0,0,0],[0,1,0,0]], dtype=np.complex64)
GAMMA = [G1, G2, G3, G4]
I4 = np.eye(4, dtype=np.complex64)
SIGMA = [[(0.5j * (GAMMA[m] @ GAMMA[n] - GAMMA[n] @ GAMMA[m])).astype(np.complex64)
          for n in range(4)] for m in range(4)]

T_, Z_, Y_, X_ = 32, 16, 16, 16
NCORES, TLOC, NSH = 8, 4, 6       # halo slices per core
SL = 32                           # cols per slice
ENGINE_MIX = True                 # offload product rows to GPSIMD, copies to ACT
USE_BF16 = True                   # bf16 compute tiles (O stays fp32)
SIM_SAFE = False                  # True: per-z Y-wrap DMAs (CoreSim-compatible)
F32 = None                        # set after mybir import (device build only)

# ----------------------------------------------------------------------
# spin-structure extraction (numerical)
# ----------------------------------------------------------------------
def _col_struct(M):
    """M has single-nonzero columns: return per-col (row, phase)."""
    out = []
    for s in range(4):
        col = M[:, s]
        r = int(np.argmax(np.abs(col)))
        ph = complex(col[r])
        assert np.sum(np.abs(col) > 1e-6) == 1, (M, s)
        out.append((r, ph))
    return out

def _proj_struct(P):
    """P rank-2 with cols 2,3 = rho * cols q in {0,1}. Returns (q2,rho2,q3,rho3)."""
    res = []
    for s in (2, 3):
        found = None
        for j in (0, 1):
            c, cj = P[:, s], P[:, j]
            nz = np.abs(cj) > 1e-6
            if not nz.any():
                continue
            ratio = c[nz] / cj[nz]
            if np.allclose(ratio, ratio[0], atol=1e-5) and np.allclose(
                    c, ratio[0] * cj, atol=1e-5):
                found = (j, complex(ratio[0]))
                break
        assert found is not None, P
        res.append(found)
    return res

# per direction k: gamma column structure and projector relations
GCOL = [_col_struct(GAMMA[k]) for k in range(4)]           # (p(s), g(s))
PROJ_F = [_proj_struct(GAMMA[k] + I4) for k in range(4)]   # for psi@(G+I)
PROJ_B = [_proj_struct(GAMMA[k] - I4) for k in range(4)]   # for psi@(G-I)
SIGCOL = [[_col_struct(SIGMA[m][n]) if m != n else None for n in range(4)]
          for m in range(4)]

def _phase_parts(ph):
    """phase in {1,-1,i,-i} -> for v = ph*u:
    re(v) = sr*u.[pr]  im(v) = si*u.[pi]   (0=re,1=im planes of u)"""
    if abs(ph - 1) < 1e-5:   return (0, 1.0, 1, 1.0)
    if abs(ph + 1) < 1e-5:   return (0, -1.0, 1, -1.0)
    if abs(ph - 1j) < 1e-5:  return (1, -1.0, 0, 1.0)
    if abs(ph + 1j) < 1e-5:  return (1, 1.0, 0, -1.0)
    raise AssertionError(ph)

# ----------------------------------------------------------------------
# host layout helpers
# ----------------------------------------------------------------------
def _to_planes(vol):
    """vol [ns, Z, Y, X] -> [128, ns*32]"""
    ns = vol.shape[0]
    v = vol.reshape(ns, 16, 8, 2, 16)        # s z yh yl x
    v = np.transpose(v, (1, 2, 0, 3, 4))     # z yh s yl x
    return np.ascontiguousarray(v.reshape(128, ns * 32))

def _from_planes(pl, ns):
    v = pl.reshape(16, 8, ns, 2, 16)
    return np.transpose(v, (2, 0, 1, 3, 4)).reshape(ns, 16, 16, 16)

def _host_inputs(field_re, field_im, gauge_re, gauge_im, core):
    t0 = core * TLOC
    ts = [(t0 - 1 + s) % T_ for s in range(NSH)]
    f = np.stack([field_re[ts], field_im[ts]], axis=0)   # [2,6,Z,Y,X,3,4]
    g = np.stack([gauge_re[:, ts], gauge_im[:, ts]], axis=0)  # [2,4,6,Z,Y,X,3,3]

    # field planes: e=(c*4+sp), plane=e*2+r
    fp = np.empty((24, 128, NSH * SL), np.float32)
    for c in range(3):
        for sp in range(4):
            for r in range(2):
                fp[(c * 4 + sp) * 2 + r] = _to_planes(f[r, :, :, :, :, c, sp])
    fld = np.ascontiguousarray(fp.transpose(1, 0, 2).reshape(128, 24 * NSH * SL))

    # gauge planes: dir k, e=(a*3+b), plane=(k*9+e)*2+r
    gp = np.empty((72, 128, NSH * SL), np.float32)
    for k in range(4):
        for a in range(3):
            for b in range(3):
                for r in range(2):
                    gp[(k * 9 + a * 3 + b) * 2 + r] = _to_planes(
                        g[r, k, :, :, :, :, a, b])
    gg = np.ascontiguousarray(gp.transpose(1, 0, 2).reshape(128, 72 * NSH * SL))
    if USE_BF16:
        import ml_dtypes
        fld = fld.astype(ml_dtypes.bfloat16)
        gg = gg.astype(ml_dtypes.bfloat16)
    return {"fld": fld, "gg": gg}

def _host_output(outp_flat):
    """device out [128, 24*4*32] -> [TLOC, Z,Y,X, 3,4,2]"""
    pl = outp_flat.reshape(128, 24, TLOC * SL).transpose(1, 0, 2)
    out = np.empty((TLOC, 16, 16, 16, 3, 4, 2), np.float32)
    for c in range(3):
        for sp in range(4):
            for r in range(2):
                out[..., c, sp, r] = _from_planes(pl[(c * 4 + sp) * 2 + r], TLOC)
    return out

# ----------------------------------------------------------------------
# device program
# ----------------------------------------------------------------------
class Grp:
    """Group of planes in one SBUF tile. nent complex entries (re+im planes).
    base = slice index of col 0; ns slices; W = ns*32 cols per plane."""
    def __init__(self, tile, nent, base, ns):
        self.t, self.nent, self.base, self.ns = tile, nent, base, ns
        self.W = ns * SL

    def fl(self):
        return self.t.rearrange("p (q w) -> p q w", q=self.nent * 2, w=self.W)

    def cs(self, s0, s1):
        return ((s0 - self.base) * SL, (s1 - self.base) * SL)

    def pap(self, e, r, s0, s1):
        """single plane AP [128, cols]"""
        c0, c1 = self.cs(s0, s1)
        v = self.fl()[:, (e * 2 + r):(e * 2 + r + 1), c0:c1]
        return v  # [p,1,w]

    def gap(self, e0, estep, n, r, s0, s1):
        """packed-entry AP [p, n, w]: entries e0 + i*estep, fixed r."""
        c0, c1 = self.cs(s0, s1)
        q0 = e0 * 2 + r
        fl = self.fl()
        return fl[:, q0:q0 + 2 * estep * (n - 1) + 1:2 * estep, c0:c1]

    def all_ap(self, s0, s1):
        c0, c1 = self.cs(s0, s1)
        return self.fl()[:, :, c0:c1]


def build_program():
    import concourse.bacc as bacc
    import concourse.mybir as mybir
    from concourse.tile import TileContext
    FP = mybir.dt.float32
    CDT = mybir.dt.bfloat16 if USE_BF16 else FP
    AL = mybir.AluOpType

    nc = bacc.Bacc("TRN2", target_bir_lowering=False, debug=False)
    fld_d = nc.declare_dram_parameter("fld", [128, 24 * NSH * SL], CDT, isOutput=False)
    gg_d = nc.declare_dram_parameter("gg", [128, 72 * NSH * SL], CDT, isOutput=False)
    out_d = nc.declare_dram_parameter("outp", [128, 24 * TLOC * SL], FP, isOutput=True)

    with TileContext(nc) as tc:
        _tc = [0]

        def mk(pool, cols, tag, dt=None):
            _tc[0] += 1
            return pool.tile([128, cols], dt or CDT, tag=tag,
                             name=f"{tag}_{_tc[0]}")

        main_cm = tc.tile_pool(name="main", bufs=1)
        main = main_cm.__enter__()
        gpool_cm = tc.tile_pool(name="gp", bufs=2)
        gpool = gpool_cm.__enter__()
        spool_cm = tc.tile_pool(name="sp", bufs=1)
        spool = spool_cm.__enter__()

        F = Grp(mk(main, 24 * NSH * SL, "F"), 12, 0, NSH)
        O = Grp(mk(main, 24 * TLOC * SL, "O", FP), 12, 1, TLOC)
        nc.sync.dma_start(out=F.t[:, :], in_=fld_d[:, :])

        def load_dir(k, tag):
            g = Grp(mk(gpool, 9 * 2 * NSH * SL, tag), 9, 0, NSH)
            nc.sync.dma_start(out=g.t[:, :],
                              in_=gg_d[:, k * 18 * NSH * SL:(k + 1) * 18 * NSH * SL])
            return g

        # merged scratch tiles (2 halves each), per compute engine
        WMX = 5 * SL
        mtA = mk(spool, 2 * 9 * WMX, "mtA")
        msA = mk(spool, 2 * 3 * WMX, "msA")
        mtG = mk(spool, 2 * 9 * WMX, "mtG")
        msG = mk(spool, 2 * 3 * WMX, "msG")

        def tview2(t, nj, nk, w):
            """[p, 2, nj, nk, w] halves of merged mul scratch"""
            v = t.rearrange("p (h m) -> p h m", h=2)
            return v[:, :, :nj * nk * w].rearrange(
                "p h (j k w) -> p h j k w", j=nj, k=nk, w=w)

        def sview2(t, nj, w):
            v = t.rearrange("p (h m) -> p h m", h=2)
            return v[:, :, :nj * w].rearrange("p h (j w) -> p h j w",
                                              j=nj, w=w)

        V = nc.vector

        def a_pack(A, i, adag, r, s0, s1, dt, nj):
            """[p, nj(bcast), 3, w] for a-values (i,k)."""
            e0, st = (i, 3) if adag else (i * 3, 1)
            ap = A.gap(e0, st, 3, r, s0 + dt, s1 + dt)       # [p,3,w]
            w = ap.shape[2]
            return ap.unsqueeze(1).broadcast_to((128, nj, 3, w))

        def b_pack(B, bdag, r, s0, s1, dt, nj):
            """[p, nj, 3, w] for b-values (k,j)."""
            c0, c1 = B.cs(s0 + dt, s1 + dt)
            w = c1 - c0
            if bdag:  # e = j*3+k
                v = B.t.rearrange("p (j k r w) -> p j k r w", j=3, k=3, r=2, w=B.W)
                return v[:, :, :, r, c0:c1]
            if B.nent == 9:  # e = k*3+j
                v = B.t.rearrange("p (k j r w) -> p k j r w", k=3, j=3, r=2, w=B.W)
                return v[:, :, :, r, c0:c1].transpose([0, 2, 1, 3])
            # halfspinor: e = k*2+j, nj=2
            v = B.t.rearrange("p (k j r w) -> p k j r w", k=3, j=2, r=2, w=B.W)
            return v[:, :, :, r, c0:c1].transpose([0, 2, 1, 3])

        def stt(out, in0, coef, in1, op1=None, eng=None):
            (eng or V).scalar_tensor_tensor(out, in0, float(coef), in1,
                                            AL.mult, op1 or AL.add)

        DV_SET = (V, mtA, msA)
        GP_SET = (nc.gpsimd, mtG, msG)
        _pc = [0]  # product counter for gp alternation

        def cmm(dst, A, B, s0, s1, adag=False, bdag=False, adt=0, bdt=0, nj=3):
            """dst[i,j] = sum_k aval(i,k)*bval(k,j); dst entries e=i*nj+j.
            No (adag and bdag) case: signs reduce to add/sub combines.
            Row i=2 goes to GPSIMD for alternating products (ENGINE_MIX)."""
            assert not (adag and bdag)
            _pc[0] += 1
            use_gp = ENGINE_MIX
            w = (s1 - s0) * SL
            for i in range(3):
                E, xt, xs = GP_SET if (use_gp and i == 2) else DV_SET
                tv = tview2(xt, nj, 3, w)      # [p,2,nj,3,w]
                sv = sview2(xs, nj, w)         # [p,2,nj,w]
                ar = a_pack(A, i, adag, 0, s0, s1, adt, nj)
                ai = a_pack(A, i, adag, 1, s0, s1, adt, nj)
                br = b_pack(B, bdag, 0, s0, s1, bdt, nj)
                bi = b_pack(B, bdag, 1, s0, s1, bdt, nj)
                dre = dst.gap(i * nj, 1, nj, 0, s0, s1)
                dim = dst.gap(i * nj, 1, nj, 1, s0, s1)
                # real: Srr - sa*sb*Sii
                E.tensor_mul(tv[:, 0], ar, br)
                E.tensor_mul(tv[:, 1], ai, bi)
                E.tensor_add(sv, tv[:, :, :, 0, :], tv[:, :, :, 1, :])
                E.tensor_add(sv, sv, tv[:, :, :, 2, :])
                if adag or bdag:
                    E.tensor_add(dre, sv[:, 0], sv[:, 1])
                else:
                    E.tensor_sub(dre, sv[:, 0], sv[:, 1])
                # imag: sb*Sri + sa*Sir
                E.tensor_mul(tv[:, 0], ar, bi)
                E.tensor_mul(tv[:, 1], ai, br)
                E.tensor_add(sv, tv[:, :, :, 0, :], tv[:, :, :, 1, :])
                E.tensor_add(sv, sv, tv[:, :, :, 2, :])
                if adag:
                    E.tensor_sub(dim, sv[:, 0], sv[:, 1])
                elif bdag:
                    E.tensor_sub(dim, sv[:, 1], sv[:, 0])
                else:
                    E.tensor_add(dim, sv[:, 0], sv[:, 1])

        # ---------- shift materialization ----------
        def xview(g, s0, s1):
            c0, c1 = g.cs(s0, s1)
            nb = (c1 - c0) // 16
            v = g.t.rearrange("p (q b x) -> p q b x", q=g.nent * 2,
                              b=g.ns * 2, x=16)
            return v[:, :, (c0 // 16):(c0 // 16) + nb, :]

        def cpy(out, in_):
            if ENGINE_MIX:
                nc.scalar.copy(out, in_)
            else:
                V.tensor_copy(out, in_)

        def mat_shift(src, axis, d, s0, s1, tag, pool=None):
            """materialize S(x)=src(x + d*e_axis) over the FULL src range
            (so DMA src/dst APs share structure). axis 1,2,3."""
            g = Grp(mk(pool or main, src.nent * 2 * src.ns * SL, tag),
                    src.nent, src.base, src.ns)
            sb, se = src.base, src.base + src.ns
            if axis == 3:   # X
                def xv(t_):
                    return t_.rearrange("p (m x) -> p m x", x=16)
                dv, sv_ = xv(g.t), xv(src.t)
                if d == +1:
                    cpy(dv[:, :, 0:15], sv_[:, :, 1:16])
                    cpy(dv[:, :, 15:16], sv_[:, :, 0:1])
                else:
                    cpy(dv[:, :, 1:16], sv_[:, :, 0:15])
                    cpy(dv[:, :, 0:1], sv_[:, :, 15:16])
            elif axis == 1:  # Z: partition +-8
                sall, dall = src.t[:, :], g.t[:, :]
                if d == +1:
                    nc.sync.dma_start(out=dall[0:120], in_=sall[8:128])
                    nc.sync.dma_start(out=dall[120:128], in_=sall[0:8])
                else:
                    nc.sync.dma_start(out=dall[8:128], in_=sall[0:120])
                    nc.sync.dma_start(out=dall[0:8], in_=sall[120:128])
            else:           # Y
                def lv(t_, lo):
                    return t_.rearrange("p (m x) -> p m x", x=16)[:, lo::2, :]
                if d == +1:
                    # lo=0 out <- lo=1 in (same p)
                    cpy(lv(g.t, 0), lv(src.t, 1))
                    # lo=1 out <- lo=0 in at p+1; wrap h=7 <- h=0 same z
                    nc.sync.dma_start(out=lv(g.t[0:127, :], 1),
                                      in_=lv(src.t[1:128, :], 0))
                    if SIM_SAFE:
                        for z in range(16):
                            nc.sync.dma_start(
                                out=lv(g.t[z * 8 + 7:z * 8 + 8, :], 1),
                                in_=lv(src.t[z * 8:z * 8 + 1, :], 0))
                    else:
                        nc.sync.dma_start(out=lv(g.t[7:128:8, :], 1),
                                          in_=lv(src.t[0:128:8, :], 0))
                else:
                    cpy(lv(g.t, 1), lv(src.t, 0))
                    nc.sync.dma_start(out=lv(g.t[1:128, :], 0),
                                      in_=lv(src.t[0:127, :], 1))
                    if SIM_SAFE:
                        for z in range(16):
                            nc.sync.dma_start(
                                out=lv(g.t[z * 8:z * 8 + 1, :], 0),
                                in_=lv(src.t[z * 8 + 7:z * 8 + 8, :], 1))
                    else:
                        nc.sync.dma_start(out=lv(g.t[0:128:8, :], 0),
                                          in_=lv(src.t[7:128:8, :], 1))
            return g

        # ---------- mass term: O = (MASS+4) * F ----------
        V.tensor_scalar_mul(O.all_ap(1, 5), F.all_ap(1, 5), float(MASS + 4.0))

        # ---------- Wilson hops ----------
        with tc.tile_pool(name="wp", bufs=2) as wpool:
            for k in range(4):
                ax = k  # lattice axis
                Uk = load_dir(k, "glA")
                # ---- forward hop ----
                rng = (0, 4) if k == 0 else (1, 5)
                s0, s1 = rng
                h = Grp(mk(wpool, 12 * (s1 - s0) * SL, "h"),
                        6, s0, s1 - s0)
                # h[c,j] = psi[c,j] + g(j)*psi[c,p(j)]
                for j in range(2):
                    pj, gj = GCOL[k][j]
                    for r in range(2):
                        pr, psgn = (_phase_parts(gj)[0:2] if r == 0
                                    else _phase_parts(gj)[2:4])
                        dst = h.gap(j, 2, 3, r, s0, s1)       # c-packed
                        a0 = F.gap(j, 4, 3, r, s0, s1)        # psi[c,j].r
                        a1 = F.gap(pj, 4, 3, pr, s0, s1)
                        stt(dst, a1, psgn, a0, AL.add)
                phi = Grp(mk(wpool, 12 * (s1 - s0) * SL, "phi"),
                          6, s0, s1 - s0)
                cmm(phi, Uk, h, s0, s1, adag=True, nj=2)
                # shift (-1 along ax) then reconstruct into O
                if k == 0:
                    psh, dt = phi, -1
                else:
                    psh, dt = mat_shift(phi, ax, -1, 1, 5, "psh", wpool), 0
                # out[:, s'] += -0.5 * rho(s') * psh[:, q(s')]
                rec = [(0, 1.0), (1, 1.0), PROJ_F[k][0], PROJ_F[k][1]]
                for sp in range(4):
                    q, rho = rec[sp]
                    for r in range(2):
                        pr, psgn = (_phase_parts(rho)[0:2] if r == 0
                                    else _phase_parts(rho)[2:4])
                        dst = O.gap(sp, 4, 3, r, 1, 5)
                        src_ = psh.gap(q, 2, 3, pr, 1 + dt, 5 + dt)
                        stt(dst, src_, -0.5 * psgn, dst, AL.add)
                # ---- backward hop ----
                rng = (2, 6) if k == 0 else (1, 5)
                s0, s1 = rng
                hb = Grp(mk(wpool, 12 * (s1 - s0) * SL, "h"),
                         6, s0, s1 - s0)
                # hb[c,j] = g(j)*psi[c,p(j)] - psi[c,j]
                for j in range(2):
                    pj, gj = GCOL[k][j]
                    for r in range(2):
                        pr, psgn = (_phase_parts(gj)[0:2] if r == 0
                                    else _phase_parts(gj)[2:4])
                        dst = hb.gap(j, 2, 3, r, s0, s1)
                        a1 = F.gap(pj, 4, 3, pr, s0, s1)
                        a0 = F.gap(j, 4, 3, r, s0, s1)
                        # dst = psgn*a1 - a0  -> stt: (a0 * -1) + ...? need scaled a1.
                        # use: dst = (a1*psgn) + (-a0): two-step via subtract:
                        # dst = (a1 * psgn) sub? op1 options: use subtract_rev?
                        # simplest: dst = (a1*psgn) + a0*(-1): do STT then sub.
                        stt(dst, a1, psgn, a0, AL.subtract)
                if k == 0:
                    hs, dt = hb, +1
                else:
                    hs, dt = mat_shift(hb, ax, +1, 1, 5, "psh", wpool), 0
                gm = Grp(mk(wpool, 12 * 4 * SL, "phi"),
                         6, 1, 4)
                cmm(gm, Uk, hs, 1, 5, nj=2, bdt=dt)
                rec = [(0, 1.0), (1, 1.0), PROJ_B[k][0], PROJ_B[k][1]]
                for sp in range(4):
                    q, rho = rec[sp]
                    for r in range(2):
                        pr, psgn = (_phase_parts(rho)[0:2] if r == 0
                                    else _phase_parts(rho)[2:4])
                        dst = O.gap(sp, 4, 3, r, 1, 5)
                        src_ = gm.gap(q, 2, 3, pr, 1, 5)
                        stt(dst, src_, 0.5 * psgn, dst, AL.add)

        # ---------- clover planes ----------
        with tc.tile_pool(name="cp", bufs=2) as cpool:
            def ctile(tag, nent, s0, s1):
                return Grp(mk(cpool, nent * 2 * (s1 - s0) * SL, tag), nent, s0, s1 - s0)

            for d1 in range(1, 5):
                for d2 in range(d1 + 1, 5):
                    a1, a2 = d1 - 1, d2 - 1
                    tpl = (d1 == 1)
                    sA, eA = (0, 5) if tpl else (1, 5)
                    U1 = load_dir(d1 - 1, "glA")
                    U2 = load_dir(d2 - 1, "glB")
                    # shifted links
                    if tpl:
                        U2m, u2dt = U2, +1       # U_d2(x+e_T): column view
                    else:
                        U2m, u2dt = mat_shift(U2, a1, +1, sA, eA, "lnk1", cpool), 0
                    U1n = mat_shift(U1, a2, +1, sA, eA, "lnk2", cpool)
                    A = ctile("pA", 9, sA, eA)
                    cmm(A, U1, U2m, sA, eA, bdt=u2dt)
                    B = ctile("pB", 9, sA, eA)
                    cmm(B, U2, U1n, sA, eA)
                    Q = ctile("pQ", 9, 1, 5)
                    cmm(Q, B, A, 1, 5, bdag=True)            # L4
                    sL1, eL1 = (0, 4) if tpl else (1, 5)
                    L1 = ctile("pL", 9, sL1, eL1)
                    cmm(L1, A, B, sL1, eL1, adag=True)
                    # L1 shift chain overlaps D/E products; Q-adds deferred
                    if tpl:
                        L1s = mat_shift(L1, a2, -1, 0, 4, "pLs", cpool)
                        l1_ap = L1s.all_ap(0, 4)
                    else:
                        L1s = mat_shift(L1, a1, -1, 1, 5, "pLs", cpool)
                        L1ss = mat_shift(L1s, a2, -1, 1, 5, "pLss", cpool)
                        l1_ap = L1ss.all_ap(1, 5)
                    D = ctile("pA", 9, sA, eA)
                    cmm(D, U1n, U2m, sA, eA, bdag=True, bdt=u2dt)
                    E = ctile("pB", 9, sA, eA)
                    cmm(E, U2, U1, sA, eA, adag=True)
                    V.tensor_add(Q.all_ap(1, 5), Q.all_ap(1, 5), l1_ap)
                    L2 = ctile("pL2", 9, 1, 5)
                    cmm(L2, D, E, 1, 5, bdag=True)
                    L2s = mat_shift(L2, a2, -1, 1, 5, "pL2s", cpool)
                    sL3, eL3 = (0, 4) if tpl else (1, 5)
                    L3 = ctile("pL", 9, sL3, eL3)
                    cmm(L3, E, D, sL3, eL3, adag=True)
                    V.tensor_add(Q.all_ap(1, 5), Q.all_ap(1, 5),
                                 L2s.all_ap(1, 5))
                    if tpl:
                        V.tensor_add(Q.all_ap(1, 5), Q.all_ap(1, 5),
                                     L3.all_ap(0, 4))
                    else:
                        L3s = mat_shift(L3, a1, -1, 1, 5, "pLs", cpool)
                        V.tensor_add(Q.all_ap(1, 5), Q.all_ap(1, 5),
                                     L3s.all_ap(1, 5))
                    # ---- G9 = Q - Q^dag ----
                    G9 = ctile("pG9", 9, 1, 5)
                    qv = Q.t.rearrange("p (i j r w) -> p i j r w",
                                       i=3, j=3, r=2, w=Q.W)
                    qT = qv.transpose([0, 2, 1, 3, 4])
                    gv = G9.t.rearrange("p (i j r w) -> p i j r w",
                                        i=3, j=3, r=2, w=G9.W)
                    V.tensor_sub(gv[:, :, :, 0, :], qv[:, :, :, 0, :],
                                 qT[:, :, :, 0, :])
                    V.tensor_add(gv[:, :, :, 1, :], qv[:, :, :, 1, :],
                                 qT[:, :, :, 1, :])
                    # ---- apply: O -= 1/16 * G9 * (psi sigma) ----
                    w4 = 4 * SL
                    tgv = tview2(mtA, 3, 3, w4)
                    tg1, tg2 = tgv[:, 0], tgv[:, 1]
                    sv4 = sview2(msA, 3, w4)[:, 0]
                    for sp in range(4):
                        wrow, phi_ph = SIGCOL[a1][a2][sp]
                        vr_p, vr_s, vi_p, vi_s = _phase_parts(phi_ph)
                        # v.re plane = vr_s * psi[j, wrow].(vr_p)
                        for outr in range(2):
                            # out.re += sum_j gr*v.re - gi*v.im
                            # out.im += sum_j gr*v.im + gi*v.re
                            if outr == 0:
                                pa, sa_ = vr_p, vr_s
                                pb, sb_ = vi_p, -vi_s
                            else:
                                pa, sa_ = vi_p, vi_s
                                pb, sb_ = vr_p, vr_s
                            # psi plane for color j: entry j*4+wrow, part pa;
                            # broadcast over i (t layout [p, i, j, w])
                            psA = F.gap(wrow, 4, 3, pa, 1, 5)   # [p,3(j),w]
                            psB = F.gap(wrow, 4, 3, pb, 1, 5)
                            psA4 = psA.unsqueeze(1).broadcast_to((128, 3, 3, w4))
                            psB4 = psB.unsqueeze(1).broadcast_to((128, 3, 3, w4))
                            V.tensor_mul(tg1, gv[:, :, :, 0, :], psA4)
                            V.tensor_mul(tg2, gv[:, :, :, 1, :], psB4)
                            # d = sa_*t1 + sb_*t2 ; reduce over j ; O += -1/16*d
                            if sa_ * sb_ > 0:
                                V.tensor_add(tg1, tg1, tg2)
                            else:
                                V.tensor_sub(tg1, tg1, tg2)
                            V.tensor_add(sv4, tg1[:, :, 0, :], tg1[:, :, 1, :])
                            V.tensor_add(sv4, sv4, tg1[:, :, 2, :])
                            dst = O.gap(sp, 4, 3, outr, 1, 5)
                            stt(dst, sv4, -(1.0 / 16.0) * sa_, dst, AL.add)

        nc.sync.dma_start(out=out_d[:, :], in_=O.t[:, :])
        spool_cm.__exit__(None, None, None)
        gpool_cm.__exit__(None, None, None)
        main_cm.__exit__(None, None, None)

    nc.compile()
    return nc


# ----------------------------------------------------------------------
# host entry
# ----------------------------------------------------------------------
_CACHE = {}

def _get_nc():
    if "nc" not in _CACHE:
        _CACHE["nc"] = build_program()
    return _CACHE["nc"]


def kernel(field_re, field_im, gauge_re, gauge_im):
    from concourse.bass_utils import run_bass_kernel_spmd
    nc = _get_nc()
    in_maps = [_host_inputs(field_re, field_im, gauge_re, gauge_im, c)
               for c in range(NCORES)]
    br = run_bass_kernel_spmd(nc, in_maps, list(range(NCORES)))
    out = np.empty((T_, Z_, Y_, X_, 3, 4, 2), np.float32)
    for c in range(NCORES):
        out[c * TLOC:(c + 1) * TLOC] = _host_output(br.results[c]["outp"])
    return out



# revision 22
# speedup vs baseline: 1.8320x; 1.8320x over previous
"""Wilson-clover Dirac operator D_WC on Trainium2, 8-core SPMD.

Self-contained: hardcodes LAT=(32,16,16,16), shards the T axis across 8
cores with host-side halo slices (t0-1..t0+4), computes everything
site-locally on-device with DVE elementwise math in an SoA layout:

  plane[p, col]: p = z*8 + (y>>1),  col = plane_idx*W + (s-base)*32
                 + (y&1)*16 + x          (s = local T-slice, W = ns*32)

Shifts: T = column offset (free), X = 2-piece ScalarE copy, Z/Y =
TensorE permutation matmul (stationary 128x128 0/1 matrix) + ScalarE
copyback from PSUM.  All elementwise arithmetic runs on the DVE only
(GPSIMD shares the DVE SBUF port; running both concurrently is a net
loss).  Gauge links for all 4 directions are DMA'd into SBUF once.
"""
import numpy as np

# ----------------------------------------------------------------------
# constants (physics)
# ----------------------------------------------------------------------
MASS, C_SW = 0.5, 1.0
_i = 1j
G1 = np.array([[0,0,0,_i],[0,0,_i,0],[0,-_i,0,0],[-_i,0,0,0]], dtype=np.complex64)
G2 = np.array([[0,0,0,-1],[0,0,1,0],[0,1,0,0],[-1,0,0,0]], dtype=np.complex64)
G3 = np.array([[0,0,_i,0],[0,0,0,-_i],[-_i,0,0,0],[0,_i,0,0]], dtype=np.complex64)
G4 = np.array([[0,0,1,0],[0,0,0,1],[1,0,0,0],[0,1,0,0]], dtype=np.complex64)
GAMMA = [G1, G2, G3, G4]
I4 = np.eye(4, dtype=np.complex64)
SIGMA = [[(0.5j * (GAMMA[m] @ GAMMA[n] - GAMMA[n] @ GAMMA[m])).astype(np.complex64)
          for n in range(4)] for m in range(4)]

T_, Z_, Y_, X_ = 32, 16, 16, 16
NCORES, TLOC, NSH = 8, 4, 6       # halo slices per core
SL = 32                           # cols per slice
USE_BF16 = True                   # bf16 compute tiles (O stays fp32)
import os as _os
DBG_PART = _os.environ.get("KRN_PART", "full")    # full|wilson|clover
DBG_PAIRS = _os.environ.get("KRN_PAIRS", "")      # e.g. "12,34" to keep only
DBG_HOPS = _os.environ.get("KRN_HOPS", "0123")    # wilson dirs to keep

# permutation matrix indices (stationary operands for TensorE shifts)
PZ_P, PZ_M, PY_P, PY_M, PZY_M = 0, 1, 2, 3, 4
NPERM = 5

# ----------------------------------------------------------------------
# spin-structure extraction (numerical)
# ----------------------------------------------------------------------
def _col_struct(M):
    """M has single-nonzero columns: return per-col (row, phase)."""
    out = []
    for s in range(4):
        col = M[:, s]
        r = int(np.argmax(np.abs(col)))
        ph = complex(col[r])
        assert np.sum(np.abs(col) > 1e-6) == 1, (M, s)
        out.append((r, ph))
    return out

def _proj_struct(P):
    """P rank-2 with cols 2,3 = rho * cols q in {0,1}. Returns (q2,rho2,q3,rho3)."""
    res = []
    for s in (2, 3):
        found = None
        for j in (0, 1):
            c, cj = P[:, s], P[:, j]
            nz = np.abs(cj) > 1e-6
            if not nz.any():
                continue
            ratio = c[nz] / cj[nz]
            if np.allclose(ratio, ratio[0], atol=1e-5) and np.allclose(
                    c, ratio[0] * cj, atol=1e-5):
                found = (j, complex(ratio[0]))
                break
        assert found is not None, P
        res.append(found)
    return res

# per direction k: gamma column structure and projector relations
GCOL = [_col_struct(GAMMA[k]) for k in range(4)]           # (p(s), g(s))
PROJ_F = [_proj_struct(GAMMA[k] + I4) for k in range(4)]   # for psi@(G+I)
PROJ_B = [_proj_struct(GAMMA[k] - I4) for k in range(4)]   # for psi@(G-I)
SIGCOL = [[_col_struct(SIGMA[m][n]) if m != n else None for n in range(4)]
          for m in range(4)]

def _phase_parts(ph):
    """phase in {1,-1,i,-i} -> for v = ph*u:
    re(v) = sr*u.[pr]  im(v) = si*u.[pi]   (0=re,1=im planes of u)"""
    if abs(ph - 1) < 1e-5:   return (0, 1.0, 1, 1.0)
    if abs(ph + 1) < 1e-5:   return (0, -1.0, 1, -1.0)
    if abs(ph - 1j) < 1e-5:  return (1, -1.0, 0, 1.0)
    if abs(ph + 1j) < 1e-5:  return (1, 1.0, 0, -1.0)
    raise AssertionError(ph)

# ----------------------------------------------------------------------
# host layout helpers
# ----------------------------------------------------------------------
def _to_planes(vol):
    """vol [ns, Z, Y, X] -> [128, ns*32]"""
    ns = vol.shape[0]
    v = vol.reshape(ns, 16, 8, 2, 16)        # s z yh yl x
    v = np.transpose(v, (1, 2, 0, 3, 4))     # z yh s yl x
    return np.ascontiguousarray(v.reshape(128, ns * 32))

def _from_planes(pl, ns):
    v = pl.reshape(16, 8, ns, 2, 16)
    return np.transpose(v, (2, 0, 1, 3, 4)).reshape(ns, 16, 16, 16)

def _perm_z(d):
    """dst partition m = z*8+yh reads p = ((z+d)%16)*8+yh; P[p, m] = 1."""
    P = np.zeros((128, 128), np.float32)
    for z in range(16):
        for yh in range(8):
            P[((z + d) % 16) * 8 + yh, z * 8 + yh] = 1.0
    return P

def _perm_y(d, dz=0):
    """Y cross-half: dst m = z*8+yh reads p = ((z+dz)%16)*8+((yh+d)%8)."""
    P = np.zeros((128, 128), np.float32)
    for z in range(16):
        for yh in range(8):
            P[((z + dz) % 16) * 8 + ((yh + d) % 8), z * 8 + yh] = 1.0
    return P

def _perm_host():
    P = np.concatenate([_perm_z(+1), _perm_z(-1), _perm_y(+1), _perm_y(-1),
                        _perm_y(-1, dz=-1)], axis=1)  # [128, NPERM*128]
    return P

def _host_inputs(field_re, field_im, gauge_re, gauge_im, core):
    t0 = core * TLOC
    ts = [(t0 - 1 + s) % T_ for s in range(NSH)]
    f = np.stack([field_re[ts], field_im[ts]], axis=0)   # [2,6,Z,Y,X,3,4]
    g = np.stack([gauge_re[:, ts], gauge_im[:, ts]], axis=0)  # [2,4,6,Z,Y,X,3,3]

    # field planes: e=(c*4+sp), plane=e*2+r
    fp = np.empty((24, 128, NSH * SL), np.float32)
    for c in range(3):
        for sp in range(4):
            for r in range(2):
                fp[(c * 4 + sp) * 2 + r] = _to_planes(f[r, :, :, :, :, c, sp])
    fld = np.ascontiguousarray(fp.transpose(1, 0, 2).reshape(128, 24 * NSH * SL))

    # gauge planes: dir k, e=(a*3+b), plane=(k*9+e)*2+r
    gp = np.empty((72, 128, NSH * SL), np.float32)
    for k in range(4):
        for a in range(3):
            for b in range(3):
                for r in range(2):
                    gp[(k * 9 + a * 3 + b) * 2 + r] = _to_planes(
                        g[r, k, :, :, :, :, a, b])
    gg = np.ascontiguousarray(gp.transpose(1, 0, 2).reshape(128, 72 * NSH * SL))
    perm = _perm_host()
    if USE_BF16:
        import ml_dtypes
        fld = fld.astype(ml_dtypes.bfloat16)
        gg = gg.astype(ml_dtypes.bfloat16)
        perm = perm.astype(ml_dtypes.bfloat16)
    return {"fld": fld, "gg": gg, "perm": perm}

def _host_output(outp_flat):
    """device out [128, 24*4*32] -> [TLOC, Z,Y,X, 3,4,2]"""
    pl = outp_flat.reshape(128, 24, TLOC * SL).transpose(1, 0, 2)
    out = np.empty((TLOC, 16, 16, 16, 3, 4, 2), np.float32)
    for c in range(3):
        for sp in range(4):
            for r in range(2):
                out[..., c, sp, r] = _from_planes(pl[(c * 4 + sp) * 2 + r], TLOC)
    return out

# ----------------------------------------------------------------------
# device program
# ----------------------------------------------------------------------
class Grp:
    """Group of planes in one SBUF tile. nent complex entries (re+im planes).
    base = slice index of col 0; ns slices; W = ns*32 cols per plane."""
    def __init__(self, tile, nent, base, ns):
        self.t, self.nent, self.base, self.ns = tile, nent, base, ns
        self.W = ns * SL

    def fl(self):
        return self.t.rearrange("p (q w) -> p q w", q=self.nent * 2, w=self.W)

    def cs(self, s0, s1):
        return ((s0 - self.base) * SL, (s1 - self.base) * SL)

    def pap(self, e, r, s0, s1):
        """single plane AP [128, cols]"""
        c0, c1 = self.cs(s0, s1)
        v = self.fl()[:, (e * 2 + r):(e * 2 + r + 1), c0:c1]
        return v  # [p,1,w]

    def gap(self, e0, estep, n, r, s0, s1):
        """packed-entry AP [p, n, w]: entries e0 + i*estep, fixed r."""
        c0, c1 = self.cs(s0, s1)
        q0 = e0 * 2 + r
        fl = self.fl()
        return fl[:, q0:q0 + 2 * estep * (n - 1) + 1:2 * estep, c0:c1]

    def all_ap(self, s0, s1):
        c0, c1 = self.cs(s0, s1)
        return self.fl()[:, :, c0:c1]


def build_program():
    import concourse.bacc as bacc
    import concourse.mybir as mybir
    from concourse.tile import TileContext
    FP = mybir.dt.float32
    CDT = mybir.dt.bfloat16 if USE_BF16 else FP
    AL = mybir.AluOpType

    nc = bacc.Bacc("TRN2", target_bir_lowering=False, debug=False)
    fld_d = nc.declare_dram_parameter("fld", [128, 24 * NSH * SL], CDT, isOutput=False)
    gg_d = nc.declare_dram_parameter("gg", [128, 72 * NSH * SL], CDT, isOutput=False)
    perm_d = nc.declare_dram_parameter("perm", [128, NPERM * 128], CDT, isOutput=False)
    out_d = nc.declare_dram_parameter("outp", [128, 24 * TLOC * SL], FP, isOutput=True)

    with TileContext(nc) as tc:
        _tc = [0]

        def mk(pool, cols, tag, dt=None):
            _tc[0] += 1
            return pool.tile([128, cols], dt or CDT, tag=tag,
                             name=f"{tag}_{_tc[0]}")

        main_cm = tc.tile_pool(name="main", bufs=1)
        main = main_cm.__enter__()
        spool_cm = tc.tile_pool(name="sp", bufs=1)
        spool = spool_cm.__enter__()
        psum_cm = tc.psum_pool(name="ps", bufs=4)
        psum = psum_cm.__enter__()

        F = Grp(mk(main, 24 * NSH * SL, "F"), 12, 0, NSH)
        O = Grp(mk(main, 24 * TLOC * SL, "O", FP), 12, 1, TLOC)
        PM = mk(main, NPERM * 128, "PM")
        nc.sync.dma_start(out=F.t[:, :], in_=fld_d[:, :])
        nc.sync.dma_start(out=PM[:, :], in_=perm_d[:, :])

        # gauge links for all 4 directions, loaded once
        GL = []
        for k in range(4):
            g = Grp(mk(main, 9 * 2 * NSH * SL, f"GL{k}"), 9, 0, NSH)
            eng = nc.sync if k % 2 == 0 else nc.scalar
            eng.dma_start(out=g.t[:, :],
                          in_=gg_d[:, k * 18 * NSH * SL:(k + 1) * 18 * NSH * SL])
            GL.append(g)

        # merged scratch tiles (4 product slots each)
        WMX = 5 * SL
        mtA = mk(spool, 4 * 9 * WMX, "mtA")
        msA = mk(spool, 4 * 3 * WMX, "msA")

        def tview4(t, nj, nk, w):
            """[p, 4, nj, nk, w] slots of merged mul scratch"""
            v = t.rearrange("p (h m) -> p h m", h=4)
            return v[:, :, :nj * nk * w].rearrange(
                "p h (j k w) -> p h j k w", j=nj, k=nk, w=w)

        def sview4(t, nj, w):
            v = t.rearrange("p (h m) -> p h m", h=4)
            return v[:, :, :nj * w].rearrange("p h (j w) -> p h j w",
                                              j=nj, w=w)

        V = nc.vector

        def a_pack(A, i, adag, r, s0, s1, dt, nj):
            """[p, nj(bcast), 3, w] for a-values (i,k)."""
            e0, st = (i, 3) if adag else (i * 3, 1)
            ap = A.gap(e0, st, 3, r, s0 + dt, s1 + dt)       # [p,3,w]
            w = ap.shape[2]
            return ap.unsqueeze(1).broadcast_to((128, nj, 3, w))

        def b_pack(B, bdag, r, s0, s1, dt, nj):
            """[p, nj, 3, w] for b-values (k,j)."""
            c0, c1 = B.cs(s0 + dt, s1 + dt)
            w = c1 - c0
            if bdag:  # e = j*3+k
                v = B.t.rearrange("p (j k r w) -> p j k r w", j=3, k=3, r=2, w=B.W)
                return v[:, :, :, r, c0:c1]
            if B.nent == 9:  # e = k*3+j
                v = B.t.rearrange("p (k j r w) -> p k j r w", k=3, j=3, r=2, w=B.W)
                return v[:, :, :, r, c0:c1].transpose([0, 2, 1, 3])
            # halfspinor: e = k*2+j, nj=2
            v = B.t.rearrange("p (k j r w) -> p k j r w", k=3, j=2, r=2, w=B.W)
            return v[:, :, :, r, c0:c1].transpose([0, 2, 1, 3])

        def stt(out, in0, coef, in1, op1=None):
            V.scalar_tensor_tensor(out, in0, float(coef), in1,
                                   AL.mult, op1 or AL.add)

        def cmm(dst, A, B, s0, s1, adag=False, bdag=False, adt=0, bdt=0, nj=3):
            """dst[i,j] = sum_k aval(i,k)*bval(k,j); dst entries e=i*nj+j.
            No (adag and bdag) case: signs reduce to add/sub combines."""
            assert not (adag and bdag)
            w = (s1 - s0) * SL
            for i in range(3):
                tv = tview4(mtA, nj, 3, w)     # [p,4,nj,3,w]
                sv = sview4(msA, nj, w)        # [p,4,nj,w]
                ar = a_pack(A, i, adag, 0, s0, s1, adt, nj)
                ai = a_pack(A, i, adag, 1, s0, s1, adt, nj)
                br = b_pack(B, bdag, 0, s0, s1, bdt, nj)
                bi = b_pack(B, bdag, 1, s0, s1, bdt, nj)
                dre = dst.gap(i * nj, 1, nj, 0, s0, s1)
                dim = dst.gap(i * nj, 1, nj, 1, s0, s1)
                # slots: 0=Srr 1=Sii 2=Sri 3=Sir
                V.tensor_mul(tv[:, 0], ar, br)
                V.tensor_mul(tv[:, 1], ai, bi)
                V.tensor_mul(tv[:, 2], ar, bi)
                V.tensor_mul(tv[:, 3], ai, br)
                V.tensor_add(sv, tv[:, :, :, 0, :], tv[:, :, :, 1, :])
                V.tensor_add(sv, sv, tv[:, :, :, 2, :])
                if adag or bdag:
                    V.tensor_add(dre, sv[:, 0], sv[:, 1])
                else:
                    V.tensor_sub(dre, sv[:, 0], sv[:, 1])
                if adag:
                    V.tensor_sub(dim, sv[:, 2], sv[:, 3])
                elif bdag:
                    V.tensor_sub(dim, sv[:, 3], sv[:, 2])
                else:
                    V.tensor_add(dim, sv[:, 2], sv[:, 3])

        # ---------- shift materialization ----------
        def cpy(out, in_):
            nc.scalar.copy(out, in_)

        def perm_ap(idx):
            return PM.rearrange("p (i m) -> p i m", i=NPERM)[:, idx, :]

        def mm_permute(dst_t, src_t, cols, pidx, dhalf=None, shalf=None,
                       xrot=False):
            """dst = P[pidx] applied to src partitions, over [0, cols).
            dhalf/shalf: copy back only dst yl==dhalf 16-col blocks, sourced
            from permuted yl==shalf blocks.
            xrot: additionally rotate X by -1 (dst x reads src x-1)."""
            P = perm_ap(pidx)
            c0 = 0
            while c0 < cols:
                cw = min(512, cols - c0)
                pt = psum.tile([128, cw], FP, tag="pshift",
                               name=f"ps_{_tc[0]}_{c0}")
                _tc[0] += 1
                nc.tensor.matmul(pt[:, :], P, src_t[:, c0:c0 + cw])
                dv = dst_t[:, c0:c0 + cw]
                if dhalf is None and not xrot:
                    cpy(dv, pt[:, :])
                elif dhalf is None and xrot:
                    d3 = dv.rearrange("p (b x) -> p b x", x=16)
                    p3 = pt.rearrange("p (b x) -> p b x", x=16)
                    cpy(d3[:, :, 1:16], p3[:, :, 0:15])
                    cpy(d3[:, :, 0:1], p3[:, :, 15:16])
                else:
                    d4 = dv.rearrange("p (b l x) -> p b l x", l=2, x=16)
                    p4 = pt.rearrange("p (b l x) -> p b l x", l=2, x=16)
                    if not xrot:
                        cpy(d4[:, :, dhalf, :], p4[:, :, shalf, :])
                    else:
                        cpy(d4[:, :, dhalf, 1:16], p4[:, :, shalf, 0:15])
                        cpy(d4[:, :, dhalf, 0:1], p4[:, :, shalf, 15:16])
                c0 += cw

        def lv(t_, lo):
            """yl==lo 16-col blocks of flat tile view"""
            return t_.rearrange("p (m x) -> p m x", x=16)[:, lo::2, :]

        def mat_shift(src, axis, d, s0, s1, tag, pool=None):
            """materialize S(x)=src(x + d*e_axis) over the FULL src range.
            axis 1(Z) 2(Y) 3(X)."""
            g = Grp(mk(pool or main, src.nent * 2 * src.ns * SL, tag),
                    src.nent, src.base, src.ns)
            cols = src.nent * 2 * src.ns * SL
            if axis == 3:   # X: 2-piece ScalarE copy
                def xv(t_):
                    return t_.rearrange("p (m x) -> p m x", x=16)
                dv, sv_ = xv(g.t), xv(src.t)
                if d == +1:
                    cpy(dv[:, :, 0:15], sv_[:, :, 1:16])
                    cpy(dv[:, :, 15:16], sv_[:, :, 0:1])
                else:
                    cpy(dv[:, :, 1:16], sv_[:, :, 0:15])
                    cpy(dv[:, :, 0:1], sv_[:, :, 15:16])
            elif axis == 1:  # Z: pure partition permutation
                mm_permute(g.t, src.t, cols, PZ_P if d == +1 else PZ_M)
            else:           # Y: in-partition half + permuted half
                if d == +1:
                    cpy(lv(g.t, 0), lv(src.t, 1))
                    mm_permute(g.t, src.t, cols, PY_P, dhalf=1, shalf=0)
                else:
                    cpy(lv(g.t, 1), lv(src.t, 0))
                    mm_permute(g.t, src.t, cols, PY_M, dhalf=0, shalf=1)
            return g

        def mat_shift_diag(src, a1, a2, tag, pool=None):
            """S(x) = src(x - e_a1 - e_a2), a1 < a2 spatial axes (1,2,3)."""
            g = Grp(mk(pool or main, src.nent * 2 * src.ns * SL, tag),
                    src.nent, src.base, src.ns)
            cols = src.nent * 2 * src.ns * SL
            if (a1, a2) == (1, 2):    # Z,Y
                mm_permute(g.t, src.t, cols, PZ_M, dhalf=1, shalf=0)
                mm_permute(g.t, src.t, cols, PZY_M, dhalf=0, shalf=1)
            elif (a1, a2) == (1, 3):  # Z,X
                mm_permute(g.t, src.t, cols, PZ_M, xrot=True)
            else:                     # Y,X
                dv = g.t.rearrange("p (m x) -> p m x", x=16)[:, 1::2, :]
                sv_ = src.t.rearrange("p (m x) -> p m x", x=16)[:, 0::2, :]
                cpy(dv[:, :, 1:16], sv_[:, :, 0:15])
                cpy(dv[:, :, 0:1], sv_[:, :, 15:16])
                mm_permute(g.t, src.t, cols, PY_M, dhalf=0, shalf=1, xrot=True)
            return g

        # ---------- mass term: O = (MASS+4) * F ----------
        V.tensor_scalar_mul(O.all_ap(1, 5), F.all_ap(1, 5), float(MASS + 4.0))

        # clover chiral blocks W[(i,sp),(j,s')] = sum_p -(1/16) ph_p(sp)
        #   * G9_p[i,j] with s' = wrow_p(sp);  plane index =
        #   chir*72 + spl*36 + sl*18 + i*6 + j*2 + r
        W4 = 4 * SL
        WCH = mk(main, 144 * W4, "WCH")
        wv3 = WCH.rearrange("p (q w) -> p q w", q=144, w=W4)
        wv6 = WCH.rearrange("p (c s l m w) -> p c s l m w",
                            c=2, s=2, l=2, m=18, w=W4)

        def wslot(chir, spl, sl, r):
            """[p, (i,j)(9), w4] view of one spin-slot of W (3D for stt)"""
            base = chir * 72 + spl * 36 + sl * 18 + r
            return wv3[:, base:base + 17:2, :]

        def wrow_ap(chir, i, spl, r):
            """[p, j(3), sl(2), w4] contraction view of one W row"""
            m0 = i * 6 + r
            return wv6[:, chir, spl, :, m0:m0 + 5:2, :].transpose([0, 2, 1, 3])

        # ---------- Wilson hops ----------
        with tc.tile_pool(name="wp", bufs=2) as wpool:
            for k in (range(4) if DBG_PART in ("full", "wilson") else []):
                if str(k) not in DBG_HOPS:
                    continue
                ax = k  # lattice axis
                Uk = GL[k]
                # ---- forward hop ----
                rng = (0, 4) if k == 0 else (1, 5)
                s0, s1 = rng
                h = Grp(mk(wpool, 12 * (s1 - s0) * SL, "h"),
                        6, s0, s1 - s0)
                # h[c,j] = psi[c,j] + g(j)*psi[c,p(j)]
                for j in range(2):
                    pj, gj = GCOL[k][j]
                    for r in range(2):
                        pr, psgn = (_phase_parts(gj)[0:2] if r == 0
                                    else _phase_parts(gj)[2:4])
                        dst = h.gap(j, 2, 3, r, s0, s1)       # c-packed
                        a0 = F.gap(j, 4, 3, r, s0, s1)        # psi[c,j].r
                        a1 = F.gap(pj, 4, 3, pr, s0, s1)
                        stt(dst, a1, psgn, a0, AL.add)
                phi = Grp(mk(wpool, 12 * (s1 - s0) * SL, "phi"),
                          6, s0, s1 - s0)
                cmm(phi, Uk, h, s0, s1, adag=True, nj=2)
                # shift (-1 along ax) then reconstruct into O
                if k == 0:
                    psh, dt = phi, -1
                else:
                    psh, dt = mat_shift(phi, ax, -1, 1, 5, "psh", wpool), 0
                # out[:, s'] += -0.5 * rho(s') * psh[:, q(s')]
                rec = [(0, 1.0), (1, 1.0), PROJ_F[k][0], PROJ_F[k][1]]
                for sp in range(4):
                    q, rho = rec[sp]
                    for r in range(2):
                        pr, psgn = (_phase_parts(rho)[0:2] if r == 0
                                    else _phase_parts(rho)[2:4])
                        dst = O.gap(sp, 4, 3, r, 1, 5)
                        src_ = psh.gap(q, 2, 3, pr, 1 + dt, 5 + dt)
                        stt(dst, src_, -0.5 * psgn, dst, AL.add)
                # ---- backward hop ----
                rng = (2, 6) if k == 0 else (1, 5)
                s0, s1 = rng
                hb = Grp(mk(wpool, 12 * (s1 - s0) * SL, "h"),
                         6, s0, s1 - s0)
                # hb[c,j] = g(j)*psi[c,p(j)] - psi[c,j]
                for j in range(2):
                    pj, gj = GCOL[k][j]
                    for r in range(2):
                        pr, psgn = (_phase_parts(gj)[0:2] if r == 0
                                    else _phase_parts(gj)[2:4])
                        dst = hb.gap(j, 2, 3, r, s0, s1)
                        a1 = F.gap(pj, 4, 3, pr, s0, s1)
                        a0 = F.gap(j, 4, 3, r, s0, s1)
                        stt(dst, a1, psgn, a0, AL.subtract)
                if k == 0:
                    hs, dt = hb, +1
                else:
                    hs, dt = mat_shift(hb, ax, +1, 1, 5, "psh", wpool), 0
                gm = Grp(mk(wpool, 12 * 4 * SL, "phi"),
                         6, 1, 4)
                cmm(gm, Uk, hs, 1, 5, nj=2, bdt=dt)
                rec = [(0, 1.0), (1, 1.0), PROJ_B[k][0], PROJ_B[k][1]]
                for sp in range(4):
                    q, rho = rec[sp]
                    for r in range(2):
                        pr, psgn = (_phase_parts(rho)[0:2] if r == 0
                                    else _phase_parts(rho)[2:4])
                        dst = O.gap(sp, 4, 3, r, 1, 5)
                        src_ = gm.gap(q, 2, 3, pr, 1, 5)
                        stt(dst, src_, 0.5 * psgn, dst, AL.add)

        # ---------- clover planes ----------
        _winit = set()
        with tc.tile_pool(name="cp", bufs=1) as cpool:
            def ctile(tag, nent, s0, s1):
                return Grp(mk(cpool, nent * 2 * (s1 - s0) * SL, tag), nent, s0, s1 - s0)

            for d1 in (range(1, 5) if DBG_PART in ("full", "clover") else []):
                for d2 in range(d1 + 1, 5):
                    if DBG_PAIRS and f"{d1}{d2}" not in DBG_PAIRS.split(","):
                        continue
                    a1, a2 = d1 - 1, d2 - 1
                    tpl = (d1 == 1)
                    sA, eA = (0, 5) if tpl else (1, 5)
                    U1 = GL[d1 - 1]
                    U2 = GL[d2 - 1]
                    # shifted links
                    if tpl:
                        U2m, u2dt = U2, +1       # U_d2(x+e_T): column view
                    else:
                        U2m, u2dt = mat_shift(U2, a1, +1, sA, eA, "lnk1", cpool), 0
                    U1n = mat_shift(U1, a2, +1, sA, eA, "lnk2", cpool)
                    A = ctile("pA", 9, sA, eA)
                    cmm(A, U1, U2m, sA, eA, bdt=u2dt)
                    B = ctile("pB", 9, sA, eA)
                    cmm(B, U2, U1n, sA, eA)
                    Q = ctile("pQ", 9, 1, 5)
                    cmm(Q, B, A, 1, 5, bdag=True)            # L4
                    sL1, eL1 = (0, 4) if tpl else (1, 5)
                    L1 = ctile("pL", 9, sL1, eL1)
                    cmm(L1, A, B, sL1, eL1, adag=True)
                    # L1 shift overlaps D/E products; Q-adds deferred
                    if tpl:
                        L1s = mat_shift(L1, a2, -1, 0, 4, "pLs", cpool)
                        l1_ap = L1s.all_ap(0, 4)
                    else:
                        L1ss = mat_shift_diag(L1, a1, a2, "pLs", cpool)
                        l1_ap = L1ss.all_ap(1, 5)
                    D = ctile("pA", 9, sA, eA)
                    cmm(D, U1n, U2m, sA, eA, bdag=True, bdt=u2dt)
                    E = ctile("pB", 9, sA, eA)
                    cmm(E, U2, U1, sA, eA, adag=True)
                    V.tensor_add(Q.all_ap(1, 5), Q.all_ap(1, 5), l1_ap)
                    L2 = ctile("pL2", 9, 1, 5)
                    cmm(L2, D, E, 1, 5, bdag=True)
                    L2s = mat_shift(L2, a2, -1, 1, 5, "pL2s", cpool)
                    sL3, eL3 = (0, 4) if tpl else (1, 5)
                    L3 = ctile("pL", 9, sL3, eL3)
                    cmm(L3, E, D, sL3, eL3, adag=True)
                    V.tensor_add(Q.all_ap(1, 5), Q.all_ap(1, 5),
                                 L2s.all_ap(1, 5))
                    if tpl:
                        V.tensor_add(Q.all_ap(1, 5), Q.all_ap(1, 5),
                                     L3.all_ap(0, 4))
                    else:
                        L3s = mat_shift(L3, a1, -1, 1, 5, "pLs", cpool)
                        V.tensor_add(Q.all_ap(1, 5), Q.all_ap(1, 5),
                                     L3s.all_ap(1, 5))
                    # ---- G9 = Q - Q^dag ----
                    G9 = ctile("pG9", 9, 1, 5)
                    qv = Q.t.rearrange("p (i j r w) -> p i j r w",
                                       i=3, j=3, r=2, w=Q.W)
                    qT = qv.transpose([0, 2, 1, 3, 4])
                    gv = G9.t.rearrange("p (i j r w) -> p i j r w",
                                        i=3, j=3, r=2, w=G9.W)
                    V.tensor_sub(gv[:, :, :, 0, :], qv[:, :, :, 0, :],
                                 qT[:, :, :, 0, :])
                    V.tensor_add(gv[:, :, :, 1, :], qv[:, :, :, 1, :],
                                 qT[:, :, :, 1, :])
                    # ---- accumulate into chiral W blocks ----
                    pidx = (a1, a2)
                    for sp in range(4):
                        wrow, phi_ph = SIGCOL[a1][a2][sp]
                        assert wrow // 2 == sp // 2, (a1, a2, sp, wrow)
                        c = -(1.0 / 16.0) * phi_ph
                        chir, spl, sl = sp // 2, sp % 2, wrow % 2
                        # re(W) += re(c) G9re - im(c) G9im
                        # im(W) += re(c) G9im + im(c) G9re
                        for outr in range(2):
                            if abs(c.imag) < 1e-6:
                                rsrc, coef = outr, c.real
                            else:
                                rsrc, coef = 1 - outr, (-c.imag if outr == 0
                                                        else c.imag)
                            wv = wslot(chir, spl, sl, outr)
                            src_ = G9.gap(0, 1, 9, rsrc, 1, 5)
                            key = (chir, spl, sl, outr)
                            if key not in _winit:
                                _winit.add(key)
                                V.tensor_scalar_mul(wv, src_, float(coef))
                            else:
                                stt(wv, src_, coef, wv, AL.add)

        # ---------- apply chiral clover blocks: O += W psi ----------
        if _winit:
            R = mk(main, 24 * W4, "Rap")
            r3 = R.rearrange("p (q w) -> p q w", q=24, w=W4)
            f5 = F.t.rearrange("p (c sp r w) -> p c sp r w",
                               c=3, sp=4, r=2, w=F.W)
            fc0, fc1 = F.cs(1, 5)
            tv = tview4(mtA, 3, 2, W4)     # [p,4,3(j),2(l),w]
            sv = sview4(msA, 3, W4)        # [p,4,3,w]
            for chir in range(2):
                br = f5[:, :, chir * 2:chir * 2 + 2, 0, fc0:fc1]
                bi = f5[:, :, chir * 2:chir * 2 + 2, 1, fc0:fc1]
                for i in range(3):
                    for spl in range(2):
                        ar = wrow_ap(chir, i, spl, 0)
                        ai = wrow_ap(chir, i, spl, 1)
                        V.tensor_mul(tv[:, 0], ar, br)
                        V.tensor_mul(tv[:, 1], ai, bi)
                        V.tensor_mul(tv[:, 2], ar, bi)
                        V.tensor_mul(tv[:, 3], ai, br)
                        t = sv[:, :, 0:2, :]       # [p,4,2,w]
                        V.tensor_add(t, tv[:, :, 0, :, :], tv[:, :, 1, :, :])
                        V.tensor_add(t, t, tv[:, :, 2, :, :])
                        s = sv[:, :, 2, :]         # [p,4,w]
                        V.tensor_add(s, t[:, :, 0, :], t[:, :, 1, :])
                        pl = i * 8 + chir * 4 + spl * 2
                        V.tensor_sub(r3[:, pl:pl + 1, :],
                                     s[:, 0:1, :], s[:, 1:2, :])
                        V.tensor_add(r3[:, pl + 1:pl + 2, :],
                                     s[:, 2:3, :], s[:, 3:4, :])
            stt(O.t[:, :], R[:, :], 1.0, O.t[:, :], AL.add)

        nc.sync.dma_start(out=out_d[:, :], in_=O.t[:, :])
        psum_cm.__exit__(None, None, None)
        spool_cm.__exit__(None, None, None)
        main_cm.__exit__(None, None, None)

    nc.compile()
    return nc


# ----------------------------------------------------------------------
# host entry
# ----------------------------------------------------------------------
_CACHE = {}

def _get_nc():
    if "nc" not in _CACHE:
        _CACHE["nc"] = build_program()
    return _CACHE["nc"]


def kernel(field_re, field_im, gauge_re, gauge_im):
    from concourse.bass_utils import run_bass_kernel_spmd
    nc = _get_nc()
    in_maps = [_host_inputs(field_re, field_im, gauge_re, gauge_im, c)
               for c in range(NCORES)]
    br = run_bass_kernel_spmd(nc, in_maps, list(range(NCORES)))
    out = np.empty((T_, Z_, Y_, X_, 3, 4, 2), np.float32)
    for c in range(NCORES):
        out[c * TLOC:(c + 1) * TLOC] = _host_output(br.results[c]["outp"])
    return out


# revision 23
# speedup vs baseline: 2.1927x; 1.1969x over previous
"""Wilson-clover Dirac operator D_WC on Trainium2, 8-core SPMD.

Self-contained: hardcodes LAT=(32,16,16,16), shards the T axis across 8
cores with host-side halo slices (t0-1..t0+4), computes everything
site-locally on-device with DVE elementwise math in an SoA layout:

  plane[p, col]: p = z*8 + (y>>1),  col = plane_idx*W + (s-base)*32
                 + (y&1)*16 + x          (s = local T-slice, W = ns*32)

Shifts: T = column offset (free), X = 2-piece ScalarE copy, Z/Y =
TensorE permutation matmul (stationary 128x128 0/1 matrix) + ScalarE
copyback from PSUM.  All elementwise arithmetic runs on the DVE only
(GPSIMD shares the DVE SBUF port; running both concurrently is a net
loss).  Gauge links for all 4 directions are DMA'd into SBUF once.
"""
import numpy as np

# ----------------------------------------------------------------------
# constants (physics)
# ----------------------------------------------------------------------
MASS, C_SW = 0.5, 1.0
_i = 1j
G1 = np.array([[0,0,0,_i],[0,0,_i,0],[0,-_i,0,0],[-_i,0,0,0]], dtype=np.complex64)
G2 = np.array([[0,0,0,-1],[0,0,1,0],[0,1,0,0],[-1,0,0,0]], dtype=np.complex64)
G3 = np.array([[0,0,_i,0],[0,0,0,-_i],[-_i,0,0,0],[0,_i,0,0]], dtype=np.complex64)
G4 = np.array([[0,0,1,0],[0,0,0,1],[1,0,0,0],[0,1,0,0]], dtype=np.complex64)
GAMMA = [G1, G2, G3, G4]
I4 = np.eye(4, dtype=np.complex64)
SIGMA = [[(0.5j * (GAMMA[m] @ GAMMA[n] - GAMMA[n] @ GAMMA[m])).astype(np.complex64)
          for n in range(4)] for m in range(4)]

T_, Z_, Y_, X_ = 32, 16, 16, 16
NCORES, TLOC, NSH = 8, 4, 6       # halo slices per core
SL = 32                           # cols per slice
USE_BF16 = True                   # bf16 compute tiles (O stays fp32)
import os as _os
DBG_PART = _os.environ.get("KRN_PART", "full")    # full|wilson|clover
DBG_PAIRS = _os.environ.get("KRN_PAIRS", "")      # e.g. "12,34" to keep only
DBG_HOPS = _os.environ.get("KRN_HOPS", "0123")    # wilson dirs to keep

# permutation matrix indices (stationary operands for TensorE shifts)
PZ_P, PZ_M, PY_P, PY_M, PZY_M = 0, 1, 2, 3, 4
NPERM = 5

# ----------------------------------------------------------------------
# spin-structure extraction (numerical)
# ----------------------------------------------------------------------
def _col_struct(M):
    """M has single-nonzero columns: return per-col (row, phase)."""
    out = []
    for s in range(4):
        col = M[:, s]
        r = int(np.argmax(np.abs(col)))
        ph = complex(col[r])
        assert np.sum(np.abs(col) > 1e-6) == 1, (M, s)
        out.append((r, ph))
    return out

def _proj_struct(P):
    """P rank-2 with cols 2,3 = rho * cols q in {0,1}. Returns (q2,rho2,q3,rho3)."""
    res = []
    for s in (2, 3):
        found = None
        for j in (0, 1):
            c, cj = P[:, s], P[:, j]
            nz = np.abs(cj) > 1e-6
            if not nz.any():
                continue
            ratio = c[nz] / cj[nz]
            if np.allclose(ratio, ratio[0], atol=1e-5) and np.allclose(
                    c, ratio[0] * cj, atol=1e-5):
                found = (j, complex(ratio[0]))
                break
        assert found is not None, P
        res.append(found)
    return res

# per direction k: gamma column structure and projector relations
GCOL = [_col_struct(GAMMA[k]) for k in range(4)]           # (p(s), g(s))
PROJ_F = [_proj_struct(GAMMA[k] + I4) for k in range(4)]   # for psi@(G+I)
PROJ_B = [_proj_struct(GAMMA[k] - I4) for k in range(4)]   # for psi@(G-I)
SIGCOL = [[_col_struct(SIGMA[m][n]) if m != n else None for n in range(4)]
          for m in range(4)]

def _phase_parts(ph):
    """phase in {1,-1,i,-i} -> for v = ph*u:
    re(v) = sr*u.[pr]  im(v) = si*u.[pi]   (0=re,1=im planes of u)"""
    if abs(ph - 1) < 1e-5:   return (0, 1.0, 1, 1.0)
    if abs(ph + 1) < 1e-5:   return (0, -1.0, 1, -1.0)
    if abs(ph - 1j) < 1e-5:  return (1, -1.0, 0, 1.0)
    if abs(ph + 1j) < 1e-5:  return (1, 1.0, 0, -1.0)
    raise AssertionError(ph)

# ----------------------------------------------------------------------
# host layout helpers
# ----------------------------------------------------------------------
def _to_planes(vol):
    """vol [ns, Z, Y, X] -> [128, ns*32]"""
    ns = vol.shape[0]
    v = vol.reshape(ns, 16, 8, 2, 16)        # s z yh yl x
    v = np.transpose(v, (1, 2, 0, 3, 4))     # z yh s yl x
    return np.ascontiguousarray(v.reshape(128, ns * 32))

def _from_planes(pl, ns):
    v = pl.reshape(16, 8, ns, 2, 16)
    return np.transpose(v, (2, 0, 1, 3, 4)).reshape(ns, 16, 16, 16)

def _perm_z(d):
    """dst partition m = z*8+yh reads p = ((z+d)%16)*8+yh; P[p, m] = 1."""
    P = np.zeros((128, 128), np.float32)
    for z in range(16):
        for yh in range(8):
            P[((z + d) % 16) * 8 + yh, z * 8 + yh] = 1.0
    return P

def _perm_y(d, dz=0):
    """Y cross-half: dst m = z*8+yh reads p = ((z+dz)%16)*8+((yh+d)%8)."""
    P = np.zeros((128, 128), np.float32)
    for z in range(16):
        for yh in range(8):
            P[((z + dz) % 16) * 8 + ((yh + d) % 8), z * 8 + yh] = 1.0
    return P

def _perm_host():
    P = np.concatenate([_perm_z(+1), _perm_z(-1), _perm_y(+1), _perm_y(-1),
                        _perm_y(-1, dz=-1)], axis=1)  # [128, NPERM*128]
    return P

def _host_inputs(field_re, field_im, gauge_re, gauge_im, core):
    t0 = core * TLOC
    ts = [(t0 - 1 + s) % T_ for s in range(NSH)]
    f = np.stack([field_re[ts], field_im[ts]], axis=0)   # [2,6,Z,Y,X,3,4]
    g = np.stack([gauge_re[:, ts], gauge_im[:, ts]], axis=0)  # [2,4,6,Z,Y,X,3,3]

    # field planes: e=(c*4+sp), plane=e*2+r
    fp = np.empty((24, 128, NSH * SL), np.float32)
    for c in range(3):
        for sp in range(4):
            for r in range(2):
                fp[(c * 4 + sp) * 2 + r] = _to_planes(f[r, :, :, :, :, c, sp])
    fld = np.ascontiguousarray(fp.transpose(1, 0, 2).reshape(128, 24 * NSH * SL))

    # gauge planes: dir k, e=(a*3+b), plane=(k*9+e)*2+r
    gp = np.empty((72, 128, NSH * SL), np.float32)
    for k in range(4):
        for a in range(3):
            for b in range(3):
                for r in range(2):
                    gp[(k * 9 + a * 3 + b) * 2 + r] = _to_planes(
                        g[r, k, :, :, :, :, a, b])
    gg = np.ascontiguousarray(gp.transpose(1, 0, 2).reshape(128, 72 * NSH * SL))
    perm = _perm_host()
    if USE_BF16:
        import ml_dtypes
        fld = fld.astype(ml_dtypes.bfloat16)
        gg = gg.astype(ml_dtypes.bfloat16)
        perm = perm.astype(ml_dtypes.bfloat16)
    return {"fld": fld, "gg": gg, "perm": perm}

def _host_output(outp_flat):
    """device out [128, 24*4*32] -> [TLOC, Z,Y,X, 3,4,2]"""
    pl = outp_flat.reshape(128, 24, TLOC * SL).transpose(1, 0, 2)
    out = np.empty((TLOC, 16, 16, 16, 3, 4, 2), np.float32)
    for c in range(3):
        for sp in range(4):
            for r in range(2):
                out[..., c, sp, r] = _from_planes(pl[(c * 4 + sp) * 2 + r], TLOC)
    return out

# ----------------------------------------------------------------------
# device program
# ----------------------------------------------------------------------
class Grp:
    """Group of planes in one SBUF tile. nent complex entries (re+im planes).
    base = slice index of col 0; ns slices; W = ns*32 cols per plane."""
    def __init__(self, tile, nent, base, ns):
        self.t, self.nent, self.base, self.ns = tile, nent, base, ns
        self.W = ns * SL

    def fl(self):
        return self.t.rearrange("p (q w) -> p q w", q=self.nent * 2, w=self.W)

    def cs(self, s0, s1):
        return ((s0 - self.base) * SL, (s1 - self.base) * SL)

    def pap(self, e, r, s0, s1):
        """single plane AP [128, cols]"""
        c0, c1 = self.cs(s0, s1)
        v = self.fl()[:, (e * 2 + r):(e * 2 + r + 1), c0:c1]
        return v  # [p,1,w]

    def gap(self, e0, estep, n, r, s0, s1):
        """packed-entry AP [p, n, w]: entries e0 + i*estep, fixed r."""
        c0, c1 = self.cs(s0, s1)
        q0 = e0 * 2 + r
        fl = self.fl()
        return fl[:, q0:q0 + 2 * estep * (n - 1) + 1:2 * estep, c0:c1]

    def all_ap(self, s0, s1):
        c0, c1 = self.cs(s0, s1)
        return self.fl()[:, :, c0:c1]


def build_program():
    import concourse.bacc as bacc
    import concourse.mybir as mybir
    from concourse.tile import TileContext
    FP = mybir.dt.float32
    CDT = mybir.dt.bfloat16 if USE_BF16 else FP
    AL = mybir.AluOpType

    nc = bacc.Bacc("TRN2", target_bir_lowering=False, debug=False)
    fld_d = nc.declare_dram_parameter("fld", [128, 24 * NSH * SL], CDT, isOutput=False)
    gg_d = nc.declare_dram_parameter("gg", [128, 72 * NSH * SL], CDT, isOutput=False)
    perm_d = nc.declare_dram_parameter("perm", [128, NPERM * 128], CDT, isOutput=False)
    out_d = nc.declare_dram_parameter("outp", [128, 24 * TLOC * SL], FP, isOutput=True)

    with TileContext(nc) as tc:
        _tc = [0]

        def mk(pool, cols, tag, dt=None):
            _tc[0] += 1
            return pool.tile([128, cols], dt or CDT, tag=tag,
                             name=f"{tag}_{_tc[0]}")

        main_cm = tc.tile_pool(name="main", bufs=1)
        main = main_cm.__enter__()
        spool_cm = tc.tile_pool(name="sp", bufs=1)
        spool = spool_cm.__enter__()
        psum_cm = tc.psum_pool(name="ps", bufs=4)
        psum = psum_cm.__enter__()

        F = Grp(mk(main, 24 * NSH * SL, "F"), 12, 0, NSH)
        O = Grp(mk(main, 24 * TLOC * SL, "O", FP), 12, 1, TLOC)
        PM = mk(main, NPERM * 128, "PM")
        nc.sync.dma_start(out=F.t[:, :], in_=fld_d[:, :])
        nc.sync.dma_start(out=PM[:, :], in_=perm_d[:, :])

        # gauge links for all 4 directions, loaded once
        GL = []
        for k in range(4):
            g = Grp(mk(main, 9 * 2 * NSH * SL, f"GL{k}"), 9, 0, NSH)
            eng = nc.sync if k % 2 == 0 else nc.scalar
            eng.dma_start(out=g.t[:, :],
                          in_=gg_d[:, k * 18 * NSH * SL:(k + 1) * 18 * NSH * SL])
            GL.append(g)

        # merged scratch tiles (4 product slots each)
        WMX = 5 * SL
        mtA = mk(spool, 4 * 9 * WMX, "mtA")
        msA = mk(spool, 4 * 3 * WMX, "msA")

        def tview4(t, nj, nk, w):
            """[p, 4, nj, nk, w] slots of merged mul scratch"""
            v = t.rearrange("p (h m) -> p h m", h=4)
            return v[:, :, :nj * nk * w].rearrange(
                "p h (j k w) -> p h j k w", j=nj, k=nk, w=w)

        def sview4(t, nj, w):
            v = t.rearrange("p (h m) -> p h m", h=4)
            return v[:, :, :nj * w].rearrange("p h (j w) -> p h j w",
                                              j=nj, w=w)

        V = nc.vector

        def a_pack(A, i, adag, r, s0, s1, dt, nj):
            """[p, nj(bcast), 3, w] for a-values (i,k)."""
            e0, st = (i, 3) if adag else (i * 3, 1)
            ap = A.gap(e0, st, 3, r, s0 + dt, s1 + dt)       # [p,3,w]
            w = ap.shape[2]
            return ap.unsqueeze(1).broadcast_to((128, nj, 3, w))

        def b_pack(B, bdag, r, s0, s1, dt, nj):
            """[p, nj, 3, w] for b-values (k,j)."""
            c0, c1 = B.cs(s0 + dt, s1 + dt)
            w = c1 - c0
            if bdag:  # e = j*3+k
                v = B.t.rearrange("p (j k r w) -> p j k r w", j=3, k=3, r=2, w=B.W)
                return v[:, :, :, r, c0:c1]
            if B.nent == 9:  # e = k*3+j
                v = B.t.rearrange("p (k j r w) -> p k j r w", k=3, j=3, r=2, w=B.W)
                return v[:, :, :, r, c0:c1].transpose([0, 2, 1, 3])
            # halfspinor: e = k*2+j, nj=2
            v = B.t.rearrange("p (k j r w) -> p k j r w", k=3, j=2, r=2, w=B.W)
            return v[:, :, :, r, c0:c1].transpose([0, 2, 1, 3])

        def stt(out, in0, coef, in1, op1=None):
            V.scalar_tensor_tensor(out, in0, float(coef), in1,
                                   AL.mult, op1 or AL.add)

        def cmm(dst, A, B, s0, s1, adag=False, bdag=False, adt=0, bdt=0, nj=3):
            """dst[i,j] = sum_k aval(i,k)*bval(k,j); dst entries e=i*nj+j.
            No (adag and bdag) case: signs reduce to add/sub combines."""
            assert not (adag and bdag)
            w = (s1 - s0) * SL
            for i in range(3):
                tv = tview4(mtA, nj, 3, w)     # [p,4,nj,3,w]
                sv = sview4(msA, nj, w)        # [p,4,nj,w]
                ar = a_pack(A, i, adag, 0, s0, s1, adt, nj)
                ai = a_pack(A, i, adag, 1, s0, s1, adt, nj)
                br = b_pack(B, bdag, 0, s0, s1, bdt, nj)
                bi = b_pack(B, bdag, 1, s0, s1, bdt, nj)
                dre = dst.gap(i * nj, 1, nj, 0, s0, s1)
                dim = dst.gap(i * nj, 1, nj, 1, s0, s1)
                # slots: 0=Srr 1=Sii 2=Sri 3=Sir
                V.tensor_mul(tv[:, 0], ar, br)
                V.tensor_mul(tv[:, 1], ai, bi)
                V.tensor_mul(tv[:, 2], ar, bi)
                V.tensor_mul(tv[:, 3], ai, br)
                V.tensor_add(sv, tv[:, :, :, 0, :], tv[:, :, :, 1, :])
                V.tensor_add(sv, sv, tv[:, :, :, 2, :])
                if adag or bdag:
                    V.tensor_add(dre, sv[:, 0], sv[:, 1])
                else:
                    V.tensor_sub(dre, sv[:, 0], sv[:, 1])
                if adag:
                    V.tensor_sub(dim, sv[:, 2], sv[:, 3])
                elif bdag:
                    V.tensor_sub(dim, sv[:, 3], sv[:, 2])
                else:
                    V.tensor_add(dim, sv[:, 2], sv[:, 3])

        # ---------- shift materialization ----------
        def cpy(out, in_):
            nc.scalar.copy(out, in_)

        def perm_ap(idx):
            return PM.rearrange("p (i m) -> p i m", i=NPERM)[:, idx, :]

        def mm_permute(dst_t, src_t, cols, pidx, dhalf=None, shalf=None,
                       xrot=False):
            """dst = P[pidx] applied to src partitions, over [0, cols).
            dhalf/shalf: copy back only dst yl==dhalf 16-col blocks, sourced
            from permuted yl==shalf blocks.
            xrot: additionally rotate X by -1 (dst x reads src x-1)."""
            P = perm_ap(pidx)
            c0 = 0
            while c0 < cols:
                cw = min(512, cols - c0)
                pt = psum.tile([128, cw], FP, tag="pshift",
                               name=f"ps_{_tc[0]}_{c0}")
                _tc[0] += 1
                nc.tensor.matmul(pt[:, :], P, src_t[:, c0:c0 + cw])
                dv = dst_t[:, c0:c0 + cw]
                if dhalf is None and not xrot:
                    cpy(dv, pt[:, :])
                elif dhalf is None and xrot:
                    d3 = dv.rearrange("p (b x) -> p b x", x=16)
                    p3 = pt.rearrange("p (b x) -> p b x", x=16)
                    cpy(d3[:, :, 1:16], p3[:, :, 0:15])
                    cpy(d3[:, :, 0:1], p3[:, :, 15:16])
                else:
                    d4 = dv.rearrange("p (b l x) -> p b l x", l=2, x=16)
                    p4 = pt.rearrange("p (b l x) -> p b l x", l=2, x=16)
                    if not xrot:
                        cpy(d4[:, :, dhalf, :], p4[:, :, shalf, :])
                    else:
                        cpy(d4[:, :, dhalf, 1:16], p4[:, :, shalf, 0:15])
                        cpy(d4[:, :, dhalf, 0:1], p4[:, :, shalf, 15:16])
                c0 += cw

        def lv(t_, lo):
            """yl==lo 16-col blocks of flat tile view"""
            return t_.rearrange("p (m x) -> p m x", x=16)[:, lo::2, :]

        def mat_shift(src, axis, d, s0, s1, tag, pool=None):
            """materialize S(x)=src(x + d*e_axis) over the FULL src range.
            axis 1(Z) 2(Y) 3(X)."""
            g = Grp(mk(pool or main, src.nent * 2 * src.ns * SL, tag),
                    src.nent, src.base, src.ns)
            cols = src.nent * 2 * src.ns * SL
            if axis == 3:   # X: 2-piece ScalarE copy
                def xv(t_):
                    return t_.rearrange("p (m x) -> p m x", x=16)
                dv, sv_ = xv(g.t), xv(src.t)
                if d == +1:
                    cpy(dv[:, :, 0:15], sv_[:, :, 1:16])
                    cpy(dv[:, :, 15:16], sv_[:, :, 0:1])
                else:
                    cpy(dv[:, :, 1:16], sv_[:, :, 0:15])
                    cpy(dv[:, :, 0:1], sv_[:, :, 15:16])
            elif axis == 1:  # Z: pure partition permutation
                mm_permute(g.t, src.t, cols, PZ_P if d == +1 else PZ_M)
            else:           # Y: in-partition half + permuted half
                if d == +1:
                    cpy(lv(g.t, 0), lv(src.t, 1))
                    mm_permute(g.t, src.t, cols, PY_P, dhalf=1, shalf=0)
                else:
                    cpy(lv(g.t, 1), lv(src.t, 0))
                    mm_permute(g.t, src.t, cols, PY_M, dhalf=0, shalf=1)
            return g

        def mat_shift_diag(src, a1, a2, tag, pool=None):
            """S(x) = src(x - e_a1 - e_a2), a1 < a2 spatial axes (1,2,3)."""
            g = Grp(mk(pool or main, src.nent * 2 * src.ns * SL, tag),
                    src.nent, src.base, src.ns)
            cols = src.nent * 2 * src.ns * SL
            if (a1, a2) == (1, 2):    # Z,Y
                mm_permute(g.t, src.t, cols, PZ_M, dhalf=1, shalf=0)
                mm_permute(g.t, src.t, cols, PZY_M, dhalf=0, shalf=1)
            elif (a1, a2) == (1, 3):  # Z,X
                mm_permute(g.t, src.t, cols, PZ_M, xrot=True)
            else:                     # Y,X
                dv = g.t.rearrange("p (m x) -> p m x", x=16)[:, 1::2, :]
                sv_ = src.t.rearrange("p (m x) -> p m x", x=16)[:, 0::2, :]
                cpy(dv[:, :, 1:16], sv_[:, :, 0:15])
                cpy(dv[:, :, 0:1], sv_[:, :, 15:16])
                mm_permute(g.t, src.t, cols, PY_M, dhalf=0, shalf=1, xrot=True)
            return g

        # ---------- mass term: O = (MASS+4) * F ----------
        V.tensor_scalar_mul(O.all_ap(1, 5), F.all_ap(1, 5), float(MASS + 4.0))

        # clover chiral blocks W[(i,sp),(j,s')] = sum_p -(1/16) ph_p(sp)
        #   * G9_p[i,j] with s' = wrow_p(sp);  plane index =
        #   chir*72 + spl*36 + sl*18 + i*6 + j*2 + r
        W4 = 4 * SL
        WCH = mk(main, 144 * W4, "WCH")
        wv3 = WCH.rearrange("p (q w) -> p q w", q=144, w=W4)
        wv6 = WCH.rearrange("p (c s l m w) -> p c s l m w",
                            c=2, s=2, l=2, m=18, w=W4)

        def wslot(chir, spl, sl, r):
            """[p, (i,j)(9), w4] view of one spin-slot of W (3D for stt)"""
            base = chir * 72 + spl * 36 + sl * 18 + r
            return wv3[:, base:base + 17:2, :]

        def wrow_ap(chir, i, spl, r):
            """[p, j(3), sl(2), w4] contraction view of one W row"""
            m0 = i * 6 + r
            return wv6[:, chir, spl, :, m0:m0 + 5:2, :].transpose([0, 2, 1, 3])

        # ---------- Wilson hops ----------
        with tc.tile_pool(name="wp", bufs=2) as wpool:
            for k in (range(4) if DBG_PART in ("full", "wilson") else []):
                if str(k) not in DBG_HOPS:
                    continue
                ax = k  # lattice axis
                Uk = GL[k]
                # ---- forward hop ----
                rng = (0, 4) if k == 0 else (1, 5)
                s0, s1 = rng
                h = Grp(mk(wpool, 12 * (s1 - s0) * SL, "h"),
                        6, s0, s1 - s0)
                # h[c,j] = psi[c,j] + g(j)*psi[c,p(j)]
                for j in range(2):
                    pj, gj = GCOL[k][j]
                    for r in range(2):
                        pr, psgn = (_phase_parts(gj)[0:2] if r == 0
                                    else _phase_parts(gj)[2:4])
                        dst = h.gap(j, 2, 3, r, s0, s1)       # c-packed
                        a0 = F.gap(j, 4, 3, r, s0, s1)        # psi[c,j].r
                        a1 = F.gap(pj, 4, 3, pr, s0, s1)
                        stt(dst, a1, psgn, a0, AL.add)
                phi = Grp(mk(wpool, 12 * (s1 - s0) * SL, "phi"),
                          6, s0, s1 - s0)
                cmm(phi, Uk, h, s0, s1, adag=True, nj=2)
                # shift (-1 along ax) then reconstruct into O
                if k == 0:
                    psh, dt = phi, -1
                else:
                    psh, dt = mat_shift(phi, ax, -1, 1, 5, "psh", wpool), 0
                # out[:, s'] += -0.5 * rho(s') * psh[:, q(s')]
                rec = [(0, 1.0), (1, 1.0), PROJ_F[k][0], PROJ_F[k][1]]
                for sp in range(4):
                    q, rho = rec[sp]
                    for r in range(2):
                        pr, psgn = (_phase_parts(rho)[0:2] if r == 0
                                    else _phase_parts(rho)[2:4])
                        dst = O.gap(sp, 4, 3, r, 1, 5)
                        src_ = psh.gap(q, 2, 3, pr, 1 + dt, 5 + dt)
                        stt(dst, src_, -0.5 * psgn, dst, AL.add)
                # ---- backward hop ----
                rng = (2, 6) if k == 0 else (1, 5)
                s0, s1 = rng
                hb = Grp(mk(wpool, 12 * (s1 - s0) * SL, "h"),
                         6, s0, s1 - s0)
                # hb[c,j] = g(j)*psi[c,p(j)] - psi[c,j]
                for j in range(2):
                    pj, gj = GCOL[k][j]
                    for r in range(2):
                        pr, psgn = (_phase_parts(gj)[0:2] if r == 0
                                    else _phase_parts(gj)[2:4])
                        dst = hb.gap(j, 2, 3, r, s0, s1)
                        a1 = F.gap(pj, 4, 3, pr, s0, s1)
                        a0 = F.gap(j, 4, 3, r, s0, s1)
                        stt(dst, a1, psgn, a0, AL.subtract)
                if k == 0:
                    hs, dt = hb, +1
                else:
                    hs, dt = mat_shift(hb, ax, +1, 1, 5, "psh", wpool), 0
                gm = Grp(mk(wpool, 12 * 4 * SL, "phi"),
                         6, 1, 4)
                cmm(gm, Uk, hs, 1, 5, nj=2, bdt=dt)
                rec = [(0, 1.0), (1, 1.0), PROJ_B[k][0], PROJ_B[k][1]]
                for sp in range(4):
                    q, rho = rec[sp]
                    for r in range(2):
                        pr, psgn = (_phase_parts(rho)[0:2] if r == 0
                                    else _phase_parts(rho)[2:4])
                        dst = O.gap(sp, 4, 3, r, 1, 5)
                        src_ = gm.gap(q, 2, 3, pr, 1, 5)
                        stt(dst, src_, 0.5 * psgn, dst, AL.add)

        # ---------- clover planes ----------
        _winit = set()
        with tc.tile_pool(name="cp", bufs=1) as cpool:
            def ctile(tag, nent, s0, s1):
                return Grp(mk(cpool, nent * 2 * (s1 - s0) * SL, tag), nent, s0, s1 - s0)

            for d1 in (range(1, 5) if DBG_PART in ("full", "clover") else []):
                for d2 in range(d1 + 1, 5):
                    if DBG_PAIRS and f"{d1}{d2}" not in DBG_PAIRS.split(","):
                        continue
                    a1, a2 = d1 - 1, d2 - 1
                    tpl = (d1 == 1)
                    sA, eA = (0, 5) if tpl else (1, 5)
                    U1 = GL[d1 - 1]
                    U2 = GL[d2 - 1]
                    # shifted links
                    if tpl:
                        U2m, u2dt = U2, +1       # U_d2(x+e_T): column view
                    else:
                        U2m, u2dt = mat_shift(U2, a1, +1, sA, eA, "lnk1", cpool), 0
                    U1n = mat_shift(U1, a2, +1, sA, eA, "lnk2", cpool)
                    A = ctile("pA", 9, sA, eA)
                    cmm(A, U1, U2m, sA, eA, bdt=u2dt)
                    B = ctile("pB", 9, sA, eA)
                    cmm(B, U2, U1n, sA, eA)
                    Q = ctile("pQ", 9, 1, 5)
                    cmm(Q, B, A, 1, 5, bdag=True)            # L4
                    sL1, eL1 = (0, 4) if tpl else (1, 5)
                    L1 = ctile("pL", 9, sL1, eL1)
                    cmm(L1, A, B, sL1, eL1, adag=True)
                    # L1 shift overlaps D/E products; Q-adds deferred
                    if tpl:
                        L1s = mat_shift(L1, a2, -1, 0, 4, "pLs", cpool)
                        l1_ap = L1s.all_ap(0, 4)
                    else:
                        L1ss = mat_shift_diag(L1, a1, a2, "pLs", cpool)
                        l1_ap = L1ss.all_ap(1, 5)
                    D = ctile("pA2", 9, sA, eA)
                    cmm(D, U1n, U2m, sA, eA, bdag=True, bdt=u2dt)
                    E = ctile("pB2", 9, sA, eA)
                    cmm(E, U2, U1, sA, eA, adag=True)
                    V.tensor_add(Q.all_ap(1, 5), Q.all_ap(1, 5), l1_ap)
                    L2 = ctile("pL2", 9, 1, 5)
                    cmm(L2, D, E, 1, 5, bdag=True)
                    L2s = mat_shift(L2, a2, -1, 1, 5, "pL2s", cpool)
                    sL3, eL3 = (0, 4) if tpl else (1, 5)
                    L3 = ctile("pL3", 9, sL3, eL3)
                    cmm(L3, E, D, sL3, eL3, adag=True)
                    V.tensor_add(Q.all_ap(1, 5), Q.all_ap(1, 5),
                                 L2s.all_ap(1, 5))
                    if tpl:
                        V.tensor_add(Q.all_ap(1, 5), Q.all_ap(1, 5),
                                     L3.all_ap(0, 4))
                    else:
                        L3s = mat_shift(L3, a1, -1, 1, 5, "pLs2", cpool)
                        V.tensor_add(Q.all_ap(1, 5), Q.all_ap(1, 5),
                                     L3s.all_ap(1, 5))
                    # ---- G9 = Q - Q^dag ----
                    G9 = ctile("pG9", 9, 1, 5)
                    qv = Q.t.rearrange("p (i j r w) -> p i j r w",
                                       i=3, j=3, r=2, w=Q.W)
                    qT = qv.transpose([0, 2, 1, 3, 4])
                    gv = G9.t.rearrange("p (i j r w) -> p i j r w",
                                        i=3, j=3, r=2, w=G9.W)
                    V.tensor_sub(gv[:, :, :, 0, :], qv[:, :, :, 0, :],
                                 qT[:, :, :, 0, :])
                    V.tensor_add(gv[:, :, :, 1, :], qv[:, :, :, 1, :],
                                 qT[:, :, :, 1, :])
                    # ---- accumulate into chiral W blocks ----
                    pidx = (a1, a2)
                    for sp in range(4):
                        wrow, phi_ph = SIGCOL[a1][a2][sp]
                        assert wrow // 2 == sp // 2, (a1, a2, sp, wrow)
                        c = -(1.0 / 16.0) * phi_ph
                        chir, spl, sl = sp // 2, sp % 2, wrow % 2
                        # re(W) += re(c) G9re - im(c) G9im
                        # im(W) += re(c) G9im + im(c) G9re
                        for outr in range(2):
                            if abs(c.imag) < 1e-6:
                                rsrc, coef = outr, c.real
                            else:
                                rsrc, coef = 1 - outr, (-c.imag if outr == 0
                                                        else c.imag)
                            wv = wslot(chir, spl, sl, outr)
                            src_ = G9.gap(0, 1, 9, rsrc, 1, 5)
                            key = (chir, spl, sl, outr)
                            if key not in _winit:
                                _winit.add(key)
                                V.tensor_scalar_mul(wv, src_, float(coef))
                            else:
                                stt(wv, src_, coef, wv, AL.add)

        # ---------- apply chiral clover blocks: O += W psi ----------
        if _winit:
            R = mk(main, 24 * W4, "Rap")
            r3 = R.rearrange("p (q w) -> p q w", q=24, w=W4)
            f5 = F.t.rearrange("p (c sp r w) -> p c sp r w",
                               c=3, sp=4, r=2, w=F.W)
            fc0, fc1 = F.cs(1, 5)
            tv = tview4(mtA, 3, 2, W4)     # [p,4,3(j),2(l),w]
            sv = sview4(msA, 3, W4)        # [p,4,3,w]
            for chir in range(2):
                br = f5[:, :, chir * 2:chir * 2 + 2, 0, fc0:fc1]
                bi = f5[:, :, chir * 2:chir * 2 + 2, 1, fc0:fc1]
                for i in range(3):
                    for spl in range(2):
                        ar = wrow_ap(chir, i, spl, 0)
                        ai = wrow_ap(chir, i, spl, 1)
                        V.tensor_mul(tv[:, 0], ar, br)
                        V.tensor_mul(tv[:, 1], ai, bi)
                        V.tensor_mul(tv[:, 2], ar, bi)
                        V.tensor_mul(tv[:, 3], ai, br)
                        t = sv[:, :, 0:2, :]       # [p,4,2,w]
                        V.tensor_add(t, tv[:, :, 0, :, :], tv[:, :, 1, :, :])
                        V.tensor_add(t, t, tv[:, :, 2, :, :])
                        s = sv[:, :, 2, :]         # [p,4,w]
                        V.tensor_add(s, t[:, :, 0, :], t[:, :, 1, :])
                        pl = i * 8 + chir * 4 + spl * 2
                        V.tensor_sub(r3[:, pl:pl + 1, :],
                                     s[:, 0:1, :], s[:, 1:2, :])
                        V.tensor_add(r3[:, pl + 1:pl + 2, :],
                                     s[:, 2:3, :], s[:, 3:4, :])
            stt(O.t[:, :], R[:, :], 1.0, O.t[:, :], AL.add)

        nc.sync.dma_start(out=out_d[:, :], in_=O.t[:, :])
        psum_cm.__exit__(None, None, None)
        spool_cm.__exit__(None, None, None)
        main_cm.__exit__(None, None, None)

    nc.compile()
    return nc


# ----------------------------------------------------------------------
# host entry
# ----------------------------------------------------------------------
_CACHE = {}

def _get_nc():
    if "nc" not in _CACHE:
        _CACHE["nc"] = build_program()
    return _CACHE["nc"]


def kernel(field_re, field_im, gauge_re, gauge_im):
    from concourse.bass_utils import run_bass_kernel_spmd
    nc = _get_nc()
    in_maps = [_host_inputs(field_re, field_im, gauge_re, gauge_im, c)
               for c in range(NCORES)]
    br = run_bass_kernel_spmd(nc, in_maps, list(range(NCORES)))
    out = np.empty((T_, Z_, Y_, X_, 3, 4, 2), np.float32)
    for c in range(NCORES):
        out[c * TLOC:(c + 1) * TLOC] = _host_output(br.results[c]["outp"])
    return out


# revision 27
# speedup vs baseline: 2.2331x; 1.0184x over previous
"""Wilson-clover Dirac operator D_WC on Trainium2, 8-core SPMD.

Self-contained: hardcodes LAT=(32,16,16,16), shards the T axis across 8
cores with host-side halo slices (t0-1..t0+4), computes everything
site-locally on-device with DVE elementwise math in an SoA layout:

  plane[p, col]: p = z*8 + (y>>1),  col = plane_idx*W + (s-base)*32
                 + (y&1)*16 + x          (s = local T-slice, W = ns*32)

Shifts: T = column offset (free), X = 2-piece ScalarE copy, Z/Y =
TensorE permutation matmul (stationary 128x128 0/1 matrix) + ScalarE
copyback from PSUM.  All elementwise arithmetic runs on the DVE only
(GPSIMD shares the DVE SBUF port; running both concurrently is a net
loss).  Gauge links for all 4 directions are DMA'd into SBUF once.
"""
import numpy as np

# ----------------------------------------------------------------------
# constants (physics)
# ----------------------------------------------------------------------
MASS, C_SW = 0.5, 1.0
_i = 1j
G1 = np.array([[0,0,0,_i],[0,0,_i,0],[0,-_i,0,0],[-_i,0,0,0]], dtype=np.complex64)
G2 = np.array([[0,0,0,-1],[0,0,1,0],[0,1,0,0],[-1,0,0,0]], dtype=np.complex64)
G3 = np.array([[0,0,_i,0],[0,0,0,-_i],[-_i,0,0,0],[0,_i,0,0]], dtype=np.complex64)
G4 = np.array([[0,0,1,0],[0,0,0,1],[1,0,0,0],[0,1,0,0]], dtype=np.complex64)
GAMMA = [G1, G2, G3, G4]
I4 = np.eye(4, dtype=np.complex64)
SIGMA = [[(0.5j * (GAMMA[m] @ GAMMA[n] - GAMMA[n] @ GAMMA[m])).astype(np.complex64)
          for n in range(4)] for m in range(4)]

T_, Z_, Y_, X_ = 32, 16, 16, 16
NCORES, TLOC, NSH = 8, 4, 6       # halo slices per core
SL = 32                           # cols per slice
USE_BF16 = True                   # bf16 compute tiles (O stays fp32)
import os as _os
DBG_PART = _os.environ.get("KRN_PART", "full")    # full|wilson|clover
DBG_PAIRS = _os.environ.get("KRN_PAIRS", "")      # e.g. "12,34" to keep only
DBG_HOPS = _os.environ.get("KRN_HOPS", "0123")    # wilson dirs to keep

# permutation matrix indices (stationary operands for TensorE shifts)
PZ_P, PZ_M, PY_P, PY_M, PZY_M = 0, 1, 2, 3, 4
NPERM = 5

# ----------------------------------------------------------------------
# spin-structure extraction (numerical)
# ----------------------------------------------------------------------
def _col_struct(M):
    """M has single-nonzero columns: return per-col (row, phase)."""
    out = []
    for s in range(4):
        col = M[:, s]
        r = int(np.argmax(np.abs(col)))
        ph = complex(col[r])
        assert np.sum(np.abs(col) > 1e-6) == 1, (M, s)
        out.append((r, ph))
    return out

def _proj_struct(P):
    """P rank-2 with cols 2,3 = rho * cols q in {0,1}. Returns (q2,rho2,q3,rho3)."""
    res = []
    for s in (2, 3):
        found = None
        for j in (0, 1):
            c, cj = P[:, s], P[:, j]
            nz = np.abs(cj) > 1e-6
            if not nz.any():
                continue
            ratio = c[nz] / cj[nz]
            if np.allclose(ratio, ratio[0], atol=1e-5) and np.allclose(
                    c, ratio[0] * cj, atol=1e-5):
                found = (j, complex(ratio[0]))
                break
        assert found is not None, P
        res.append(found)
    return res

# per direction k: gamma column structure and projector relations
GCOL = [_col_struct(GAMMA[k]) for k in range(4)]           # (p(s), g(s))
PROJ_F = [_proj_struct(GAMMA[k] + I4) for k in range(4)]   # for psi@(G+I)
PROJ_B = [_proj_struct(GAMMA[k] - I4) for k in range(4)]   # for psi@(G-I)
SIGCOL = [[_col_struct(SIGMA[m][n]) if m != n else None for n in range(4)]
          for m in range(4)]

def _phase_parts(ph):
    """phase in {1,-1,i,-i} -> for v = ph*u:
    re(v) = sr*u.[pr]  im(v) = si*u.[pi]   (0=re,1=im planes of u)"""
    if abs(ph - 1) < 1e-5:   return (0, 1.0, 1, 1.0)
    if abs(ph + 1) < 1e-5:   return (0, -1.0, 1, -1.0)
    if abs(ph - 1j) < 1e-5:  return (1, -1.0, 0, 1.0)
    if abs(ph + 1j) < 1e-5:  return (1, 1.0, 0, -1.0)
    raise AssertionError(ph)

# ----------------------------------------------------------------------
# host layout helpers
# ----------------------------------------------------------------------
def _to_planes(vol):
    """vol [ns, Z, Y, X] -> [128, ns*32]"""
    ns = vol.shape[0]
    v = vol.reshape(ns, 16, 8, 2, 16)        # s z yh yl x
    v = np.transpose(v, (1, 2, 0, 3, 4))     # z yh s yl x
    return np.ascontiguousarray(v.reshape(128, ns * 32))

def _from_planes(pl, ns):
    v = pl.reshape(16, 8, ns, 2, 16)
    return np.transpose(v, (2, 0, 1, 3, 4)).reshape(ns, 16, 16, 16)

def _perm_z(d):
    """dst partition m = z*8+yh reads p = ((z+d)%16)*8+yh; P[p, m] = 1."""
    P = np.zeros((128, 128), np.float32)
    for z in range(16):
        for yh in range(8):
            P[((z + d) % 16) * 8 + yh, z * 8 + yh] = 1.0
    return P

def _perm_y(d, dz=0):
    """Y cross-half: dst m = z*8+yh reads p = ((z+dz)%16)*8+((yh+d)%8)."""
    P = np.zeros((128, 128), np.float32)
    for z in range(16):
        for yh in range(8):
            P[((z + dz) % 16) * 8 + ((yh + d) % 8), z * 8 + yh] = 1.0
    return P

def _perm_host():
    P = np.concatenate([_perm_z(+1), _perm_z(-1), _perm_y(+1), _perm_y(-1),
                        _perm_y(-1, dz=-1)], axis=1)  # [128, NPERM*128]
    return P

def _host_inputs(field_re, field_im, gauge_re, gauge_im, core):
    t0 = core * TLOC
    ts = [(t0 - 1 + s) % T_ for s in range(NSH)]
    f = np.stack([field_re[ts], field_im[ts]], axis=0)   # [2,6,Z,Y,X,3,4]
    g = np.stack([gauge_re[:, ts], gauge_im[:, ts]], axis=0)  # [2,4,6,Z,Y,X,3,3]

    # field planes: e=(c*4+sp), plane=e*2+r
    fp = np.empty((24, 128, NSH * SL), np.float32)
    for c in range(3):
        for sp in range(4):
            for r in range(2):
                fp[(c * 4 + sp) * 2 + r] = _to_planes(f[r, :, :, :, :, c, sp])
    fld = np.ascontiguousarray(fp.transpose(1, 0, 2).reshape(128, 24 * NSH * SL))

    # gauge planes: dir k, e=(a*3+b), plane=(k*9+e)*2+r
    gp = np.empty((72, 128, NSH * SL), np.float32)
    for k in range(4):
        for a in range(3):
            for b in range(3):
                for r in range(2):
                    gp[(k * 9 + a * 3 + b) * 2 + r] = _to_planes(
                        g[r, k, :, :, :, :, a, b])
    gg = np.ascontiguousarray(gp.transpose(1, 0, 2).reshape(128, 72 * NSH * SL))
    perm = _perm_host()
    if USE_BF16:
        import ml_dtypes
        fld = fld.astype(ml_dtypes.bfloat16)
        gg = gg.astype(ml_dtypes.bfloat16)
        perm = perm.astype(ml_dtypes.bfloat16)
    return {"fld": fld, "gg": gg, "perm": perm}

def _host_output(outp_flat):
    """device out [128, 24*4*32] -> [TLOC, Z,Y,X, 3,4,2]"""
    pl = outp_flat.reshape(128, 24, TLOC * SL).transpose(1, 0, 2)
    out = np.empty((TLOC, 16, 16, 16, 3, 4, 2), np.float32)
    for c in range(3):
        for sp in range(4):
            for r in range(2):
                out[..., c, sp, r] = _from_planes(pl[(c * 4 + sp) * 2 + r], TLOC)
    return out

# ----------------------------------------------------------------------
# device program
# ----------------------------------------------------------------------
class Grp:
    """Group of planes in one SBUF tile. nent complex entries (re+im planes).
    base = slice index of col 0; ns slices; W = ns*32 cols per plane."""
    def __init__(self, tile, nent, base, ns):
        self.t, self.nent, self.base, self.ns = tile, nent, base, ns
        self.W = ns * SL

    def fl(self):
        return self.t.rearrange("p (q w) -> p q w", q=self.nent * 2, w=self.W)

    def cs(self, s0, s1):
        return ((s0 - self.base) * SL, (s1 - self.base) * SL)

    def pap(self, e, r, s0, s1):
        """single plane AP [128, cols]"""
        c0, c1 = self.cs(s0, s1)
        v = self.fl()[:, (e * 2 + r):(e * 2 + r + 1), c0:c1]
        return v  # [p,1,w]

    def gap(self, e0, estep, n, r, s0, s1):
        """packed-entry AP [p, n, w]: entries e0 + i*estep, fixed r."""
        c0, c1 = self.cs(s0, s1)
        q0 = e0 * 2 + r
        fl = self.fl()
        return fl[:, q0:q0 + 2 * estep * (n - 1) + 1:2 * estep, c0:c1]

    def all_ap(self, s0, s1):
        c0, c1 = self.cs(s0, s1)
        return self.fl()[:, :, c0:c1]


def build_program():
    import concourse.bacc as bacc
    import concourse.mybir as mybir
    from concourse.tile import TileContext
    FP = mybir.dt.float32
    CDT = mybir.dt.bfloat16 if USE_BF16 else FP
    AL = mybir.AluOpType

    nc = bacc.Bacc("TRN2", target_bir_lowering=False, debug=False)
    fld_d = nc.declare_dram_parameter("fld", [128, 24 * NSH * SL], CDT, isOutput=False)
    gg_d = nc.declare_dram_parameter("gg", [128, 72 * NSH * SL], CDT, isOutput=False)
    perm_d = nc.declare_dram_parameter("perm", [128, NPERM * 128], CDT, isOutput=False)
    out_d = nc.declare_dram_parameter("outp", [128, 24 * TLOC * SL], FP, isOutput=True)

    with TileContext(nc) as tc:
        _tc = [0]

        def mk(pool, cols, tag, dt=None):
            _tc[0] += 1
            return pool.tile([128, cols], dt or CDT, tag=tag,
                             name=f"{tag}_{_tc[0]}")

        main_cm = tc.tile_pool(name="main", bufs=1)
        main = main_cm.__enter__()
        spool_cm = tc.tile_pool(name="sp", bufs=1)
        spool = spool_cm.__enter__()
        psum_cm = tc.psum_pool(name="ps", bufs=4)
        psum = psum_cm.__enter__()

        F = Grp(mk(main, 24 * NSH * SL, "F"), 12, 0, NSH)
        O = Grp(mk(main, 24 * TLOC * SL, "O", FP), 12, 1, TLOC)
        PM = mk(main, NPERM * 128, "PM")
        nc.sync.dma_start(out=F.t[:, :], in_=fld_d[:, :])
        nc.sync.dma_start(out=PM[:, :], in_=perm_d[:, :])

        # gauge links for all 4 directions, loaded once
        GL = []
        for k in range(4):
            g = Grp(mk(main, 9 * 2 * NSH * SL, f"GL{k}"), 9, 0, NSH)
            eng = nc.sync if k % 2 == 0 else nc.scalar
            eng.dma_start(out=g.t[:, :],
                          in_=gg_d[:, k * 18 * NSH * SL:(k + 1) * 18 * NSH * SL])
            GL.append(g)

        # merged scratch tiles (4 product slots each)
        WMX = 5 * SL
        mtA = mk(spool, 4 * 9 * WMX, "mtA")
        msA = mk(spool, 4 * 3 * WMX, "msA")

        def tview4(t, nj, nk, w):
            """[p, 4, nj, nk, w] slots of merged mul scratch"""
            v = t.rearrange("p (h m) -> p h m", h=4)
            return v[:, :, :nj * nk * w].rearrange(
                "p h (j k w) -> p h j k w", j=nj, k=nk, w=w)

        def sview4(t, nj, w):
            v = t.rearrange("p (h m) -> p h m", h=4)
            return v[:, :, :nj * w].rearrange("p h (j w) -> p h j w",
                                              j=nj, w=w)

        V = nc.vector

        def a_pack(A, i, adag, r, s0, s1, dt, nj):
            """[p, nj(bcast), 3, w] for a-values (i,k)."""
            e0, st = (i, 3) if adag else (i * 3, 1)
            ap = A.gap(e0, st, 3, r, s0 + dt, s1 + dt)       # [p,3,w]
            w = ap.shape[2]
            return ap.unsqueeze(1).broadcast_to((128, nj, 3, w))

        def b_pack(B, bdag, r, s0, s1, dt, nj):
            """[p, nj, 3, w] for b-values (k,j)."""
            c0, c1 = B.cs(s0 + dt, s1 + dt)
            w = c1 - c0
            if bdag:  # e = j*3+k
                v = B.t.rearrange("p (j k r w) -> p j k r w", j=3, k=3, r=2, w=B.W)
                return v[:, :, :, r, c0:c1]
            if B.nent == 9:  # e = k*3+j
                v = B.t.rearrange("p (k j r w) -> p k j r w", k=3, j=3, r=2, w=B.W)
                return v[:, :, :, r, c0:c1].transpose([0, 2, 1, 3])
            # halfspinor: e = k*2+j, nj=2
            v = B.t.rearrange("p (k j r w) -> p k j r w", k=3, j=2, r=2, w=B.W)
            return v[:, :, :, r, c0:c1].transpose([0, 2, 1, 3])

        def stt(out, in0, coef, in1, op1=None):
            V.scalar_tensor_tensor(out, in0, float(coef), in1,
                                   AL.mult, op1 or AL.add)

        def cmm(dst, A, B, s0, s1, adag=False, bdag=False, adt=0, bdt=0, nj=3):
            """dst[i,j] = sum_k aval(i,k)*bval(k,j); dst entries e=i*nj+j.
            No (adag and bdag) case: signs reduce to add/sub combines."""
            assert not (adag and bdag)
            w = (s1 - s0) * SL
            for i in range(3):
                tv = tview4(mtA, nj, 3, w)     # [p,4,nj,3,w]
                sv = sview4(msA, nj, w)        # [p,4,nj,w]
                ar = a_pack(A, i, adag, 0, s0, s1, adt, nj)
                ai = a_pack(A, i, adag, 1, s0, s1, adt, nj)
                br = b_pack(B, bdag, 0, s0, s1, bdt, nj)
                bi = b_pack(B, bdag, 1, s0, s1, bdt, nj)
                dre = dst.gap(i * nj, 1, nj, 0, s0, s1)
                dim = dst.gap(i * nj, 1, nj, 1, s0, s1)
                # slots: 0=Srr 1=Sii 2=Sri 3=Sir
                V.tensor_mul(tv[:, 0], ar, br)
                V.tensor_mul(tv[:, 1], ai, bi)
                V.tensor_mul(tv[:, 2], ar, bi)
                V.tensor_mul(tv[:, 3], ai, br)
                V.tensor_add(sv, tv[:, :, :, 0, :], tv[:, :, :, 1, :])
                V.tensor_add(sv, sv, tv[:, :, :, 2, :])
                if adag or bdag:
                    V.tensor_add(dre, sv[:, 0], sv[:, 1])
                else:
                    V.tensor_sub(dre, sv[:, 0], sv[:, 1])
                if adag:
                    V.tensor_sub(dim, sv[:, 2], sv[:, 3])
                elif bdag:
                    V.tensor_sub(dim, sv[:, 3], sv[:, 2])
                else:
                    V.tensor_add(dim, sv[:, 2], sv[:, 3])

        # ---------- shift materialization ----------
        def cpy(out, in_):
            nc.scalar.copy(out, in_)

        def perm_ap(idx):
            return PM.rearrange("p (i m) -> p i m", i=NPERM)[:, idx, :]

        def mm_permute(dst_t, src_t, cols, pidx, dhalf=None, shalf=None,
                       xrot=False):
            """dst = P[pidx] applied to src partitions, over [0, cols).
            dhalf/shalf: copy back only dst yl==dhalf 16-col blocks, sourced
            from permuted yl==shalf blocks.
            xrot: additionally rotate X by -1 (dst x reads src x-1)."""
            P = perm_ap(pidx)
            c0 = 0
            while c0 < cols:
                cw = min(512, cols - c0)
                pt = psum.tile([128, cw], FP, tag="pshift",
                               name=f"ps_{_tc[0]}_{c0}")
                _tc[0] += 1
                nc.tensor.matmul(pt[:, :], P, src_t[:, c0:c0 + cw])
                dv = dst_t[:, c0:c0 + cw]
                if dhalf is None and not xrot:
                    cpy(dv, pt[:, :])
                elif dhalf is None and xrot:
                    d3 = dv.rearrange("p (b x) -> p b x", x=16)
                    p3 = pt.rearrange("p (b x) -> p b x", x=16)
                    cpy(d3[:, :, 1:16], p3[:, :, 0:15])
                    cpy(d3[:, :, 0:1], p3[:, :, 15:16])
                else:
                    d4 = dv.rearrange("p (b l x) -> p b l x", l=2, x=16)
                    p4 = pt.rearrange("p (b l x) -> p b l x", l=2, x=16)
                    if not xrot:
                        cpy(d4[:, :, dhalf, :], p4[:, :, shalf, :])
                    else:
                        cpy(d4[:, :, dhalf, 1:16], p4[:, :, shalf, 0:15])
                        cpy(d4[:, :, dhalf, 0:1], p4[:, :, shalf, 15:16])
                c0 += cw

        def lv(t_, lo):
            """yl==lo 16-col blocks of flat tile view"""
            return t_.rearrange("p (m x) -> p m x", x=16)[:, lo::2, :]

        def mat_shift(src, axis, d, s0, s1, tag, pool=None):
            """materialize S(x)=src(x + d*e_axis) over the FULL src range.
            axis 1(Z) 2(Y) 3(X)."""
            g = Grp(mk(pool or main, src.nent * 2 * src.ns * SL, tag),
                    src.nent, src.base, src.ns)
            cols = src.nent * 2 * src.ns * SL
            if axis == 3:   # X: 2-piece ScalarE copy
                def xv(t_):
                    return t_.rearrange("p (m x) -> p m x", x=16)
                dv, sv_ = xv(g.t), xv(src.t)
                if d == +1:
                    cpy(dv[:, :, 0:15], sv_[:, :, 1:16])
                    cpy(dv[:, :, 15:16], sv_[:, :, 0:1])
                else:
                    cpy(dv[:, :, 1:16], sv_[:, :, 0:15])
                    cpy(dv[:, :, 0:1], sv_[:, :, 15:16])
            elif axis == 1:  # Z: pure partition permutation
                mm_permute(g.t, src.t, cols, PZ_P if d == +1 else PZ_M)
            else:           # Y: in-partition half + permuted half
                if d == +1:
                    cpy(lv(g.t, 0), lv(src.t, 1))
                    mm_permute(g.t, src.t, cols, PY_P, dhalf=1, shalf=0)
                else:
                    cpy(lv(g.t, 1), lv(src.t, 0))
                    mm_permute(g.t, src.t, cols, PY_M, dhalf=0, shalf=1)
            return g

        def mat_shift_diag(src, a1, a2, tag, pool=None):
            """S(x) = src(x - e_a1 - e_a2), a1 < a2 spatial axes (1,2,3)."""
            g = Grp(mk(pool or main, src.nent * 2 * src.ns * SL, tag),
                    src.nent, src.base, src.ns)
            cols = src.nent * 2 * src.ns * SL
            if (a1, a2) == (1, 2):    # Z,Y
                mm_permute(g.t, src.t, cols, PZ_M, dhalf=1, shalf=0)
                mm_permute(g.t, src.t, cols, PZY_M, dhalf=0, shalf=1)
            elif (a1, a2) == (1, 3):  # Z,X
                mm_permute(g.t, src.t, cols, PZ_M, xrot=True)
            else:                     # Y,X
                dv = g.t.rearrange("p (m x) -> p m x", x=16)[:, 1::2, :]
                sv_ = src.t.rearrange("p (m x) -> p m x", x=16)[:, 0::2, :]
                cpy(dv[:, :, 1:16], sv_[:, :, 0:15])
                cpy(dv[:, :, 0:1], sv_[:, :, 15:16])
                mm_permute(g.t, src.t, cols, PY_M, dhalf=0, shalf=1, xrot=True)
            return g

        # ---------- mass term: O = (MASS+4) * F ----------
        V.tensor_scalar_mul(O.all_ap(1, 5), F.all_ap(1, 5), float(MASS + 4.0))

        # clover chiral blocks W[(i,sp),(j,s')] = sum_p -(1/16) ph_p(sp)
        #   * G9_p[i,j] with s' = wrow_p(sp);  plane index =
        #   chir*72 + spl*36 + sl*18 + i*6 + j*2 + r
        W4 = 4 * SL
        WCH = mk(main, 144 * W4, "WCH")
        wv3 = WCH.rearrange("p (q w) -> p q w", q=144, w=W4)
        wv6 = WCH.rearrange("p (c s l m w) -> p c s l m w",
                            c=2, s=2, l=2, m=18, w=W4)

        def wslot(chir, spl, sl, r):
            """[p, (i,j)(9), w4] view of one spin-slot of W (3D for stt)"""
            base = chir * 72 + spl * 36 + sl * 18 + r
            return wv3[:, base:base + 17:2, :]

        def wrow_ap(chir, i, spl, r):
            """[p, j(3), sl(2), w4] contraction view of one W row"""
            m0 = i * 6 + r
            return wv6[:, chir, spl, :, m0:m0 + 5:2, :].transpose([0, 2, 1, 3])

        # ---------- Wilson hops ----------
        with tc.tile_pool(name="wp", bufs=2) as wpool:
            for k in (range(4) if DBG_PART in ("full", "wilson") else []):
                if str(k) not in DBG_HOPS:
                    continue
                ax = k  # lattice axis
                Uk = GL[k]
                # ---- forward hop ----
                rng = (0, 4) if k == 0 else (1, 5)
                s0, s1 = rng
                h = Grp(mk(wpool, 12 * (s1 - s0) * SL, "h"),
                        6, s0, s1 - s0)
                # h[c,j] = psi[c,j] + g(j)*psi[c,p(j)]
                for j in range(2):
                    pj, gj = GCOL[k][j]
                    for r in range(2):
                        pr, psgn = (_phase_parts(gj)[0:2] if r == 0
                                    else _phase_parts(gj)[2:4])
                        dst = h.gap(j, 2, 3, r, s0, s1)       # c-packed
                        a0 = F.gap(j, 4, 3, r, s0, s1)        # psi[c,j].r
                        a1 = F.gap(pj, 4, 3, pr, s0, s1)
                        if psgn > 0:
                            V.tensor_add(dst, a1, a0)
                        else:
                            V.tensor_sub(dst, a0, a1)
                phi = Grp(mk(wpool, 12 * (s1 - s0) * SL, "phi"),
                          6, s0, s1 - s0)
                cmm(phi, Uk, h, s0, s1, adag=True, nj=2)
                # shift (-1 along ax) then reconstruct into O
                if k == 0:
                    psh, dt = phi, -1
                else:
                    psh, dt = mat_shift(phi, ax, -1, 1, 5, "psh", wpool), 0
                # out[:, s'] += -0.5 * rho(s') * psh[:, q(s')]
                rec = [(0, 1.0), (1, 1.0), PROJ_F[k][0], PROJ_F[k][1]]
                for sp in range(4):
                    q, rho = rec[sp]
                    for r in range(2):
                        pr, psgn = (_phase_parts(rho)[0:2] if r == 0
                                    else _phase_parts(rho)[2:4])
                        dst = O.gap(sp, 4, 3, r, 1, 5)
                        src_ = psh.gap(q, 2, 3, pr, 1 + dt, 5 + dt)
                        stt(dst, src_, -0.5 * psgn, dst, AL.add)
                # ---- backward hop ----
                rng = (2, 6) if k == 0 else (1, 5)
                s0, s1 = rng
                hb = Grp(mk(wpool, 12 * (s1 - s0) * SL, "h"),
                         6, s0, s1 - s0)
                # hb[c,j] = g(j)*psi[c,p(j)] - psi[c,j]
                for j in range(2):
                    pj, gj = GCOL[k][j]
                    for r in range(2):
                        pr, psgn = (_phase_parts(gj)[0:2] if r == 0
                                    else _phase_parts(gj)[2:4])
                        dst = hb.gap(j, 2, 3, r, s0, s1)
                        a1 = F.gap(pj, 4, 3, pr, s0, s1)
                        a0 = F.gap(j, 4, 3, r, s0, s1)
                        if psgn > 0:
                            V.tensor_sub(dst, a1, a0)
                        else:
                            stt(dst, a1, psgn, a0, AL.subtract)
                if k == 0:
                    hs, dt = hb, +1
                else:
                    hs, dt = mat_shift(hb, ax, +1, 1, 5, "psh", wpool), 0
                gm = Grp(mk(wpool, 12 * 4 * SL, "phi"),
                         6, 1, 4)
                cmm(gm, Uk, hs, 1, 5, nj=2, bdt=dt)
                rec = [(0, 1.0), (1, 1.0), PROJ_B[k][0], PROJ_B[k][1]]
                for sp in range(4):
                    q, rho = rec[sp]
                    for r in range(2):
                        pr, psgn = (_phase_parts(rho)[0:2] if r == 0
                                    else _phase_parts(rho)[2:4])
                        dst = O.gap(sp, 4, 3, r, 1, 5)
                        src_ = gm.gap(q, 2, 3, pr, 1, 5)
                        stt(dst, src_, 0.5 * psgn, dst, AL.add)

        # ---------- clover planes ----------
        _winit = set()
        with tc.tile_pool(name="cp", bufs=1) as cpool:
            def ctile(tag, nent, s0, s1):
                return Grp(mk(cpool, nent * 2 * (s1 - s0) * SL, tag), nent, s0, s1 - s0)

            for d1 in (range(1, 5) if DBG_PART in ("full", "clover") else []):
                for d2 in range(d1 + 1, 5):
                    if DBG_PAIRS and f"{d1}{d2}" not in DBG_PAIRS.split(","):
                        continue
                    a1, a2 = d1 - 1, d2 - 1
                    tpl = (d1 == 1)
                    sA, eA = (0, 5) if tpl else (1, 5)
                    U1 = GL[d1 - 1]
                    U2 = GL[d2 - 1]
                    # shifted links
                    if tpl:
                        U2m, u2dt = U2, +1       # U_d2(x+e_T): column view
                    else:
                        U2m, u2dt = mat_shift(U2, a1, +1, sA, eA, "lnk1", cpool), 0
                    U1n = mat_shift(U1, a2, +1, sA, eA, "lnk2", cpool)
                    A = ctile("pA", 9, sA, eA)
                    cmm(A, U1, U2m, sA, eA, bdt=u2dt)
                    B = ctile("pB", 9, sA, eA)
                    cmm(B, U2, U1n, sA, eA)
                    Q = ctile("pQ", 9, 1, 5)
                    cmm(Q, B, A, 1, 5, bdag=True)            # L4
                    sL1, eL1 = (0, 4) if tpl else (1, 5)
                    L1 = ctile("pL", 9, sL1, eL1)
                    cmm(L1, A, B, sL1, eL1, adag=True)
                    # L1 shift overlaps D/E products; Q-adds deferred
                    if tpl:
                        L1s = mat_shift(L1, a2, -1, 0, 4, "pLs", cpool)
                        l1_ap = L1s.all_ap(0, 4)
                    else:
                        L1ss = mat_shift_diag(L1, a1, a2, "pLs", cpool)
                        l1_ap = L1ss.all_ap(1, 5)
                    D = ctile("pA2", 9, sA, eA)
                    cmm(D, U1n, U2m, sA, eA, bdag=True, bdt=u2dt)
                    E = ctile("pB2", 9, sA, eA)
                    cmm(E, U2, U1, sA, eA, adag=True)
                    V.tensor_add(Q.all_ap(1, 5), Q.all_ap(1, 5), l1_ap)
                    L2 = ctile("pL2", 9, 1, 5)
                    cmm(L2, D, E, 1, 5, bdag=True)
                    L2s = mat_shift(L2, a2, -1, 1, 5, "pL2s", cpool)
                    sL3, eL3 = (0, 4) if tpl else (1, 5)
                    L3 = ctile("pL3", 9, sL3, eL3)
                    cmm(L3, E, D, sL3, eL3, adag=True)
                    V.tensor_add(Q.all_ap(1, 5), Q.all_ap(1, 5),
                                 L2s.all_ap(1, 5))
                    if tpl:
                        V.tensor_add(Q.all_ap(1, 5), Q.all_ap(1, 5),
                                     L3.all_ap(0, 4))
                    else:
                        L3s = mat_shift(L3, a1, -1, 1, 5, "pLs2", cpool)
                        V.tensor_add(Q.all_ap(1, 5), Q.all_ap(1, 5),
                                     L3s.all_ap(1, 5))
                    # ---- G9 = Q - Q^dag ----
                    G9 = ctile("pG9", 9, 1, 5)
                    qv = Q.t.rearrange("p (i j r w) -> p i j r w",
                                       i=3, j=3, r=2, w=Q.W)
                    qT = qv.transpose([0, 2, 1, 3, 4])
                    gv = G9.t.rearrange("p (i j r w) -> p i j r w",
                                        i=3, j=3, r=2, w=G9.W)
                    V.tensor_sub(gv[:, :, :, 0, :], qv[:, :, :, 0, :],
                                 qT[:, :, :, 0, :])
                    V.tensor_add(gv[:, :, :, 1, :], qv[:, :, :, 1, :],
                                 qT[:, :, :, 1, :])
                    # ---- accumulate into chiral W blocks (coef = +-1;
                    #      the 1/16 is applied in the final O accumulate) ----
                    for sp in range(4):
                        wrow, phi_ph = SIGCOL[a1][a2][sp]
                        assert wrow // 2 == sp // 2, (a1, a2, sp, wrow)
                        c = -phi_ph
                        chir, spl, sl = sp // 2, sp % 2, wrow % 2
                        # re(W) += re(c) G9re - im(c) G9im
                        # im(W) += re(c) G9im + im(c) G9re
                        for outr in range(2):
                            if abs(c.imag) < 1e-6:
                                rsrc, coef = outr, c.real
                            else:
                                rsrc, coef = 1 - outr, (-c.imag if outr == 0
                                                        else c.imag)
                            wv = wslot(chir, spl, sl, outr)
                            src_ = G9.gap(0, 1, 9, rsrc, 1, 5)
                            key = (chir, spl, sl, outr)
                            if key not in _winit:
                                _winit.add(key)
                                V.tensor_scalar_mul(wv, src_, float(coef))
                            elif coef > 0:
                                V.tensor_add(wv, wv, src_)
                            else:
                                V.tensor_sub(wv, wv, src_)

        # ---------- apply chiral clover blocks: O += W psi ----------
        if _winit:
            R = mk(main, 24 * W4, "Rap")
            r3 = R.rearrange("p (q w) -> p q w", q=24, w=W4)
            f5 = F.t.rearrange("p (c sp r w) -> p c sp r w",
                               c=3, sp=4, r=2, w=F.W)
            fc0, fc1 = F.cs(1, 5)
            tv = tview4(mtA, 3, 2, W4)     # [p,4,3(j),2(l),w]
            sv = sview4(msA, 3, W4)        # [p,4,3,w]
            for chir in range(2):
                br = f5[:, :, chir * 2:chir * 2 + 2, 0, fc0:fc1]
                bi = f5[:, :, chir * 2:chir * 2 + 2, 1, fc0:fc1]
                for i in range(3):
                    for spl in range(2):
                        ar = wrow_ap(chir, i, spl, 0)
                        ai = wrow_ap(chir, i, spl, 1)
                        V.tensor_mul(tv[:, 0], ar, br)
                        V.tensor_mul(tv[:, 1], ai, bi)
                        V.tensor_mul(tv[:, 2], ar, bi)
                        V.tensor_mul(tv[:, 3], ai, br)
                        t = sv[:, :, 0:2, :]       # [p,4,2,w]
                        V.tensor_add(t, tv[:, :, 0, :, :], tv[:, :, 1, :, :])
                        V.tensor_add(t, t, tv[:, :, 2, :, :])
                        s = sv[:, :, 2, :]         # [p,4,w]
                        V.tensor_add(s, t[:, :, 0, :], t[:, :, 1, :])
                        pl = i * 8 + chir * 4 + spl * 2
                        V.tensor_sub(r3[:, pl:pl + 1, :],
                                     s[:, 0:1, :], s[:, 1:2, :])
                        V.tensor_add(r3[:, pl + 1:pl + 2, :],
                                     s[:, 2:3, :], s[:, 3:4, :])
            stt(O.t[:, :], R[:, :], 1.0 / 16.0, O.t[:, :], AL.add)

        nc.sync.dma_start(out=out_d[:, :], in_=O.t[:, :])
        psum_cm.__exit__(None, None, None)
        spool_cm.__exit__(None, None, None)
        main_cm.__exit__(None, None, None)

    nc.compile()
    return nc


# ----------------------------------------------------------------------
# host entry
# ----------------------------------------------------------------------
_CACHE = {}

def _get_nc():
    if "nc" not in _CACHE:
        _CACHE["nc"] = build_program()
    return _CACHE["nc"]


def kernel(field_re, field_im, gauge_re, gauge_im):
    from concourse.bass_utils import run_bass_kernel_spmd
    nc = _get_nc()
    in_maps = [_host_inputs(field_re, field_im, gauge_re, gauge_im, c)
               for c in range(NCORES)]
    br = run_bass_kernel_spmd(nc, in_maps, list(range(NCORES)))
    out = np.empty((T_, Z_, Y_, X_, 3, 4, 2), np.float32)
    for c in range(NCORES):
        out[c * TLOC:(c + 1) * TLOC] = _host_output(br.results[c]["outp"])
    return out


# revision 29
# speedup vs baseline: 2.3798x; 1.0657x over previous
"""Wilson-clover Dirac operator D_WC on Trainium2, 8-core SPMD.

Self-contained: hardcodes LAT=(32,16,16,16), shards the T axis across 8
cores with host-side halo slices (t0-1..t0+4), computes everything
site-locally on-device with DVE elementwise math in an SoA layout:

  plane[p, col]: p = z*8 + (y>>1),  col = plane_idx*W + (s-base)*32
                 + (y&1)*16 + x          (s = local T-slice, W = ns*32)

Shifts: T = column offset (free), X = 2-piece ScalarE copy, Z/Y =
TensorE permutation matmul (stationary 128x128 0/1 matrix) + ScalarE
copyback from PSUM.  All elementwise arithmetic runs on the DVE only
(GPSIMD shares the DVE SBUF port; running both concurrently is a net
loss).  Gauge links for all 4 directions are DMA'd into SBUF once.
"""
import numpy as np

# ----------------------------------------------------------------------
# constants (physics)
# ----------------------------------------------------------------------
MASS, C_SW = 0.5, 1.0
_i = 1j
G1 = np.array([[0,0,0,_i],[0,0,_i,0],[0,-_i,0,0],[-_i,0,0,0]], dtype=np.complex64)
G2 = np.array([[0,0,0,-1],[0,0,1,0],[0,1,0,0],[-1,0,0,0]], dtype=np.complex64)
G3 = np.array([[0,0,_i,0],[0,0,0,-_i],[-_i,0,0,0],[0,_i,0,0]], dtype=np.complex64)
G4 = np.array([[0,0,1,0],[0,0,0,1],[1,0,0,0],[0,1,0,0]], dtype=np.complex64)
GAMMA = [G1, G2, G3, G4]
I4 = np.eye(4, dtype=np.complex64)
SIGMA = [[(0.5j * (GAMMA[m] @ GAMMA[n] - GAMMA[n] @ GAMMA[m])).astype(np.complex64)
          for n in range(4)] for m in range(4)]

T_, Z_, Y_, X_ = 32, 16, 16, 16
NCORES, TLOC, NSH = 8, 4, 6       # halo slices per core
SL = 32                           # cols per slice
USE_BF16 = True                   # bf16 compute tiles (O stays fp32)
import os as _os
DBG_PART = _os.environ.get("KRN_PART", "full")    # full|wilson|clover
DBG_PAIRS = _os.environ.get("KRN_PAIRS", "")      # e.g. "12,34" to keep only
DBG_HOPS = _os.environ.get("KRN_HOPS", "0123")    # wilson dirs to keep

# permutation matrix indices (stationary operands for TensorE shifts)
PZ_P, PZ_M, PY_P, PY_M, PZY_M = 0, 1, 2, 3, 4
NPERM = 5

# ----------------------------------------------------------------------
# spin-structure extraction (numerical)
# ----------------------------------------------------------------------
def _col_struct(M):
    """M has single-nonzero columns: return per-col (row, phase)."""
    out = []
    for s in range(4):
        col = M[:, s]
        r = int(np.argmax(np.abs(col)))
        ph = complex(col[r])
        assert np.sum(np.abs(col) > 1e-6) == 1, (M, s)
        out.append((r, ph))
    return out

def _proj_struct(P):
    """P rank-2 with cols 2,3 = rho * cols q in {0,1}. Returns (q2,rho2,q3,rho3)."""
    res = []
    for s in (2, 3):
        found = None
        for j in (0, 1):
            c, cj = P[:, s], P[:, j]
            nz = np.abs(cj) > 1e-6
            if not nz.any():
                continue
            ratio = c[nz] / cj[nz]
            if np.allclose(ratio, ratio[0], atol=1e-5) and np.allclose(
                    c, ratio[0] * cj, atol=1e-5):
                found = (j, complex(ratio[0]))
                break
        assert found is not None, P
        res.append(found)
    return res

# per direction k: gamma column structure and projector relations
GCOL = [_col_struct(GAMMA[k]) for k in range(4)]           # (p(s), g(s))
PROJ_F = [_proj_struct(GAMMA[k] + I4) for k in range(4)]   # for psi@(G+I)
PROJ_B = [_proj_struct(GAMMA[k] - I4) for k in range(4)]   # for psi@(G-I)
SIGCOL = [[_col_struct(SIGMA[m][n]) if m != n else None for n in range(4)]
          for m in range(4)]

def _phase_parts(ph):
    """phase in {1,-1,i,-i} -> for v = ph*u:
    re(v) = sr*u.[pr]  im(v) = si*u.[pi]   (0=re,1=im planes of u)"""
    if abs(ph - 1) < 1e-5:   return (0, 1.0, 1, 1.0)
    if abs(ph + 1) < 1e-5:   return (0, -1.0, 1, -1.0)
    if abs(ph - 1j) < 1e-5:  return (1, -1.0, 0, 1.0)
    if abs(ph + 1j) < 1e-5:  return (1, 1.0, 0, -1.0)
    raise AssertionError(ph)

# ----------------------------------------------------------------------
# host layout helpers
# ----------------------------------------------------------------------
def _to_planes(vol):
    """vol [ns, Z, Y, X] -> [128, ns*32]"""
    ns = vol.shape[0]
    v = vol.reshape(ns, 16, 8, 2, 16)        # s z yh yl x
    v = np.transpose(v, (1, 2, 0, 3, 4))     # z yh s yl x
    return np.ascontiguousarray(v.reshape(128, ns * 32))

def _from_planes(pl, ns):
    v = pl.reshape(16, 8, ns, 2, 16)
    return np.transpose(v, (2, 0, 1, 3, 4)).reshape(ns, 16, 16, 16)

def _perm_z(d):
    """dst partition m = z*8+yh reads p = ((z+d)%16)*8+yh; P[p, m] = 1."""
    P = np.zeros((128, 128), np.float32)
    for z in range(16):
        for yh in range(8):
            P[((z + d) % 16) * 8 + yh, z * 8 + yh] = 1.0
    return P

def _perm_y(d, dz=0):
    """Y cross-half: dst m = z*8+yh reads p = ((z+dz)%16)*8+((yh+d)%8)."""
    P = np.zeros((128, 128), np.float32)
    for z in range(16):
        for yh in range(8):
            P[((z + dz) % 16) * 8 + ((yh + d) % 8), z * 8 + yh] = 1.0
    return P

def _perm_host():
    P = np.concatenate([_perm_z(+1), _perm_z(-1), _perm_y(+1), _perm_y(-1),
                        _perm_y(-1, dz=-1)], axis=1)  # [128, NPERM*128]
    return P

def _host_inputs(field_re, field_im, gauge_re, gauge_im, core):
    t0 = core * TLOC
    ts = [(t0 - 1 + s) % T_ for s in range(NSH)]
    f = np.stack([field_re[ts], field_im[ts]], axis=0)   # [2,6,Z,Y,X,3,4]
    g = np.stack([gauge_re[:, ts], gauge_im[:, ts]], axis=0)  # [2,4,6,Z,Y,X,3,3]

    # field planes: e=(c*4+sp), plane=e*2+r
    fp = np.empty((24, 128, NSH * SL), np.float32)
    for c in range(3):
        for sp in range(4):
            for r in range(2):
                fp[(c * 4 + sp) * 2 + r] = _to_planes(f[r, :, :, :, :, c, sp])
    fld = np.ascontiguousarray(fp.transpose(1, 0, 2).reshape(128, 24 * NSH * SL))

    # gauge planes: dir k, e=(a*3+b), plane=(k*9+e)*2+r
    gp = np.empty((72, 128, NSH * SL), np.float32)
    for k in range(4):
        for a in range(3):
            for b in range(3):
                for r in range(2):
                    gp[(k * 9 + a * 3 + b) * 2 + r] = _to_planes(
                        g[r, k, :, :, :, :, a, b])
    gg = np.ascontiguousarray(gp.transpose(1, 0, 2).reshape(128, 72 * NSH * SL))
    perm = _perm_host()
    if USE_BF16:
        import ml_dtypes
        fld = fld.astype(ml_dtypes.bfloat16)
        gg = gg.astype(ml_dtypes.bfloat16)
        perm = perm.astype(ml_dtypes.bfloat16)
    return {"fld": fld, "gg": gg, "perm": perm}

def _host_output(outp_flat):
    """device out [128, 24*4*32] -> [TLOC, Z,Y,X, 3,4,2]"""
    pl = outp_flat.reshape(128, 24, TLOC * SL).transpose(1, 0, 2)
    out = np.empty((TLOC, 16, 16, 16, 3, 4, 2), np.float32)
    for c in range(3):
        for sp in range(4):
            for r in range(2):
                out[..., c, sp, r] = _from_planes(pl[(c * 4 + sp) * 2 + r], TLOC)
    return out

# ----------------------------------------------------------------------
# device program
# ----------------------------------------------------------------------
class Grp:
    """Group of planes in one SBUF tile. nent complex entries (re+im planes).
    base = slice index of col 0; ns slices; W = ns*32 cols per plane."""
    def __init__(self, tile, nent, base, ns):
        self.t, self.nent, self.base, self.ns = tile, nent, base, ns
        self.W = ns * SL

    def fl(self):
        return self.t.rearrange("p (q w) -> p q w", q=self.nent * 2, w=self.W)

    def cs(self, s0, s1):
        return ((s0 - self.base) * SL, (s1 - self.base) * SL)

    def pap(self, e, r, s0, s1):
        """single plane AP [128, cols]"""
        c0, c1 = self.cs(s0, s1)
        v = self.fl()[:, (e * 2 + r):(e * 2 + r + 1), c0:c1]
        return v  # [p,1,w]

    def gap(self, e0, estep, n, r, s0, s1):
        """packed-entry AP [p, n, w]: entries e0 + i*estep, fixed r."""
        c0, c1 = self.cs(s0, s1)
        q0 = e0 * 2 + r
        fl = self.fl()
        return fl[:, q0:q0 + 2 * estep * (n - 1) + 1:2 * estep, c0:c1]

    def all_ap(self, s0, s1):
        c0, c1 = self.cs(s0, s1)
        return self.fl()[:, :, c0:c1]


def build_program():
    import concourse.bacc as bacc
    import concourse.mybir as mybir
    from concourse.tile import TileContext
    FP = mybir.dt.float32
    CDT = mybir.dt.bfloat16 if USE_BF16 else FP
    AL = mybir.AluOpType

    nc = bacc.Bacc("TRN2", target_bir_lowering=False, debug=False)
    fld_d = nc.declare_dram_parameter("fld", [128, 24 * NSH * SL], CDT, isOutput=False)
    gg_d = nc.declare_dram_parameter("gg", [128, 72 * NSH * SL], CDT, isOutput=False)
    perm_d = nc.declare_dram_parameter("perm", [128, NPERM * 128], CDT, isOutput=False)
    out_d = nc.declare_dram_parameter("outp", [128, 24 * TLOC * SL], FP, isOutput=True)

    with TileContext(nc) as tc:
        _tc = [0]

        def mk(pool, cols, tag, dt=None):
            _tc[0] += 1
            return pool.tile([128, cols], dt or CDT, tag=tag,
                             name=f"{tag}_{_tc[0]}")

        main_cm = tc.tile_pool(name="main", bufs=1)
        main = main_cm.__enter__()
        spool_cm = tc.tile_pool(name="sp", bufs=1)
        spool = spool_cm.__enter__()
        psum_cm = tc.psum_pool(name="ps", bufs=4)
        psum = psum_cm.__enter__()

        F = Grp(mk(main, 24 * NSH * SL, "F"), 12, 0, NSH)
        O = Grp(mk(main, 24 * TLOC * SL, "O", FP), 12, 1, TLOC)
        PM = mk(main, NPERM * 128, "PM")
        nc.sync.dma_start(out=F.t[:, :], in_=fld_d[:, :])
        nc.sync.dma_start(out=PM[:, :], in_=perm_d[:, :])

        # gauge links for all 4 directions, loaded once
        GL = []
        for k in range(4):
            g = Grp(mk(main, 9 * 2 * NSH * SL, f"GL{k}"), 9, 0, NSH)
            eng = nc.sync if k % 2 == 0 else nc.scalar
            eng.dma_start(out=g.t[:, :],
                          in_=gg_d[:, k * 18 * NSH * SL:(k + 1) * 18 * NSH * SL])
            GL.append(g)

        # merged scratch tiles (4 product slots each)
        WMX = 5 * SL
        mtA = mk(spool, 4 * 9 * WMX, "mtA")
        msA = mk(spool, 4 * 3 * WMX, "msA")
        asT = mk(spool, 9 * WMX, "asT")      # Gauss a-sums (ar +- ai)
        bsT = mk(spool, 9 * WMX, "bsT")      # Gauss b-sums (br +- bi)

        def tview4(t, nj, nk, w):
            """[p, 4, nj, nk, w] slots of merged mul scratch"""
            v = t.rearrange("p (h m) -> p h m", h=4)
            return v[:, :, :nj * nk * w].rearrange(
                "p h (j k w) -> p h j k w", j=nj, k=nk, w=w)

        def sview4(t, nj, w):
            v = t.rearrange("p (h m) -> p h m", h=4)
            return v[:, :, :nj * w].rearrange("p h (j w) -> p h j w",
                                              j=nj, w=w)

        V = nc.vector

        def a_pack(A, i, adag, r, s0, s1, dt, nj):
            """[p, nj(bcast), 3, w] for a-values (i,k)."""
            e0, st = (i, 3) if adag else (i * 3, 1)
            ap = A.gap(e0, st, 3, r, s0 + dt, s1 + dt)       # [p,3,w]
            w = ap.shape[2]
            return ap.unsqueeze(1).broadcast_to((128, nj, 3, w))

        def b_pack(B, bdag, r, s0, s1, dt, nj):
            """[p, nj, 3, w] for b-values (k,j)."""
            c0, c1 = B.cs(s0 + dt, s1 + dt)
            w = c1 - c0
            if bdag:  # e = j*3+k
                v = B.t.rearrange("p (j k r w) -> p j k r w", j=3, k=3, r=2, w=B.W)
                return v[:, :, :, r, c0:c1]
            if B.nent == 9:  # e = k*3+j
                v = B.t.rearrange("p (k j r w) -> p k j r w", k=3, j=3, r=2, w=B.W)
                return v[:, :, :, r, c0:c1].transpose([0, 2, 1, 3])
            # halfspinor: e = k*2+j, nj=2
            v = B.t.rearrange("p (k j r w) -> p k j r w", k=3, j=2, r=2, w=B.W)
            return v[:, :, :, r, c0:c1].transpose([0, 2, 1, 3])

        def stt(out, in0, coef, in1, op1=None):
            V.scalar_tensor_tensor(out, in0, float(coef), in1,
                                   AL.mult, op1 or AL.add)

        def cmm(dst, A, B, s0, s1, adag=False, bdag=False, adt=0, bdt=0, nj=3):
            """dst[i,j] = sum_k aval(i,k)*bval(k,j); dst entries e=i*nj+j.
            3-mult (Gauss) complex product: m1=ar*br, m2=ai*bi,
            m3=(ar+sa*ai)*(br+sb*bi); re=R1-s*R2, im=R3-R1-s*R2, s=sa*sb."""
            assert not (adag and bdag)
            w = (s1 - s0) * SL
            neg = adag or bdag
            br_ = b_pack(B, bdag, 0, s0, s1, bdt, nj)
            bi_ = b_pack(B, bdag, 1, s0, s1, bdt, nj)
            bs = bsT.rearrange("p (j k w) -> p j k w", j=3, k=3,
                               w=WMX)[:, :nj, :, :w]
            if bdag:
                V.tensor_sub(bs, br_, bi_)
            else:
                V.tensor_add(bs, br_, bi_)
            ar_all = A.gap(0, 1, 9, 0, s0 + adt, s1 + adt)
            ai_all = A.gap(0, 1, 9, 1, s0 + adt, s1 + adt)
            as3 = asT.rearrange("p (e w) -> p e w", e=9, w=WMX)[:, :, :w]
            if adag:
                V.tensor_sub(as3, ar_all, ai_all)
            else:
                V.tensor_add(as3, ar_all, ai_all)
            for i in range(3):
                tv = tview4(mtA, nj, 3, w)     # slots 0..2 used
                sv = sview4(msA, nj, w)
                ar = a_pack(A, i, adag, 0, s0, s1, adt, nj)
                ai = a_pack(A, i, adag, 1, s0, s1, adt, nj)
                if adag:
                    asb = as3[:, i:i + 7:3, :]
                else:
                    asb = as3[:, i * 3:i * 3 + 3, :]
                asb = asb.unsqueeze(1).broadcast_to((128, nj, 3, w))
                dre = dst.gap(i * nj, 1, nj, 0, s0, s1)
                dim = dst.gap(i * nj, 1, nj, 1, s0, s1)
                V.tensor_mul(tv[:, 0], ar, br_)
                V.tensor_mul(tv[:, 1], ai, bi_)
                V.tensor_mul(tv[:, 2], asb, bs)
                sv3 = sv[:, 0:3]
                V.tensor_add(sv3, tv[:, 0:3, :, 0, :], tv[:, 0:3, :, 1, :])
                V.tensor_add(sv3, sv3, tv[:, 0:3, :, 2, :])
                if neg:
                    V.tensor_add(dre, sv[:, 0], sv[:, 1])
                else:
                    V.tensor_sub(dre, sv[:, 0], sv[:, 1])
                V.tensor_sub(dim, sv[:, 2], sv[:, 0])
                if neg:
                    V.tensor_add(dim, dim, sv[:, 1])
                else:
                    V.tensor_sub(dim, dim, sv[:, 1])

        # ---------- shift materialization ----------
        def cpy(out, in_):
            nc.scalar.copy(out, in_)

        def perm_ap(idx):
            return PM.rearrange("p (i m) -> p i m", i=NPERM)[:, idx, :]

        def mm_permute(dst_t, src_t, cols, pidx, dhalf=None, shalf=None,
                       xrot=False):
            """dst = P[pidx] applied to src partitions, over [0, cols).
            dhalf/shalf: copy back only dst yl==dhalf 16-col blocks, sourced
            from permuted yl==shalf blocks.
            xrot: additionally rotate X by -1 (dst x reads src x-1)."""
            P = perm_ap(pidx)
            c0 = 0
            while c0 < cols:
                cw = min(512, cols - c0)
                pt = psum.tile([128, cw], FP, tag="pshift",
                               name=f"ps_{_tc[0]}_{c0}")
                _tc[0] += 1
                nc.tensor.matmul(pt[:, :], P, src_t[:, c0:c0 + cw])
                dv = dst_t[:, c0:c0 + cw]
                if dhalf is None and not xrot:
                    cpy(dv, pt[:, :])
                elif dhalf is None and xrot:
                    d3 = dv.rearrange("p (b x) -> p b x", x=16)
                    p3 = pt.rearrange("p (b x) -> p b x", x=16)
                    cpy(d3[:, :, 1:16], p3[:, :, 0:15])
                    cpy(d3[:, :, 0:1], p3[:, :, 15:16])
                else:
                    d4 = dv.rearrange("p (b l x) -> p b l x", l=2, x=16)
                    p4 = pt.rearrange("p (b l x) -> p b l x", l=2, x=16)
                    if not xrot:
                        cpy(d4[:, :, dhalf, :], p4[:, :, shalf, :])
                    else:
                        cpy(d4[:, :, dhalf, 1:16], p4[:, :, shalf, 0:15])
                        cpy(d4[:, :, dhalf, 0:1], p4[:, :, shalf, 15:16])
                c0 += cw

        def lv(t_, lo):
            """yl==lo 16-col blocks of flat tile view"""
            return t_.rearrange("p (m x) -> p m x", x=16)[:, lo::2, :]

        def mat_shift(src, axis, d, s0, s1, tag, pool=None):
            """materialize S(x)=src(x + d*e_axis) over the FULL src range.
            axis 1(Z) 2(Y) 3(X)."""
            g = Grp(mk(pool or main, src.nent * 2 * src.ns * SL, tag),
                    src.nent, src.base, src.ns)
            cols = src.nent * 2 * src.ns * SL
            if axis == 3:   # X: 2-piece ScalarE copy
                def xv(t_):
                    return t_.rearrange("p (m x) -> p m x", x=16)
                dv, sv_ = xv(g.t), xv(src.t)
                if d == +1:
                    cpy(dv[:, :, 0:15], sv_[:, :, 1:16])
                    cpy(dv[:, :, 15:16], sv_[:, :, 0:1])
                else:
                    cpy(dv[:, :, 1:16], sv_[:, :, 0:15])
                    cpy(dv[:, :, 0:1], sv_[:, :, 15:16])
            elif axis == 1:  # Z: pure partition permutation
                mm_permute(g.t, src.t, cols, PZ_P if d == +1 else PZ_M)
            else:           # Y: in-partition half + permuted half
                if d == +1:
                    cpy(lv(g.t, 0), lv(src.t, 1))
                    mm_permute(g.t, src.t, cols, PY_P, dhalf=1, shalf=0)
                else:
                    cpy(lv(g.t, 1), lv(src.t, 0))
                    mm_permute(g.t, src.t, cols, PY_M, dhalf=0, shalf=1)
            return g

        def mat_shift_diag(src, a1, a2, tag, pool=None):
            """S(x) = src(x - e_a1 - e_a2), a1 < a2 spatial axes (1,2,3)."""
            g = Grp(mk(pool or main, src.nent * 2 * src.ns * SL, tag),
                    src.nent, src.base, src.ns)
            cols = src.nent * 2 * src.ns * SL
            if (a1, a2) == (1, 2):    # Z,Y
                mm_permute(g.t, src.t, cols, PZ_M, dhalf=1, shalf=0)
                mm_permute(g.t, src.t, cols, PZY_M, dhalf=0, shalf=1)
            elif (a1, a2) == (1, 3):  # Z,X
                mm_permute(g.t, src.t, cols, PZ_M, xrot=True)
            else:                     # Y,X
                dv = g.t.rearrange("p (m x) -> p m x", x=16)[:, 1::2, :]
                sv_ = src.t.rearrange("p (m x) -> p m x", x=16)[:, 0::2, :]
                cpy(dv[:, :, 1:16], sv_[:, :, 0:15])
                cpy(dv[:, :, 0:1], sv_[:, :, 15:16])
                mm_permute(g.t, src.t, cols, PY_M, dhalf=0, shalf=1, xrot=True)
            return g

        # ---------- mass term: O = (MASS+4) * F ----------
        V.tensor_scalar_mul(O.all_ap(1, 5), F.all_ap(1, 5), float(MASS + 4.0))

        # clover chiral blocks W[(i,sp),(j,s')] = sum_p -(1/16) ph_p(sp)
        #   * G9_p[i,j] with s' = wrow_p(sp);  plane index =
        #   chir*72 + spl*36 + sl*18 + i*6 + j*2 + r
        W4 = 4 * SL
        WCH = mk(main, 144 * W4, "WCH")
        wv3 = WCH.rearrange("p (q w) -> p q w", q=144, w=W4)
        wv6 = WCH.rearrange("p (c s l m w) -> p c s l m w",
                            c=2, s=2, l=2, m=18, w=W4)

        def wslot(chir, spl, sl, r):
            """[p, (i,j)(9), w4] view of one spin-slot of W (3D for stt)"""
            base = chir * 72 + spl * 36 + sl * 18 + r
            return wv3[:, base:base + 17:2, :]

        def wrow_ap(chir, i, spl, r):
            """[p, j(3), sl(2), w4] contraction view of one W row"""
            m0 = i * 6 + r
            return wv6[:, chir, spl, :, m0:m0 + 5:2, :].transpose([0, 2, 1, 3])

        # ---------- Wilson hops ----------
        with tc.tile_pool(name="wp", bufs=2) as wpool:
            for k in (range(4) if DBG_PART in ("full", "wilson") else []):
                if str(k) not in DBG_HOPS:
                    continue
                ax = k  # lattice axis
                Uk = GL[k]
                # ---- forward hop ----
                rng = (0, 4) if k == 0 else (1, 5)
                s0, s1 = rng
                h = Grp(mk(wpool, 12 * (s1 - s0) * SL, "h"),
                        6, s0, s1 - s0)
                # h[c,j] = psi[c,j] + g(j)*psi[c,p(j)]
                for j in range(2):
                    pj, gj = GCOL[k][j]
                    for r in range(2):
                        pr, psgn = (_phase_parts(gj)[0:2] if r == 0
                                    else _phase_parts(gj)[2:4])
                        dst = h.gap(j, 2, 3, r, s0, s1)       # c-packed
                        a0 = F.gap(j, 4, 3, r, s0, s1)        # psi[c,j].r
                        a1 = F.gap(pj, 4, 3, pr, s0, s1)
                        if psgn > 0:
                            V.tensor_add(dst, a1, a0)
                        else:
                            V.tensor_sub(dst, a0, a1)
                phi = Grp(mk(wpool, 12 * (s1 - s0) * SL, "phi"),
                          6, s0, s1 - s0)
                cmm(phi, Uk, h, s0, s1, adag=True, nj=2)
                # shift (-1 along ax) then reconstruct into O
                if k == 0:
                    psh, dt = phi, -1
                else:
                    psh, dt = mat_shift(phi, ax, -1, 1, 5, "psh", wpool), 0
                # out[:, s'] += -0.5 * rho(s') * psh[:, q(s')]
                rec = [(0, 1.0), (1, 1.0), PROJ_F[k][0], PROJ_F[k][1]]
                for sp in range(4):
                    q, rho = rec[sp]
                    for r in range(2):
                        pr, psgn = (_phase_parts(rho)[0:2] if r == 0
                                    else _phase_parts(rho)[2:4])
                        dst = O.gap(sp, 4, 3, r, 1, 5)
                        src_ = psh.gap(q, 2, 3, pr, 1 + dt, 5 + dt)
                        stt(dst, src_, -0.5 * psgn, dst, AL.add)
                # ---- backward hop ----
                rng = (2, 6) if k == 0 else (1, 5)
                s0, s1 = rng
                hb = Grp(mk(wpool, 12 * (s1 - s0) * SL, "h"),
                         6, s0, s1 - s0)
                # hb[c,j] = g(j)*psi[c,p(j)] - psi[c,j]
                for j in range(2):
                    pj, gj = GCOL[k][j]
                    for r in range(2):
                        pr, psgn = (_phase_parts(gj)[0:2] if r == 0
                                    else _phase_parts(gj)[2:4])
                        dst = hb.gap(j, 2, 3, r, s0, s1)
                        a1 = F.gap(pj, 4, 3, pr, s0, s1)
                        a0 = F.gap(j, 4, 3, r, s0, s1)
                        if psgn > 0:
                            V.tensor_sub(dst, a1, a0)
                        else:
                            stt(dst, a1, psgn, a0, AL.subtract)
                if k == 0:
                    hs, dt = hb, +1
                else:
                    hs, dt = mat_shift(hb, ax, +1, 1, 5, "psh", wpool), 0
                gm = Grp(mk(wpool, 12 * 4 * SL, "phi"),
                         6, 1, 4)
                cmm(gm, Uk, hs, 1, 5, nj=2, bdt=dt)
                rec = [(0, 1.0), (1, 1.0), PROJ_B[k][0], PROJ_B[k][1]]
                for sp in range(4):
                    q, rho = rec[sp]
                    for r in range(2):
                        pr, psgn = (_phase_parts(rho)[0:2] if r == 0
                                    else _phase_parts(rho)[2:4])
                        dst = O.gap(sp, 4, 3, r, 1, 5)
                        src_ = gm.gap(q, 2, 3, pr, 1, 5)
                        stt(dst, src_, 0.5 * psgn, dst, AL.add)

        # ---------- clover planes ----------
        _winit = set()
        with tc.tile_pool(name="cp", bufs=1) as cpool:
            def ctile(tag, nent, s0, s1):
                return Grp(mk(cpool, nent * 2 * (s1 - s0) * SL, tag), nent, s0, s1 - s0)

            for d1 in (range(1, 5) if DBG_PART in ("full", "clover") else []):
                for d2 in range(d1 + 1, 5):
                    if DBG_PAIRS and f"{d1}{d2}" not in DBG_PAIRS.split(","):
                        continue
                    a1, a2 = d1 - 1, d2 - 1
                    tpl = (d1 == 1)
                    sA, eA = (0, 5) if tpl else (1, 5)
                    U1 = GL[d1 - 1]
                    U2 = GL[d2 - 1]
                    # shifted links
                    if tpl:
                        U2m, u2dt = U2, +1       # U_d2(x+e_T): column view
                    else:
                        U2m, u2dt = mat_shift(U2, a1, +1, sA, eA, "lnk1", cpool), 0
                    U1n = mat_shift(U1, a2, +1, sA, eA, "lnk2", cpool)
                    A = ctile("pA", 9, sA, eA)
                    cmm(A, U1, U2m, sA, eA, bdt=u2dt)
                    B = ctile("pB", 9, sA, eA)
                    cmm(B, U2, U1n, sA, eA)
                    Q = ctile("pQ", 9, 1, 5)
                    cmm(Q, B, A, 1, 5, bdag=True)            # L4
                    sL1, eL1 = (0, 4) if tpl else (1, 5)
                    L1 = ctile("pL", 9, sL1, eL1)
                    cmm(L1, A, B, sL1, eL1, adag=True)
                    # L1 shift overlaps D/E products; Q-adds deferred
                    if tpl:
                        L1s = mat_shift(L1, a2, -1, 0, 4, "pLs", cpool)
                        l1_ap = L1s.all_ap(0, 4)
                    else:
                        L1ss = mat_shift_diag(L1, a1, a2, "pLs", cpool)
                        l1_ap = L1ss.all_ap(1, 5)
                    D = ctile("pA2", 9, sA, eA)
                    cmm(D, U1n, U2m, sA, eA, bdag=True, bdt=u2dt)
                    E = ctile("pB2", 9, sA, eA)
                    cmm(E, U2, U1, sA, eA, adag=True)
                    V.tensor_add(Q.all_ap(1, 5), Q.all_ap(1, 5), l1_ap)
                    L2 = ctile("pL2", 9, 1, 5)
                    cmm(L2, D, E, 1, 5, bdag=True)
                    L2s = mat_shift(L2, a2, -1, 1, 5, "pL2s", cpool)
                    sL3, eL3 = (0, 4) if tpl else (1, 5)
                    L3 = ctile("pL3", 9, sL3, eL3)
                    cmm(L3, E, D, sL3, eL3, adag=True)
                    V.tensor_add(Q.all_ap(1, 5), Q.all_ap(1, 5),
                                 L2s.all_ap(1, 5))
                    if tpl:
                        V.tensor_add(Q.all_ap(1, 5), Q.all_ap(1, 5),
                                     L3.all_ap(0, 4))
                    else:
                        L3s = mat_shift(L3, a1, -1, 1, 5, "pLs2", cpool)
                        V.tensor_add(Q.all_ap(1, 5), Q.all_ap(1, 5),
                                     L3s.all_ap(1, 5))
                    # ---- G9 = Q - Q^dag ----
                    G9 = ctile("pG9", 9, 1, 5)
                    qv = Q.t.rearrange("p (i j r w) -> p i j r w",
                                       i=3, j=3, r=2, w=Q.W)
                    qT = qv.transpose([0, 2, 1, 3, 4])
                    gv = G9.t.rearrange("p (i j r w) -> p i j r w",
                                        i=3, j=3, r=2, w=G9.W)
                    V.tensor_sub(gv[:, :, :, 0, :], qv[:, :, :, 0, :],
                                 qT[:, :, :, 0, :])
                    V.tensor_add(gv[:, :, :, 1, :], qv[:, :, :, 1, :],
                                 qT[:, :, :, 1, :])
                    # ---- accumulate into chiral W blocks (coef = +-1;
                    #      the 1/16 is applied in the final O accumulate) ----
                    for sp in range(4):
                        wrow, phi_ph = SIGCOL[a1][a2][sp]
                        assert wrow // 2 == sp // 2, (a1, a2, sp, wrow)
                        c = -phi_ph
                        chir, spl, sl = sp // 2, sp % 2, wrow % 2
                        # re(W) += re(c) G9re - im(c) G9im
                        # im(W) += re(c) G9im + im(c) G9re
                        for outr in range(2):
                            if abs(c.imag) < 1e-6:
                                rsrc, coef = outr, c.real
                            else:
                                rsrc, coef = 1 - outr, (-c.imag if outr == 0
                                                        else c.imag)
                            wv = wslot(chir, spl, sl, outr)
                            src_ = G9.gap(0, 1, 9, rsrc, 1, 5)
                            key = (chir, spl, sl, outr)
                            if key not in _winit:
                                _winit.add(key)
                                V.tensor_scalar_mul(wv, src_, float(coef))
                            elif coef > 0:
                                V.tensor_add(wv, wv, src_)
                            else:
                                V.tensor_sub(wv, wv, src_)

        # ---------- apply chiral clover blocks: O += W psi ----------
        if _winit:
            R = mk(main, 24 * W4, "Rap")
            r3 = R.rearrange("p (q w) -> p q w", q=24, w=W4)
            f5 = F.t.rearrange("p (c sp r w) -> p c sp r w",
                               c=3, sp=4, r=2, w=F.W)
            fc0, fc1 = F.cs(1, 5)
            tv = tview4(mtA, 3, 2, W4)     # [p,4,3(j),2(l),w]
            sv = sview4(msA, 3, W4)        # [p,4,3,w]
            for chir in range(2):
                br = f5[:, :, chir * 2:chir * 2 + 2, 0, fc0:fc1]
                bi = f5[:, :, chir * 2:chir * 2 + 2, 1, fc0:fc1]
                for i in range(3):
                    for spl in range(2):
                        ar = wrow_ap(chir, i, spl, 0)
                        ai = wrow_ap(chir, i, spl, 1)
                        V.tensor_mul(tv[:, 0], ar, br)
                        V.tensor_mul(tv[:, 1], ai, bi)
                        V.tensor_mul(tv[:, 2], ar, bi)
                        V.tensor_mul(tv[:, 3], ai, br)
                        t = sv[:, :, 0:2, :]       # [p,4,2,w]
                        V.tensor_add(t, tv[:, :, 0, :, :], tv[:, :, 1, :, :])
                        V.tensor_add(t, t, tv[:, :, 2, :, :])
                        s = sv[:, :, 2, :]         # [p,4,w]
                        V.tensor_add(s, t[:, :, 0, :], t[:, :, 1, :])
                        pl = i * 8 + chir * 4 + spl * 2
                        V.tensor_sub(r3[:, pl:pl + 1, :],
                                     s[:, 0:1, :], s[:, 1:2, :])
                        V.tensor_add(r3[:, pl + 1:pl + 2, :],
                                     s[:, 2:3, :], s[:, 3:4, :])
            stt(O.t[:, :], R[:, :], 1.0 / 16.0, O.t[:, :], AL.add)

        nc.sync.dma_start(out=out_d[:, :], in_=O.t[:, :])
        psum_cm.__exit__(None, None, None)
        spool_cm.__exit__(None, None, None)
        main_cm.__exit__(None, None, None)

    nc.compile()
    return nc


# ----------------------------------------------------------------------
# host entry
# ----------------------------------------------------------------------
_CACHE = {}

def _get_nc():
    if "nc" not in _CACHE:
        _CACHE["nc"] = build_program()
    return _CACHE["nc"]


def kernel(field_re, field_im, gauge_re, gauge_im):
    from concourse.bass_utils import run_bass_kernel_spmd
    nc = _get_nc()
    in_maps = [_host_inputs(field_re, field_im, gauge_re, gauge_im, c)
               for c in range(NCORES)]
    br = run_bass_kernel_spmd(nc, in_maps, list(range(NCORES)))
    out = np.empty((T_, Z_, Y_, X_, 3, 4, 2), np.float32)
    for c in range(NCORES):
        out[c * TLOC:(c + 1) * TLOC] = _host_output(br.results[c]["outp"])
    return out


# revision 31
# speedup vs baseline: 2.3969x; 1.0072x over previous
"""Wilson-clover Dirac operator D_WC on Trainium2, 8-core SPMD.

Self-contained: hardcodes LAT=(32,16,16,16), shards the T axis across 8
cores with host-side halo slices (t0-1..t0+4), computes everything
site-locally on-device with DVE elementwise math in an SoA layout:

  plane[p, col]: p = z*8 + (y>>1),  col = plane_idx*W + (s-base)*32
                 + (y&1)*16 + x          (s = local T-slice, W = ns*32)

Shifts: T = column offset (free), X = 2-piece ScalarE copy, Z/Y =
TensorE permutation matmul (stationary 128x128 0/1 matrix) + ScalarE
copyback from PSUM.  All elementwise arithmetic runs on the DVE only
(GPSIMD shares the DVE SBUF port; running both concurrently is a net
loss).  Gauge links for all 4 directions are DMA'd into SBUF once.
"""
import numpy as np

# ----------------------------------------------------------------------
# constants (physics)
# ----------------------------------------------------------------------
MASS, C_SW = 0.5, 1.0
_i = 1j
G1 = np.array([[0,0,0,_i],[0,0,_i,0],[0,-_i,0,0],[-_i,0,0,0]], dtype=np.complex64)
G2 = np.array([[0,0,0,-1],[0,0,1,0],[0,1,0,0],[-1,0,0,0]], dtype=np.complex64)
G3 = np.array([[0,0,_i,0],[0,0,0,-_i],[-_i,0,0,0],[0,_i,0,0]], dtype=np.complex64)
G4 = np.array([[0,0,1,0],[0,0,0,1],[1,0,0,0],[0,1,0,0]], dtype=np.complex64)
GAMMA = [G1, G2, G3, G4]
I4 = np.eye(4, dtype=np.complex64)
SIGMA = [[(0.5j * (GAMMA[m] @ GAMMA[n] - GAMMA[n] @ GAMMA[m])).astype(np.complex64)
          for n in range(4)] for m in range(4)]

T_, Z_, Y_, X_ = 32, 16, 16, 16
NCORES, TLOC, NSH = 8, 4, 6       # halo slices per core
SL = 32                           # cols per slice
USE_BF16 = True                   # bf16 compute tiles (O stays fp32)
import os as _os
DBG_PART = _os.environ.get("KRN_PART", "full")    # full|wilson|clover
DBG_PAIRS = _os.environ.get("KRN_PAIRS", "")      # e.g. "12,34" to keep only
DBG_HOPS = _os.environ.get("KRN_HOPS", "0123")    # wilson dirs to keep

# permutation matrix indices (stationary operands for TensorE shifts)
PZ_P, PZ_M, PY_P, PY_M, PZY_M = 0, 1, 2, 3, 4
NPERM = 5

# ----------------------------------------------------------------------
# spin-structure extraction (numerical)
# ----------------------------------------------------------------------
def _col_struct(M):
    """M has single-nonzero columns: return per-col (row, phase)."""
    out = []
    for s in range(4):
        col = M[:, s]
        r = int(np.argmax(np.abs(col)))
        ph = complex(col[r])
        assert np.sum(np.abs(col) > 1e-6) == 1, (M, s)
        out.append((r, ph))
    return out

def _proj_struct(P):
    """P rank-2 with cols 2,3 = rho * cols q in {0,1}. Returns (q2,rho2,q3,rho3)."""
    res = []
    for s in (2, 3):
        found = None
        for j in (0, 1):
            c, cj = P[:, s], P[:, j]
            nz = np.abs(cj) > 1e-6
            if not nz.any():
                continue
            ratio = c[nz] / cj[nz]
            if np.allclose(ratio, ratio[0], atol=1e-5) and np.allclose(
                    c, ratio[0] * cj, atol=1e-5):
                found = (j, complex(ratio[0]))
                break
        assert found is not None, P
        res.append(found)
    return res

# per direction k: gamma column structure and projector relations
GCOL = [_col_struct(GAMMA[k]) for k in range(4)]           # (p(s), g(s))
PROJ_F = [_proj_struct(GAMMA[k] + I4) for k in range(4)]   # for psi@(G+I)
PROJ_B = [_proj_struct(GAMMA[k] - I4) for k in range(4)]   # for psi@(G-I)
SIGCOL = [[_col_struct(SIGMA[m][n]) if m != n else None for n in range(4)]
          for m in range(4)]

def _phase_parts(ph):
    """phase in {1,-1,i,-i} -> for v = ph*u:
    re(v) = sr*u.[pr]  im(v) = si*u.[pi]   (0=re,1=im planes of u)"""
    if abs(ph - 1) < 1e-5:   return (0, 1.0, 1, 1.0)
    if abs(ph + 1) < 1e-5:   return (0, -1.0, 1, -1.0)
    if abs(ph - 1j) < 1e-5:  return (1, -1.0, 0, 1.0)
    if abs(ph + 1j) < 1e-5:  return (1, 1.0, 0, -1.0)
    raise AssertionError(ph)

# ----------------------------------------------------------------------
# host layout helpers
# ----------------------------------------------------------------------
def _to_planes(vol):
    """vol [ns, Z, Y, X] -> [128, ns*32]"""
    ns = vol.shape[0]
    v = vol.reshape(ns, 16, 8, 2, 16)        # s z yh yl x
    v = np.transpose(v, (1, 2, 0, 3, 4))     # z yh s yl x
    return np.ascontiguousarray(v.reshape(128, ns * 32))

def _from_planes(pl, ns):
    v = pl.reshape(16, 8, ns, 2, 16)
    return np.transpose(v, (2, 0, 1, 3, 4)).reshape(ns, 16, 16, 16)

def _perm_z(d):
    """dst partition m = z*8+yh reads p = ((z+d)%16)*8+yh; P[p, m] = 1."""
    P = np.zeros((128, 128), np.float32)
    for z in range(16):
        for yh in range(8):
            P[((z + d) % 16) * 8 + yh, z * 8 + yh] = 1.0
    return P

def _perm_y(d, dz=0):
    """Y cross-half: dst m = z*8+yh reads p = ((z+dz)%16)*8+((yh+d)%8)."""
    P = np.zeros((128, 128), np.float32)
    for z in range(16):
        for yh in range(8):
            P[((z + dz) % 16) * 8 + ((yh + d) % 8), z * 8 + yh] = 1.0
    return P

def _perm_host():
    P = np.concatenate([_perm_z(+1), _perm_z(-1), _perm_y(+1), _perm_y(-1),
                        _perm_y(-1, dz=-1)], axis=1)  # [128, NPERM*128]
    return P

def _host_inputs(field_re, field_im, gauge_re, gauge_im, core):
    t0 = core * TLOC
    ts = [(t0 - 1 + s) % T_ for s in range(NSH)]
    f = np.stack([field_re[ts], field_im[ts]], axis=0)   # [2,6,Z,Y,X,3,4]
    g = np.stack([gauge_re[:, ts], gauge_im[:, ts]], axis=0)  # [2,4,6,Z,Y,X,3,3]

    # field planes: e=(c*4+sp), plane=e*2+r
    fp = np.empty((24, 128, NSH * SL), np.float32)
    for c in range(3):
        for sp in range(4):
            for r in range(2):
                fp[(c * 4 + sp) * 2 + r] = _to_planes(f[r, :, :, :, :, c, sp])
    fld = np.ascontiguousarray(fp.transpose(1, 0, 2).reshape(128, 24 * NSH * SL))

    # gauge planes: dir k, e=(a*3+b), plane=(k*9+e)*2+r
    gp = np.empty((72, 128, NSH * SL), np.float32)
    for k in range(4):
        for a in range(3):
            for b in range(3):
                for r in range(2):
                    gp[(k * 9 + a * 3 + b) * 2 + r] = _to_planes(
                        g[r, k, :, :, :, :, a, b])
    gg = np.ascontiguousarray(gp.transpose(1, 0, 2).reshape(128, 72 * NSH * SL))
    perm = _perm_host()
    if USE_BF16:
        import ml_dtypes
        fld = fld.astype(ml_dtypes.bfloat16)
        gg = gg.astype(ml_dtypes.bfloat16)
        perm = perm.astype(ml_dtypes.bfloat16)
    return {"fld": fld, "gg": gg, "perm": perm}

def _host_output(outp_flat):
    """device out [128, 24*4*32] -> [TLOC, Z,Y,X, 3,4,2]"""
    pl = outp_flat.reshape(128, 24, TLOC * SL).transpose(1, 0, 2)
    out = np.empty((TLOC, 16, 16, 16, 3, 4, 2), np.float32)
    for c in range(3):
        for sp in range(4):
            for r in range(2):
                out[..., c, sp, r] = _from_planes(pl[(c * 4 + sp) * 2 + r], TLOC)
    return out

# ----------------------------------------------------------------------
# device program
# ----------------------------------------------------------------------
class Grp:
    """Group of planes in one SBUF tile. nent complex entries (re+im planes).
    base = slice index of col 0; ns slices; W = ns*32 cols per plane."""
    def __init__(self, tile, nent, base, ns):
        self.t, self.nent, self.base, self.ns = tile, nent, base, ns
        self.W = ns * SL

    def fl(self):
        return self.t.rearrange("p (q w) -> p q w", q=self.nent * 2, w=self.W)

    def cs(self, s0, s1):
        return ((s0 - self.base) * SL, (s1 - self.base) * SL)

    def pap(self, e, r, s0, s1):
        """single plane AP [128, cols]"""
        c0, c1 = self.cs(s0, s1)
        v = self.fl()[:, (e * 2 + r):(e * 2 + r + 1), c0:c1]
        return v  # [p,1,w]

    def gap(self, e0, estep, n, r, s0, s1):
        """packed-entry AP [p, n, w]: entries e0 + i*estep, fixed r."""
        c0, c1 = self.cs(s0, s1)
        q0 = e0 * 2 + r
        fl = self.fl()
        return fl[:, q0:q0 + 2 * estep * (n - 1) + 1:2 * estep, c0:c1]

    def all_ap(self, s0, s1):
        c0, c1 = self.cs(s0, s1)
        return self.fl()[:, :, c0:c1]


def build_program():
    import concourse.bacc as bacc
    import concourse.mybir as mybir
    from concourse.tile import TileContext
    FP = mybir.dt.float32
    CDT = mybir.dt.bfloat16 if USE_BF16 else FP
    AL = mybir.AluOpType

    nc = bacc.Bacc("TRN2", target_bir_lowering=False, debug=False)
    fld_d = nc.declare_dram_parameter("fld", [128, 24 * NSH * SL], CDT, isOutput=False)
    gg_d = nc.declare_dram_parameter("gg", [128, 72 * NSH * SL], CDT, isOutput=False)
    perm_d = nc.declare_dram_parameter("perm", [128, NPERM * 128], CDT, isOutput=False)
    out_d = nc.declare_dram_parameter("outp", [128, 24 * TLOC * SL], FP, isOutput=True)

    with TileContext(nc) as tc:
        _tc = [0]

        def mk(pool, cols, tag, dt=None):
            _tc[0] += 1
            return pool.tile([128, cols], dt or CDT, tag=tag,
                             name=f"{tag}_{_tc[0]}")

        main_cm = tc.tile_pool(name="main", bufs=1)
        main = main_cm.__enter__()
        spool_cm = tc.tile_pool(name="sp", bufs=1)
        spool = spool_cm.__enter__()
        psum_cm = tc.psum_pool(name="ps", bufs=4)
        psum = psum_cm.__enter__()

        F = Grp(mk(main, 24 * NSH * SL, "F"), 12, 0, NSH)
        O = Grp(mk(main, 24 * TLOC * SL, "O", FP), 12, 1, TLOC)
        PM = mk(main, NPERM * 128, "PM")
        FH = 12 * NSH * SL
        nc.sync.dma_start(out=F.t[:, :FH], in_=fld_d[:, :FH])
        nc.scalar.dma_start(out=F.t[:, FH:], in_=fld_d[:, FH:])
        nc.sync.dma_start(out=PM[:, :], in_=perm_d[:, :])

        # gauge links for all 4 directions, loaded once
        GL = []
        for k in range(4):
            g = Grp(mk(main, 9 * 2 * NSH * SL, f"GL{k}"), 9, 0, NSH)
            eng = nc.sync if k % 2 == 0 else nc.scalar
            eng.dma_start(out=g.t[:, :],
                          in_=gg_d[:, k * 18 * NSH * SL:(k + 1) * 18 * NSH * SL])
            GL.append(g)

        # merged scratch tiles (4 product slots each)
        WMX = 5 * SL
        mtA = mk(spool, 4 * 9 * WMX, "mtA")
        msA = mk(spool, 4 * 3 * WMX, "msA")
        asT = mk(spool, 9 * WMX, "asT")      # Gauss a-sums (ar +- ai)
        bsT = mk(spool, 9 * WMX, "bsT")      # Gauss b-sums (br +- bi)

        def tview4(t, nj, nk, w):
            """[p, 4, nj, nk, w] slots of merged mul scratch"""
            v = t.rearrange("p (h m) -> p h m", h=4)
            return v[:, :, :nj * nk * w].rearrange(
                "p h (j k w) -> p h j k w", j=nj, k=nk, w=w)

        def sview4(t, nj, w):
            v = t.rearrange("p (h m) -> p h m", h=4)
            return v[:, :, :nj * w].rearrange("p h (j w) -> p h j w",
                                              j=nj, w=w)

        V = nc.vector

        def a_pack(A, i, adag, r, s0, s1, dt, nj):
            """[p, nj(bcast), 3, w] for a-values (i,k)."""
            e0, st = (i, 3) if adag else (i * 3, 1)
            ap = A.gap(e0, st, 3, r, s0 + dt, s1 + dt)       # [p,3,w]
            w = ap.shape[2]
            return ap.unsqueeze(1).broadcast_to((128, nj, 3, w))

        def b_pack(B, bdag, r, s0, s1, dt, nj):
            """[p, nj, 3, w] for b-values (k,j)."""
            c0, c1 = B.cs(s0 + dt, s1 + dt)
            w = c1 - c0
            if bdag:  # e = j*3+k
                v = B.t.rearrange("p (j k r w) -> p j k r w", j=3, k=3, r=2, w=B.W)
                return v[:, :, :, r, c0:c1]
            if B.nent == 9:  # e = k*3+j
                v = B.t.rearrange("p (k j r w) -> p k j r w", k=3, j=3, r=2, w=B.W)
                return v[:, :, :, r, c0:c1].transpose([0, 2, 1, 3])
            # halfspinor: e = k*2+j, nj=2
            v = B.t.rearrange("p (k j r w) -> p k j r w", k=3, j=2, r=2, w=B.W)
            return v[:, :, :, r, c0:c1].transpose([0, 2, 1, 3])

        def stt(out, in0, coef, in1, op1=None):
            V.scalar_tensor_tensor(out, in0, float(coef), in1,
                                   AL.mult, op1 or AL.add)

        def cmm(dst, A, B, s0, s1, adag=False, bdag=False, adt=0, bdt=0, nj=3):
            """dst[i,j] = sum_k aval(i,k)*bval(k,j); dst entries e=i*nj+j.
            3-mult (Gauss) complex product: m1=ar*br, m2=ai*bi,
            m3=(ar+sa*ai)*(br+sb*bi); re=R1-s*R2, im=R3-R1-s*R2, s=sa*sb."""
            assert not (adag and bdag)
            w = (s1 - s0) * SL
            neg = adag or bdag
            br_ = b_pack(B, bdag, 0, s0, s1, bdt, nj)
            bi_ = b_pack(B, bdag, 1, s0, s1, bdt, nj)
            bs = bsT.rearrange("p (j k w) -> p j k w", j=3, k=3,
                               w=WMX)[:, :nj, :, :w]
            if bdag:
                V.tensor_sub(bs, br_, bi_)
            else:
                V.tensor_add(bs, br_, bi_)
            ar_all = A.gap(0, 1, 9, 0, s0 + adt, s1 + adt)
            ai_all = A.gap(0, 1, 9, 1, s0 + adt, s1 + adt)
            as3 = asT.rearrange("p (e w) -> p e w", e=9, w=WMX)[:, :, :w]
            if adag:
                V.tensor_sub(as3, ar_all, ai_all)
            else:
                V.tensor_add(as3, ar_all, ai_all)
            for i in range(3):
                tv = tview4(mtA, nj, 3, w)     # slots 0..2 used
                sv = sview4(msA, nj, w)
                ar = a_pack(A, i, adag, 0, s0, s1, adt, nj)
                ai = a_pack(A, i, adag, 1, s0, s1, adt, nj)
                if adag:
                    asb = as3[:, i:i + 7:3, :]
                else:
                    asb = as3[:, i * 3:i * 3 + 3, :]
                asb = asb.unsqueeze(1).broadcast_to((128, nj, 3, w))
                dre = dst.gap(i * nj, 1, nj, 0, s0, s1)
                dim = dst.gap(i * nj, 1, nj, 1, s0, s1)
                V.tensor_mul(tv[:, 0], ar, br_)
                V.tensor_mul(tv[:, 1], ai, bi_)
                V.tensor_mul(tv[:, 2], asb, bs)
                sv3 = sv[:, 0:3]
                V.tensor_add(sv3, tv[:, 0:3, :, 0, :], tv[:, 0:3, :, 1, :])
                V.tensor_add(sv3, sv3, tv[:, 0:3, :, 2, :])
                if neg:
                    V.tensor_add(dre, sv[:, 0], sv[:, 1])
                else:
                    V.tensor_sub(dre, sv[:, 0], sv[:, 1])
                V.tensor_sub(dim, sv[:, 2], sv[:, 0])
                if neg:
                    V.tensor_add(dim, dim, sv[:, 1])
                else:
                    V.tensor_sub(dim, dim, sv[:, 1])

        # ---------- shift materialization ----------
        def cpy(out, in_):
            nc.scalar.copy(out, in_)

        def perm_ap(idx):
            return PM.rearrange("p (i m) -> p i m", i=NPERM)[:, idx, :]

        def mm_permute(dst_t, src_t, cols, pidx, dhalf=None, shalf=None,
                       xrot=False):
            """dst = P[pidx] applied to src partitions, over [0, cols).
            dhalf/shalf: copy back only dst yl==dhalf 16-col blocks, sourced
            from permuted yl==shalf blocks.
            xrot: additionally rotate X by -1 (dst x reads src x-1)."""
            P = perm_ap(pidx)
            c0 = 0
            while c0 < cols:
                cw = min(512, cols - c0)
                pt = psum.tile([128, cw], FP, tag="pshift",
                               name=f"ps_{_tc[0]}_{c0}")
                _tc[0] += 1
                nc.tensor.matmul(pt[:, :], P, src_t[:, c0:c0 + cw])
                dv = dst_t[:, c0:c0 + cw]
                if dhalf is None and not xrot:
                    cpy(dv, pt[:, :])
                elif dhalf is None and xrot:
                    d3 = dv.rearrange("p (b x) -> p b x", x=16)
                    p3 = pt.rearrange("p (b x) -> p b x", x=16)
                    cpy(d3[:, :, 1:16], p3[:, :, 0:15])
                    cpy(d3[:, :, 0:1], p3[:, :, 15:16])
                else:
                    d4 = dv.rearrange("p (b l x) -> p b l x", l=2, x=16)
                    p4 = pt.rearrange("p (b l x) -> p b l x", l=2, x=16)
                    if not xrot:
                        cpy(d4[:, :, dhalf, :], p4[:, :, shalf, :])
                    else:
                        cpy(d4[:, :, dhalf, 1:16], p4[:, :, shalf, 0:15])
                        cpy(d4[:, :, dhalf, 0:1], p4[:, :, shalf, 15:16])
                c0 += cw

        def lv(t_, lo):
            """yl==lo 16-col blocks of flat tile view"""
            return t_.rearrange("p (m x) -> p m x", x=16)[:, lo::2, :]

        def mat_shift(src, axis, d, s0, s1, tag, pool=None):
            """materialize S(x)=src(x + d*e_axis) over the FULL src range.
            axis 1(Z) 2(Y) 3(X)."""
            g = Grp(mk(pool or main, src.nent * 2 * src.ns * SL, tag),
                    src.nent, src.base, src.ns)
            cols = src.nent * 2 * src.ns * SL
            if axis == 3:   # X: 2-piece ScalarE copy
                def xv(t_):
                    return t_.rearrange("p (m x) -> p m x", x=16)
                dv, sv_ = xv(g.t), xv(src.t)
                if d == +1:
                    cpy(dv[:, :, 0:15], sv_[:, :, 1:16])
                    cpy(dv[:, :, 15:16], sv_[:, :, 0:1])
                else:
                    cpy(dv[:, :, 1:16], sv_[:, :, 0:15])
                    cpy(dv[:, :, 0:1], sv_[:, :, 15:16])
            elif axis == 1:  # Z: pure partition permutation
                mm_permute(g.t, src.t, cols, PZ_P if d == +1 else PZ_M)
            else:           # Y: in-partition half + permuted half
                if d == +1:
                    cpy(lv(g.t, 0), lv(src.t, 1))
                    mm_permute(g.t, src.t, cols, PY_P, dhalf=1, shalf=0)
                else:
                    cpy(lv(g.t, 1), lv(src.t, 0))
                    mm_permute(g.t, src.t, cols, PY_M, dhalf=0, shalf=1)
            return g

        def mat_shift_diag(src, a1, a2, tag, pool=None):
            """S(x) = src(x - e_a1 - e_a2), a1 < a2 spatial axes (1,2,3)."""
            g = Grp(mk(pool or main, src.nent * 2 * src.ns * SL, tag),
                    src.nent, src.base, src.ns)
            cols = src.nent * 2 * src.ns * SL
            if (a1, a2) == (1, 2):    # Z,Y
                mm_permute(g.t, src.t, cols, PZ_M, dhalf=1, shalf=0)
                mm_permute(g.t, src.t, cols, PZY_M, dhalf=0, shalf=1)
            elif (a1, a2) == (1, 3):  # Z,X
                mm_permute(g.t, src.t, cols, PZ_M, xrot=True)
            else:                     # Y,X
                dv = g.t.rearrange("p (m x) -> p m x", x=16)[:, 1::2, :]
                sv_ = src.t.rearrange("p (m x) -> p m x", x=16)[:, 0::2, :]
                cpy(dv[:, :, 1:16], sv_[:, :, 0:15])
                cpy(dv[:, :, 0:1], sv_[:, :, 15:16])
                mm_permute(g.t, src.t, cols, PY_M, dhalf=0, shalf=1, xrot=True)
            return g

        # ---------- mass term: O = (MASS+4) * F ----------
        V.tensor_scalar_mul(O.all_ap(1, 5), F.all_ap(1, 5), float(MASS + 4.0))

        # clover chiral blocks W[(i,sp),(j,s')] = sum_p -(1/16) ph_p(sp)
        #   * G9_p[i,j] with s' = wrow_p(sp);  plane index =
        #   chir*72 + spl*36 + sl*18 + i*6 + j*2 + r
        W4 = 4 * SL
        WCH = mk(main, 144 * W4, "WCH")
        wv3 = WCH.rearrange("p (q w) -> p q w", q=144, w=W4)
        wv6 = WCH.rearrange("p (c s l m w) -> p c s l m w",
                            c=2, s=2, l=2, m=18, w=W4)

        def wslot(chir, spl, sl, r):
            """[p, (i,j)(9), w4] view of one spin-slot of W (3D for stt)"""
            base = chir * 72 + spl * 36 + sl * 18 + r
            return wv3[:, base:base + 17:2, :]

        def wrow_ap(chir, i, spl, r):
            """[p, j(3), sl(2), w4] contraction view of one W row"""
            m0 = i * 6 + r
            return wv6[:, chir, spl, :, m0:m0 + 5:2, :].transpose([0, 2, 1, 3])

        # ---------- Wilson hops ----------
        with tc.tile_pool(name="wp", bufs=2) as wpool:
            for k in (range(4) if DBG_PART in ("full", "wilson") else []):
                if str(k) not in DBG_HOPS:
                    continue
                ax = k  # lattice axis
                Uk = GL[k]
                # ---- forward hop ----
                rng = (0, 4) if k == 0 else (1, 5)
                s0, s1 = rng
                h = Grp(mk(wpool, 12 * (s1 - s0) * SL, "h"),
                        6, s0, s1 - s0)
                # h[c,j] = psi[c,j] + g(j)*psi[c,p(j)]
                for j in range(2):
                    pj, gj = GCOL[k][j]
                    for r in range(2):
                        pr, psgn = (_phase_parts(gj)[0:2] if r == 0
                                    else _phase_parts(gj)[2:4])
                        dst = h.gap(j, 2, 3, r, s0, s1)       # c-packed
                        a0 = F.gap(j, 4, 3, r, s0, s1)        # psi[c,j].r
                        a1 = F.gap(pj, 4, 3, pr, s0, s1)
                        if psgn > 0:
                            V.tensor_add(dst, a1, a0)
                        else:
                            V.tensor_sub(dst, a0, a1)
                phi = Grp(mk(wpool, 12 * (s1 - s0) * SL, "phi"),
                          6, s0, s1 - s0)
                cmm(phi, Uk, h, s0, s1, adag=True, nj=2)
                # shift (-1 along ax) then reconstruct into O
                if k == 0:
                    psh, dt = phi, -1
                else:
                    psh, dt = mat_shift(phi, ax, -1, 1, 5, "psh", wpool), 0
                # out[:, s'] += -0.5 * rho(s') * psh[:, q(s')]
                rec = [(0, 1.0), (1, 1.0), PROJ_F[k][0], PROJ_F[k][1]]
                for sp in range(4):
                    q, rho = rec[sp]
                    for r in range(2):
                        pr, psgn = (_phase_parts(rho)[0:2] if r == 0
                                    else _phase_parts(rho)[2:4])
                        dst = O.gap(sp, 4, 3, r, 1, 5)
                        src_ = psh.gap(q, 2, 3, pr, 1 + dt, 5 + dt)
                        stt(dst, src_, -0.5 * psgn, dst, AL.add)
                # ---- backward hop ----
                rng = (2, 6) if k == 0 else (1, 5)
                s0, s1 = rng
                hb = Grp(mk(wpool, 12 * (s1 - s0) * SL, "h"),
                         6, s0, s1 - s0)
                # hb[c,j] = g(j)*psi[c,p(j)] - psi[c,j]
                for j in range(2):
                    pj, gj = GCOL[k][j]
                    for r in range(2):
                        pr, psgn = (_phase_parts(gj)[0:2] if r == 0
                                    else _phase_parts(gj)[2:4])
                        dst = hb.gap(j, 2, 3, r, s0, s1)
                        a1 = F.gap(pj, 4, 3, pr, s0, s1)
                        a0 = F.gap(j, 4, 3, r, s0, s1)
                        if psgn > 0:
                            V.tensor_sub(dst, a1, a0)
                        else:
                            stt(dst, a1, psgn, a0, AL.subtract)
                if k == 0:
                    hs, dt = hb, +1
                else:
                    hs, dt = mat_shift(hb, ax, +1, 1, 5, "psh", wpool), 0
                gm = Grp(mk(wpool, 12 * 4 * SL, "phi"),
                         6, 1, 4)
                cmm(gm, Uk, hs, 1, 5, nj=2, bdt=dt)
                rec = [(0, 1.0), (1, 1.0), PROJ_B[k][0], PROJ_B[k][1]]
                for sp in range(4):
                    q, rho = rec[sp]
                    for r in range(2):
                        pr, psgn = (_phase_parts(rho)[0:2] if r == 0
                                    else _phase_parts(rho)[2:4])
                        dst = O.gap(sp, 4, 3, r, 1, 5)
                        src_ = gm.gap(q, 2, 3, pr, 1, 5)
                        stt(dst, src_, 0.5 * psgn, dst, AL.add)

        # ---------- clover planes ----------
        _winit = set()
        with tc.tile_pool(name="cp", bufs=1) as cpool:
            def ctile(tag, nent, s0, s1):
                return Grp(mk(cpool, nent * 2 * (s1 - s0) * SL, tag), nent, s0, s1 - s0)

            for d1 in (range(1, 5) if DBG_PART in ("full", "clover") else []):
                for d2 in range(d1 + 1, 5):
                    if DBG_PAIRS and f"{d1}{d2}" not in DBG_PAIRS.split(","):
                        continue
                    a1, a2 = d1 - 1, d2 - 1
                    tpl = (d1 == 1)
                    sA, eA = (0, 5) if tpl else (1, 5)
                    U1 = GL[d1 - 1]
                    U2 = GL[d2 - 1]
                    # shifted links
                    if tpl:
                        U2m, u2dt = U2, +1       # U_d2(x+e_T): column view
                    else:
                        U2m, u2dt = mat_shift(U2, a1, +1, sA, eA, "lnk1", cpool), 0
                    U1n = mat_shift(U1, a2, +1, sA, eA, "lnk2", cpool)
                    A = ctile("pA", 9, sA, eA)
                    cmm(A, U1, U2m, sA, eA, bdt=u2dt)
                    B = ctile("pB", 9, sA, eA)
                    cmm(B, U2, U1n, sA, eA)
                    Q = ctile("pQ", 9, 1, 5)
                    cmm(Q, B, A, 1, 5, bdag=True)            # L4
                    sL1, eL1 = (0, 4) if tpl else (1, 5)
                    L1 = ctile("pL", 9, sL1, eL1)
                    cmm(L1, A, B, sL1, eL1, adag=True)
                    # L1 shift overlaps D/E products; Q-adds deferred
                    if tpl:
                        L1s = mat_shift(L1, a2, -1, 0, 4, "pLs", cpool)
                        l1_ap = L1s.all_ap(0, 4)
                    else:
                        L1ss = mat_shift_diag(L1, a1, a2, "pLs", cpool)
                        l1_ap = L1ss.all_ap(1, 5)
                    D = ctile("pA2", 9, sA, eA)
                    cmm(D, U1n, U2m, sA, eA, bdag=True, bdt=u2dt)
                    E = ctile("pB2", 9, sA, eA)
                    cmm(E, U2, U1, sA, eA, adag=True)
                    V.tensor_add(Q.all_ap(1, 5), Q.all_ap(1, 5), l1_ap)
                    L2 = ctile("pL2", 9, 1, 5)
                    cmm(L2, D, E, 1, 5, bdag=True)
                    L2s = mat_shift(L2, a2, -1, 1, 5, "pL2s", cpool)
                    sL3, eL3 = (0, 4) if tpl else (1, 5)
                    L3 = ctile("pL3", 9, sL3, eL3)
                    cmm(L3, E, D, sL3, eL3, adag=True)
                    V.tensor_add(Q.all_ap(1, 5), Q.all_ap(1, 5),
                                 L2s.all_ap(1, 5))
                    if tpl:
                        V.tensor_add(Q.all_ap(1, 5), Q.all_ap(1, 5),
                                     L3.all_ap(0, 4))
                    else:
                        L3s = mat_shift(L3, a1, -1, 1, 5, "pLs2", cpool)
                        V.tensor_add(Q.all_ap(1, 5), Q.all_ap(1, 5),
                                     L3s.all_ap(1, 5))
                    # ---- G9 = Q - Q^dag ----
                    G9 = ctile("pG9", 9, 1, 5)
                    qv = Q.t.rearrange("p (i j r w) -> p i j r w",
                                       i=3, j=3, r=2, w=Q.W)
                    qT = qv.transpose([0, 2, 1, 3, 4])
                    gv = G9.t.rearrange("p (i j r w) -> p i j r w",
                                        i=3, j=3, r=2, w=G9.W)
                    V.tensor_sub(gv[:, :, :, 0, :], qv[:, :, :, 0, :],
                                 qT[:, :, :, 0, :])
                    V.tensor_add(gv[:, :, :, 1, :], qv[:, :, :, 1, :],
                                 qT[:, :, :, 1, :])
                    # ---- accumulate into chiral W blocks (coef = +-1;
                    #      the 1/16 is applied in the final O accumulate) ----
                    for sp in range(4):
                        wrow, phi_ph = SIGCOL[a1][a2][sp]
                        assert wrow // 2 == sp // 2, (a1, a2, sp, wrow)
                        c = -phi_ph
                        chir, spl, sl = sp // 2, sp % 2, wrow % 2
                        # re(W) += re(c) G9re - im(c) G9im
                        # im(W) += re(c) G9im + im(c) G9re
                        for outr in range(2):
                            if abs(c.imag) < 1e-6:
                                rsrc, coef = outr, c.real
                            else:
                                rsrc, coef = 1 - outr, (-c.imag if outr == 0
                                                        else c.imag)
                            wv = wslot(chir, spl, sl, outr)
                            src_ = G9.gap(0, 1, 9, rsrc, 1, 5)
                            key = (chir, spl, sl, outr)
                            if key not in _winit:
                                _winit.add(key)
                                V.tensor_scalar_mul(wv, src_, float(coef))
                            elif coef > 0:
                                V.tensor_add(wv, wv, src_)
                            else:
                                V.tensor_sub(wv, wv, src_)

        # ---------- apply chiral clover blocks: O += W psi ----------
        if _winit:
            R = mk(main, 24 * W4, "Rap")
            r3 = R.rearrange("p (q w) -> p q w", q=24, w=W4)
            f5 = F.t.rearrange("p (c sp r w) -> p c sp r w",
                               c=3, sp=4, r=2, w=F.W)
            fc0, fc1 = F.cs(1, 5)
            tv = tview4(mtA, 3, 2, W4)     # [p,4,3(j),2(l),w]
            sv = sview4(msA, 3, W4)        # [p,4,3,w]
            for chir in range(2):
                br = f5[:, :, chir * 2:chir * 2 + 2, 0, fc0:fc1]
                bi = f5[:, :, chir * 2:chir * 2 + 2, 1, fc0:fc1]
                for i in range(3):
                    for spl in range(2):
                        ar = wrow_ap(chir, i, spl, 0)
                        ai = wrow_ap(chir, i, spl, 1)
                        V.tensor_mul(tv[:, 0], ar, br)
                        V.tensor_mul(tv[:, 1], ai, bi)
                        V.tensor_mul(tv[:, 2], ar, bi)
                        V.tensor_mul(tv[:, 3], ai, br)
                        t = sv[:, :, 0:2, :]       # [p,4,2,w]
                        V.tensor_add(t, tv[:, :, 0, :, :], tv[:, :, 1, :, :])
                        V.tensor_add(t, t, tv[:, :, 2, :, :])
                        s = sv[:, :, 2, :]         # [p,4,w]
                        V.tensor_add(s, t[:, :, 0, :], t[:, :, 1, :])
                        pl = i * 8 + chir * 4 + spl * 2
                        V.tensor_sub(r3[:, pl:pl + 1, :],
                                     s[:, 0:1, :], s[:, 1:2, :])
                        V.tensor_add(r3[:, pl + 1:pl + 2, :],
                                     s[:, 2:3, :], s[:, 3:4, :])
            # chunked final accumulate + store so DMA overlaps the stt tail
            OC = 24 * TLOC * SL
            for ci in range(4):
                c0, c1 = ci * OC // 4, (ci + 1) * OC // 4
                stt(O.t[:, c0:c1], R[:, c0:c1], 1.0 / 16.0,
                    O.t[:, c0:c1], AL.add)
                eng = nc.sync if ci % 2 == 0 else nc.scalar
                eng.dma_start(out=out_d[:, c0:c1], in_=O.t[:, c0:c1])
        else:
            nc.sync.dma_start(out=out_d[:, :], in_=O.t[:, :])
        psum_cm.__exit__(None, None, None)
        spool_cm.__exit__(None, None, None)
        main_cm.__exit__(None, None, None)

    nc.compile()
    return nc


# ----------------------------------------------------------------------
# host entry
# ----------------------------------------------------------------------
_CACHE = {}

def _get_nc():
    if "nc" not in _CACHE:
        _CACHE["nc"] = build_program()
    return _CACHE["nc"]


def kernel(field_re, field_im, gauge_re, gauge_im):
    from concourse.bass_utils import run_bass_kernel_spmd
    nc = _get_nc()
    in_maps = [_host_inputs(field_re, field_im, gauge_re, gauge_im, c)
               for c in range(NCORES)]
    br = run_bass_kernel_spmd(nc, in_maps, list(range(NCORES)))
    out = np.empty((T_, Z_, Y_, X_, 3, 4, 2), np.float32)
    for c in range(NCORES):
        out[c * TLOC:(c + 1) * TLOC] = _host_output(br.results[c]["outp"])
    return out


# revision 39
# speedup vs baseline: 2.4368x; 1.0166x over previous
"""Wilson-clover Dirac operator D_WC on Trainium2, 8-core SPMD.

Self-contained: hardcodes LAT=(32,16,16,16), shards the T axis across 8
cores with host-side halo slices (t0-1..t0+4), computes everything
site-locally on-device with DVE elementwise math in an SoA layout:

  plane[p, col]: p = z*8 + (y>>1),  col = plane_idx*W + (s-base)*32
                 + (y&1)*16 + x          (s = local T-slice, W = ns*32)

Shifts: T = column offset (free), X = 2-piece ScalarE copy, Z/Y =
TensorE permutation matmul (stationary 128x128 0/1 matrix) + ScalarE
copyback from PSUM.  All elementwise arithmetic runs on the DVE only
(GPSIMD shares the DVE SBUF port; running both concurrently is a net
loss).  Gauge links for all 4 directions are DMA'd into SBUF once.
"""
import numpy as np

# ----------------------------------------------------------------------
# constants (physics)
# ----------------------------------------------------------------------
MASS, C_SW = 0.5, 1.0
_i = 1j
G1 = np.array([[0,0,0,_i],[0,0,_i,0],[0,-_i,0,0],[-_i,0,0,0]], dtype=np.complex64)
G2 = np.array([[0,0,0,-1],[0,0,1,0],[0,1,0,0],[-1,0,0,0]], dtype=np.complex64)
G3 = np.array([[0,0,_i,0],[0,0,0,-_i],[-_i,0,0,0],[0,_i,0,0]], dtype=np.complex64)
G4 = np.array([[0,0,1,0],[0,0,0,1],[1,0,0,0],[0,1,0,0]], dtype=np.complex64)
GAMMA = [G1, G2, G3, G4]
I4 = np.eye(4, dtype=np.complex64)
SIGMA = [[(0.5j * (GAMMA[m] @ GAMMA[n] - GAMMA[n] @ GAMMA[m])).astype(np.complex64)
          for n in range(4)] for m in range(4)]

T_, Z_, Y_, X_ = 32, 16, 16, 16
NCORES, TLOC, NSH = 8, 4, 6       # halo slices per core
SL = 32                           # cols per slice
USE_BF16 = True                   # bf16 compute tiles (O stays fp32)
import os as _os
DBG_PART = _os.environ.get("KRN_PART", "full")    # full|wilson|clover
DBG_PAIRS = _os.environ.get("KRN_PAIRS", "")      # e.g. "12,34" to keep only
DBG_HOPS = _os.environ.get("KRN_HOPS", "0123")    # wilson dirs to keep

# permutation matrix indices (stationary operands for TensorE shifts)
PZ_P, PZ_M, PY_P, PY_M, PZY_M = 0, 1, 2, 3, 4
NPERM = 5

# ----------------------------------------------------------------------
# spin-structure extraction (numerical)
# ----------------------------------------------------------------------
def _col_struct(M):
    """M has single-nonzero columns: return per-col (row, phase)."""
    out = []
    for s in range(4):
        col = M[:, s]
        r = int(np.argmax(np.abs(col)))
        ph = complex(col[r])
        assert np.sum(np.abs(col) > 1e-6) == 1, (M, s)
        out.append((r, ph))
    return out

def _proj_struct(P):
    """P rank-2 with cols 2,3 = rho * cols q in {0,1}. Returns (q2,rho2,q3,rho3)."""
    res = []
    for s in (2, 3):
        found = None
        for j in (0, 1):
            c, cj = P[:, s], P[:, j]
            nz = np.abs(cj) > 1e-6
            if not nz.any():
                continue
            ratio = c[nz] / cj[nz]
            if np.allclose(ratio, ratio[0], atol=1e-5) and np.allclose(
                    c, ratio[0] * cj, atol=1e-5):
                found = (j, complex(ratio[0]))
                break
        assert found is not None, P
        res.append(found)
    return res

# per direction k: gamma column structure and projector relations
GCOL = [_col_struct(GAMMA[k]) for k in range(4)]           # (p(s), g(s))
PROJ_F = [_proj_struct(GAMMA[k] + I4) for k in range(4)]   # for psi@(G+I)
PROJ_B = [_proj_struct(GAMMA[k] - I4) for k in range(4)]   # for psi@(G-I)
SIGCOL = [[_col_struct(SIGMA[m][n]) if m != n else None for n in range(4)]
          for m in range(4)]

def _phase_parts(ph):
    """phase in {1,-1,i,-i} -> for v = ph*u:
    re(v) = sr*u.[pr]  im(v) = si*u.[pi]   (0=re,1=im planes of u)"""
    if abs(ph - 1) < 1e-5:   return (0, 1.0, 1, 1.0)
    if abs(ph + 1) < 1e-5:   return (0, -1.0, 1, -1.0)
    if abs(ph - 1j) < 1e-5:  return (1, -1.0, 0, 1.0)
    if abs(ph + 1j) < 1e-5:  return (1, 1.0, 0, -1.0)
    raise AssertionError(ph)

# ----------------------------------------------------------------------
# host layout helpers
# ----------------------------------------------------------------------
def _to_planes(vol):
    """vol [ns, Z, Y, X] -> [128, ns*32]"""
    ns = vol.shape[0]
    v = vol.reshape(ns, 16, 8, 2, 16)        # s z yh yl x
    v = np.transpose(v, (1, 2, 0, 3, 4))     # z yh s yl x
    return np.ascontiguousarray(v.reshape(128, ns * 32))

def _from_planes(pl, ns):
    v = pl.reshape(16, 8, ns, 2, 16)
    return np.transpose(v, (2, 0, 1, 3, 4)).reshape(ns, 16, 16, 16)

def _perm_z(d):
    """dst partition m = z*8+yh reads p = ((z+d)%16)*8+yh; P[p, m] = 1."""
    P = np.zeros((128, 128), np.float32)
    for z in range(16):
        for yh in range(8):
            P[((z + d) % 16) * 8 + yh, z * 8 + yh] = 1.0
    return P

def _perm_y(d, dz=0):
    """Y cross-half: dst m = z*8+yh reads p = ((z+dz)%16)*8+((yh+d)%8)."""
    P = np.zeros((128, 128), np.float32)
    for z in range(16):
        for yh in range(8):
            P[((z + dz) % 16) * 8 + ((yh + d) % 8), z * 8 + yh] = 1.0
    return P

def _perm_host():
    P = np.concatenate([_perm_z(+1), _perm_z(-1), _perm_y(+1), _perm_y(-1),
                        _perm_y(-1, dz=-1)], axis=1)  # [128, NPERM*128]
    return P

def _host_inputs(field_re, field_im, gauge_re, gauge_im, core):
    t0 = core * TLOC
    ts = [(t0 - 1 + s) % T_ for s in range(NSH)]
    f = np.stack([field_re[ts], field_im[ts]], axis=0)   # [2,6,Z,Y,X,3,4]
    g = np.stack([gauge_re[:, ts], gauge_im[:, ts]], axis=0)  # [2,4,6,Z,Y,X,3,3]

    # field planes: e=(c*4+sp), plane=e*2+r
    fp = np.empty((24, 128, NSH * SL), np.float32)
    for c in range(3):
        for sp in range(4):
            for r in range(2):
                fp[(c * 4 + sp) * 2 + r] = _to_planes(f[r, :, :, :, :, c, sp])
    fld = np.ascontiguousarray(fp.transpose(1, 0, 2).reshape(128, 24 * NSH * SL))

    # gauge planes: dir k, e=(a*3+b), plane=(k*9+e)*2+r
    gp = np.empty((72, 128, NSH * SL), np.float32)
    for k in range(4):
        for a in range(3):
            for b in range(3):
                for r in range(2):
                    gp[(k * 9 + a * 3 + b) * 2 + r] = _to_planes(
                        g[r, k, :, :, :, :, a, b])
    gg = np.ascontiguousarray(gp.transpose(1, 0, 2).reshape(128, 72 * NSH * SL))
    perm = _perm_host()
    if USE_BF16:
        import ml_dtypes
        fld = fld.astype(ml_dtypes.bfloat16)
        gg = gg.astype(ml_dtypes.bfloat16)
        perm = perm.astype(ml_dtypes.bfloat16)
    return {"fld": fld, "gg": gg, "perm": perm}

def _host_output(outp_flat):
    """device out [128, 24*4*32] -> [TLOC, Z,Y,X, 3,4,2]"""
    pl = outp_flat.reshape(128, 24, TLOC * SL).transpose(1, 0, 2)
    out = np.empty((TLOC, 16, 16, 16, 3, 4, 2), np.float32)
    for c in range(3):
        for sp in range(4):
            for r in range(2):
                out[..., c, sp, r] = _from_planes(pl[(c * 4 + sp) * 2 + r], TLOC)
    return out

# ----------------------------------------------------------------------
# device program
# ----------------------------------------------------------------------
class Grp:
    """Group of planes in one SBUF tile. nent complex entries (re+im planes).
    base = slice index of col 0; ns slices; W = ns*32 cols per plane."""
    def __init__(self, tile, nent, base, ns):
        self.t, self.nent, self.base, self.ns = tile, nent, base, ns
        self.W = ns * SL

    def fl(self):
        return self.t.rearrange("p (q w) -> p q w", q=self.nent * 2, w=self.W)

    def cs(self, s0, s1):
        return ((s0 - self.base) * SL, (s1 - self.base) * SL)

    def pap(self, e, r, s0, s1):
        """single plane AP [128, cols]"""
        c0, c1 = self.cs(s0, s1)
        v = self.fl()[:, (e * 2 + r):(e * 2 + r + 1), c0:c1]
        return v  # [p,1,w]

    def gap(self, e0, estep, n, r, s0, s1):
        """packed-entry AP [p, n, w]: entries e0 + i*estep, fixed r."""
        c0, c1 = self.cs(s0, s1)
        q0 = e0 * 2 + r
        fl = self.fl()
        return fl[:, q0:q0 + 2 * estep * (n - 1) + 1:2 * estep, c0:c1]

    def all_ap(self, s0, s1):
        c0, c1 = self.cs(s0, s1)
        return self.fl()[:, :, c0:c1]


def build_program():
    import concourse.bacc as bacc
    import concourse.mybir as mybir
    from concourse.tile import TileContext
    FP = mybir.dt.float32
    CDT = mybir.dt.bfloat16 if USE_BF16 else FP
    AL = mybir.AluOpType

    nc = bacc.Bacc("TRN2", target_bir_lowering=False, debug=False)
    fld_d = nc.declare_dram_parameter("fld", [128, 24 * NSH * SL], CDT, isOutput=False)
    gg_d = nc.declare_dram_parameter("gg", [128, 72 * NSH * SL], CDT, isOutput=False)
    perm_d = nc.declare_dram_parameter("perm", [128, NPERM * 128], CDT, isOutput=False)
    out_d = nc.declare_dram_parameter("outp", [128, 24 * TLOC * SL], FP, isOutput=True)

    with TileContext(nc) as tc:
        _tc = [0]

        def mk(pool, cols, tag, dt=None):
            _tc[0] += 1
            return pool.tile([128, cols], dt or CDT, tag=tag,
                             name=f"{tag}_{_tc[0]}")

        main_cm = tc.tile_pool(name="main", bufs=1)
        main = main_cm.__enter__()
        spool_cm = tc.tile_pool(name="sp", bufs=1)
        spool = spool_cm.__enter__()
        psum_cm = tc.psum_pool(name="ps", bufs=4)
        psum = psum_cm.__enter__()

        F = Grp(mk(main, 24 * NSH * SL, "F"), 12, 0, NSH)
        O = Grp(mk(main, 24 * TLOC * SL, "O", FP), 12, 1, TLOC)
        PM = mk(main, NPERM * 128, "PM")
        FH = 12 * NSH * SL
        nc.sync.dma_start(out=F.t[:, :FH], in_=fld_d[:, :FH])
        nc.scalar.dma_start(out=F.t[:, FH:], in_=fld_d[:, FH:])
        nc.sync.dma_start(out=PM[:, :], in_=perm_d[:, :])

        # gauge links for all 4 directions, loaded once
        GL = []
        for k in range(4):
            g = Grp(mk(main, 9 * 2 * NSH * SL, f"GL{k}"), 9, 0, NSH)
            eng = nc.sync if k % 2 == 0 else nc.scalar
            eng.dma_start(out=g.t[:, :],
                          in_=gg_d[:, k * 18 * NSH * SL:(k + 1) * 18 * NSH * SL])
            GL.append(g)

        # merged scratch tiles (4 product slots each)
        WMX = 5 * SL
        mtA = mk(spool, 4 * 9 * WMX, "mtA")
        msA = mk(spool, 4 * 3 * WMX, "msA")
        asT = mk(spool, 9 * WMX, "asT")      # Gauss a-sums (ar +- ai)
        bsT = mk(spool, 9 * WMX, "bsT")      # Gauss b-sums (br +- bi)

        def tview4(t, nj, nk, w):
            """[p, 4, nj, nk, w] slots of merged mul scratch"""
            v = t.rearrange("p (h m) -> p h m", h=4)
            return v[:, :, :nj * nk * w].rearrange(
                "p h (j k w) -> p h j k w", j=nj, k=nk, w=w)

        def sview4(t, nj, w):
            v = t.rearrange("p (h m) -> p h m", h=4)
            return v[:, :, :nj * w].rearrange("p h (j w) -> p h j w",
                                              j=nj, w=w)

        V = nc.vector

        def a_pack(A, i, adag, r, s0, s1, dt, nj):
            """[p, nj(bcast), 3, w] for a-values (i,k)."""
            e0, st = (i, 3) if adag else (i * 3, 1)
            ap = A.gap(e0, st, 3, r, s0 + dt, s1 + dt)       # [p,3,w]
            w = ap.shape[2]
            return ap.unsqueeze(1).broadcast_to((128, nj, 3, w))

        def b_pack(B, bdag, r, s0, s1, dt, nj):
            """[p, nj, 3, w] for b-values (k,j)."""
            c0, c1 = B.cs(s0 + dt, s1 + dt)
            w = c1 - c0
            if bdag:  # e = j*3+k
                v = B.t.rearrange("p (j k r w) -> p j k r w", j=3, k=3, r=2, w=B.W)
                return v[:, :, :, r, c0:c1]
            if B.nent == 9:  # e = k*3+j
                v = B.t.rearrange("p (k j r w) -> p k j r w", k=3, j=3, r=2, w=B.W)
                return v[:, :, :, r, c0:c1].transpose([0, 2, 1, 3])
            # halfspinor: e = k*2+j, nj=2
            v = B.t.rearrange("p (k j r w) -> p k j r w", k=3, j=2, r=2, w=B.W)
            return v[:, :, :, r, c0:c1].transpose([0, 2, 1, 3])

        def stt(out, in0, coef, in1, op1=None):
            V.scalar_tensor_tensor(out, in0, float(coef), in1,
                                   AL.mult, op1 or AL.add)

        class SGrp:
            """9 single sum-planes (re +- im of a matrix Grp)."""
            def __init__(self, tile, base, ns):
                self.t, self.base, self.ns = tile, base, ns
                self.W = ns * SL

            def cs(self, s0, s1):
                return ((s0 - self.base) * SL, (s1 - self.base) * SL)

            def fl(self):
                return self.t.rearrange("p (e w) -> p e w", e=9, w=self.W)

            def aview(self, i, adag, s0, s1, dt):
                c0, c1 = self.cs(s0 + dt, s1 + dt)
                e0, st = (i, 3) if adag else (i * 3, 1)
                return self.fl()[:, e0:e0 + 2 * st + 1:st, c0:c1]

            def bview(self, bdag, s0, s1, dt):
                c0, c1 = self.cs(s0 + dt, s1 + dt)
                if bdag:  # e = j*3+k
                    v = self.t.rearrange("p (j k w) -> p j k w",
                                         j=3, k=3, w=self.W)
                    return v[:, :, :, c0:c1]
                v = self.t.rearrange("p (k j w) -> p k j w",
                                     k=3, j=3, w=self.W)
                return v[:, :, :, c0:c1].transpose([0, 2, 1, 3])

        def msum(M, sign, tag, pool, rng=None):
            """SGrp of M.re + sign*M.im over rng (default M's full range)."""
            e0, e1 = rng if rng else (M.base, M.base + M.ns)
            g = SGrp(mk(pool, 9 * (e1 - e0) * SL, tag), e0, e1 - e0)
            mr = M.gap(0, 1, 9, 0, e0, e1)
            mi = M.gap(0, 1, 9, 1, e0, e1)
            if sign > 0:
                V.tensor_add(g.fl(), mr, mi)
            else:
                V.tensor_sub(g.fl(), mr, mi)
            return g

        def cmm(dst, A, B, s0, s1, adag=False, bdag=False, adt=0, bdt=0,
                nj=3, asum=None, bsum=None):
            """dst[i,j] = sum_k aval(i,k)*bval(k,j); dst entries e=i*nj+j.
            3-mult (Gauss) complex product: m1=ar*br, m2=ai*bi,
            m3=(ar+sa*ai)*(br+sb*bi); re=R1-s*R2, im=R3-R1-s*R2, s=sa*sb.
            asum/bsum: precomputed SGrp of (re -+ im) matching the dag mode."""
            assert not (adag and bdag)
            w = (s1 - s0) * SL
            neg = adag or bdag
            br_ = b_pack(B, bdag, 0, s0, s1, bdt, nj)
            bi_ = b_pack(B, bdag, 1, s0, s1, bdt, nj)
            if bsum is not None:
                bs = bsum.bview(bdag, s0, s1, bdt)[:, :nj]
            else:
                bs = bsT.rearrange("p (j k w) -> p j k w", j=3, k=3,
                                   w=WMX)[:, :nj, :, :w]
                if bdag:
                    V.tensor_sub(bs, br_, bi_)
                else:
                    V.tensor_add(bs, br_, bi_)
            if asum is None:
                ar_all = A.gap(0, 1, 9, 0, s0 + adt, s1 + adt)
                ai_all = A.gap(0, 1, 9, 1, s0 + adt, s1 + adt)
                as3 = asT.rearrange("p (e w) -> p e w", e=9, w=WMX)[:, :, :w]
                if adag:
                    V.tensor_sub(as3, ar_all, ai_all)
                else:
                    V.tensor_add(as3, ar_all, ai_all)
            for i in range(3):
                tv = tview4(mtA, nj, 3, w)     # slots 0..2 used
                sv = sview4(msA, nj, w)
                ar = a_pack(A, i, adag, 0, s0, s1, adt, nj)
                ai = a_pack(A, i, adag, 1, s0, s1, adt, nj)
                if asum is not None:
                    asb = asum.aview(i, adag, s0, s1, adt)
                elif adag:
                    asb = as3[:, i:i + 7:3, :]
                else:
                    asb = as3[:, i * 3:i * 3 + 3, :]
                asb = asb.unsqueeze(1).broadcast_to((128, nj, 3, w))
                dre = dst.gap(i * nj, 1, nj, 0, s0, s1)
                dim = dst.gap(i * nj, 1, nj, 1, s0, s1)
                V.tensor_mul(tv[:, 0], ar, br_)
                V.tensor_mul(tv[:, 1], ai, bi_)
                V.tensor_mul(tv[:, 2], asb, bs)
                sv3 = sv[:, 0:3]
                V.tensor_add(sv3, tv[:, 0:3, :, 0, :], tv[:, 0:3, :, 1, :])
                V.tensor_add(sv3, sv3, tv[:, 0:3, :, 2, :])
                if neg:
                    V.tensor_add(dre, sv[:, 0], sv[:, 1])
                else:
                    V.tensor_sub(dre, sv[:, 0], sv[:, 1])
                V.tensor_sub(dim, sv[:, 2], sv[:, 0])
                if neg:
                    V.tensor_add(dim, dim, sv[:, 1])
                else:
                    V.tensor_sub(dim, dim, sv[:, 1])

        # ---------- shift materialization ----------
        def cpy(out, in_):
            nc.scalar.copy(out, in_)

        def perm_ap(idx):
            return PM.rearrange("p (i m) -> p i m", i=NPERM)[:, idx, :]

        def mm_permute(dst_t, src_t, cols, pidx, dhalf=None, shalf=None,
                       xrot=False):
            """dst = P[pidx] applied to src partitions, over [0, cols).
            dhalf/shalf: copy back only dst yl==dhalf 16-col blocks, sourced
            from permuted yl==shalf blocks.
            xrot: additionally rotate X by -1 (dst x reads src x-1)."""
            P = perm_ap(pidx)
            c0 = 0
            while c0 < cols:
                cw = min(512, cols - c0)
                pt = psum.tile([128, cw], FP, tag="pshift",
                               name=f"ps_{_tc[0]}_{c0}")
                _tc[0] += 1
                nc.tensor.matmul(pt[:, :], P, src_t[:, c0:c0 + cw])
                dv = dst_t[:, c0:c0 + cw]
                if dhalf is None and not xrot:
                    cpy(dv, pt[:, :])
                elif dhalf is None and xrot:
                    d3 = dv.rearrange("p (b x) -> p b x", x=16)
                    p3 = pt.rearrange("p (b x) -> p b x", x=16)
                    cpy(d3[:, :, 1:16], p3[:, :, 0:15])
                    cpy(d3[:, :, 0:1], p3[:, :, 15:16])
                else:
                    d4 = dv.rearrange("p (b l x) -> p b l x", l=2, x=16)
                    p4 = pt.rearrange("p (b l x) -> p b l x", l=2, x=16)
                    if not xrot:
                        cpy(d4[:, :, dhalf, :], p4[:, :, shalf, :])
                    else:
                        cpy(d4[:, :, dhalf, 1:16], p4[:, :, shalf, 0:15])
                        cpy(d4[:, :, dhalf, 0:1], p4[:, :, shalf, 15:16])
                c0 += cw

        def lv(t_, lo):
            """yl==lo 16-col blocks of flat tile view"""
            return t_.rearrange("p (m x) -> p m x", x=16)[:, lo::2, :]

        def mat_shift(src, axis, d, s0, s1, tag, pool=None):
            """materialize S(x)=src(x + d*e_axis) over the FULL src range.
            axis 1(Z) 2(Y) 3(X)."""
            g = Grp(mk(pool or main, src.nent * 2 * src.ns * SL, tag),
                    src.nent, src.base, src.ns)
            cols = src.nent * 2 * src.ns * SL
            if axis == 3:   # X: 2-piece ScalarE copy
                def xv(t_):
                    return t_.rearrange("p (m x) -> p m x", x=16)
                dv, sv_ = xv(g.t), xv(src.t)
                if d == +1:
                    cpy(dv[:, :, 0:15], sv_[:, :, 1:16])
                    cpy(dv[:, :, 15:16], sv_[:, :, 0:1])
                else:
                    cpy(dv[:, :, 1:16], sv_[:, :, 0:15])
                    cpy(dv[:, :, 0:1], sv_[:, :, 15:16])
            elif axis == 1:  # Z: pure partition permutation
                mm_permute(g.t, src.t, cols, PZ_P if d == +1 else PZ_M)
            else:           # Y: in-partition half + permuted half
                if d == +1:
                    cpy(lv(g.t, 0), lv(src.t, 1))
                    mm_permute(g.t, src.t, cols, PY_P, dhalf=1, shalf=0)
                else:
                    cpy(lv(g.t, 1), lv(src.t, 0))
                    mm_permute(g.t, src.t, cols, PY_M, dhalf=0, shalf=1)
            return g

        def mat_shift_diag(src, a1, a2, tag, pool=None):
            """S(x) = src(x - e_a1 - e_a2), a1 < a2 spatial axes (1,2,3)."""
            g = Grp(mk(pool or main, src.nent * 2 * src.ns * SL, tag),
                    src.nent, src.base, src.ns)
            cols = src.nent * 2 * src.ns * SL
            if (a1, a2) == (1, 2):    # Z,Y
                mm_permute(g.t, src.t, cols, PZ_M, dhalf=1, shalf=0)
                mm_permute(g.t, src.t, cols, PZY_M, dhalf=0, shalf=1)
            elif (a1, a2) == (1, 3):  # Z,X
                mm_permute(g.t, src.t, cols, PZ_M, xrot=True)
            else:                     # Y,X
                dv = g.t.rearrange("p (m x) -> p m x", x=16)[:, 1::2, :]
                sv_ = src.t.rearrange("p (m x) -> p m x", x=16)[:, 0::2, :]
                cpy(dv[:, :, 1:16], sv_[:, :, 0:15])
                cpy(dv[:, :, 0:1], sv_[:, :, 15:16])
                mm_permute(g.t, src.t, cols, PY_M, dhalf=0, shalf=1, xrot=True)
            return g

        # ---------- mass term: O = (MASS+4) * F ----------
        V.tensor_scalar_mul(O.all_ap(1, 5), F.all_ap(1, 5), float(MASS + 4.0))

        # clover chiral blocks W[(i,sp),(j,s')] = sum_p -(1/16) ph_p(sp)
        #   * G9_p[i,j] with s' = wrow_p(sp);  plane index =
        #   chir*72 + spl*36 + sl*18 + i*6 + j*2 + r
        W4 = 4 * SL
        WCH = mk(main, 144 * W4, "WCH")
        wv3 = WCH.rearrange("p (q w) -> p q w", q=144, w=W4)
        wv6 = WCH.rearrange("p (c s l m w) -> p c s l m w",
                            c=2, s=2, l=2, m=18, w=W4)

        def wslot(chir, spl, sl, r):
            """[p, (i,j)(9), w4] view of one spin-slot of W (3D for stt)"""
            base = chir * 72 + spl * 36 + sl * 18 + r
            return wv3[:, base:base + 17:2, :]

        def wrow_ap(chir, i, spl, r):
            """[p, j(3), sl(2), w4] contraction view of one W row"""
            m0 = i * 6 + r
            return wv6[:, chir, spl, :, m0:m0 + 5:2, :].transpose([0, 2, 1, 3])

        # ---------- Wilson hops ----------
        with tc.tile_pool(name="wp", bufs=1) as wpool:
            for k in (range(4) if DBG_PART in ("full", "wilson") else []):
                if str(k) not in DBG_HOPS:
                    continue
                ax = k  # lattice axis
                Uk = GL[k]
                # ---- forward hop ----
                rng = (0, 4) if k == 0 else (1, 5)
                s0, s1 = rng
                h = Grp(mk(wpool, 12 * (s1 - s0) * SL, "h"),
                        6, s0, s1 - s0)
                # h[c,j] = psi[c,j] + g(j)*psi[c,p(j)]
                for j in range(2):
                    pj, gj = GCOL[k][j]
                    for r in range(2):
                        pr, psgn = (_phase_parts(gj)[0:2] if r == 0
                                    else _phase_parts(gj)[2:4])
                        dst = h.gap(j, 2, 3, r, s0, s1)       # c-packed
                        a0 = F.gap(j, 4, 3, r, s0, s1)        # psi[c,j].r
                        a1 = F.gap(pj, 4, 3, pr, s0, s1)
                        if psgn > 0:
                            V.tensor_add(dst, a1, a0)
                        else:
                            V.tensor_sub(dst, a0, a1)
                phi = Grp(mk(wpool, 12 * (s1 - s0) * SL, "phi"),
                          6, s0, s1 - s0)
                cmm(phi, Uk, h, s0, s1, adag=True, nj=2)
                # shift (-1 along ax) then reconstruct into O
                if k == 0:
                    psh, dt = phi, -1
                else:
                    psh, dt = mat_shift(phi, ax, -1, 1, 5, "psh", wpool), 0
                # out[:, s'] += -0.5 * rho(s') * psh[:, q(s')]
                rec = [(0, 1.0), (1, 1.0), PROJ_F[k][0], PROJ_F[k][1]]
                for sp in range(4):
                    q, rho = rec[sp]
                    for r in range(2):
                        pr, psgn = (_phase_parts(rho)[0:2] if r == 0
                                    else _phase_parts(rho)[2:4])
                        dst = O.gap(sp, 4, 3, r, 1, 5)
                        src_ = psh.gap(q, 2, 3, pr, 1 + dt, 5 + dt)
                        stt(dst, src_, -0.5 * psgn, dst, AL.add)
                # ---- backward hop ----
                rng = (2, 6) if k == 0 else (1, 5)
                s0, s1 = rng
                hb = Grp(mk(wpool, 12 * (s1 - s0) * SL, "h"),
                         6, s0, s1 - s0)
                # hb[c,j] = g(j)*psi[c,p(j)] - psi[c,j]
                for j in range(2):
                    pj, gj = GCOL[k][j]
                    for r in range(2):
                        pr, psgn = (_phase_parts(gj)[0:2] if r == 0
                                    else _phase_parts(gj)[2:4])
                        dst = hb.gap(j, 2, 3, r, s0, s1)
                        a1 = F.gap(pj, 4, 3, pr, s0, s1)
                        a0 = F.gap(j, 4, 3, r, s0, s1)
                        if psgn > 0:
                            V.tensor_sub(dst, a1, a0)
                        else:
                            stt(dst, a1, psgn, a0, AL.subtract)
                if k == 0:
                    hs, dt = hb, +1
                else:
                    hs, dt = mat_shift(hb, ax, +1, 1, 5, "psh", wpool), 0
                gm = Grp(mk(wpool, 12 * 4 * SL, "phi"),
                         6, 1, 4)
                cmm(gm, Uk, hs, 1, 5, nj=2, bdt=dt)
                rec = [(0, 1.0), (1, 1.0), PROJ_B[k][0], PROJ_B[k][1]]
                for sp in range(4):
                    q, rho = rec[sp]
                    for r in range(2):
                        pr, psgn = (_phase_parts(rho)[0:2] if r == 0
                                    else _phase_parts(rho)[2:4])
                        dst = O.gap(sp, 4, 3, r, 1, 5)
                        src_ = gm.gap(q, 2, 3, pr, 1, 5)
                        stt(dst, src_, 0.5 * psgn, dst, AL.add)

        # ---------- clover planes ----------
        _winit = set()
        with tc.tile_pool(name="cp", bufs=1) as cpool:
            def ctile(tag, nent, s0, s1):
                return Grp(mk(cpool, nent * 2 * (s1 - s0) * SL, tag), nent, s0, s1 - s0)

            for d1 in (range(1, 5) if DBG_PART in ("full", "clover") else []):
                for d2 in range(d1 + 1, 5):
                    if DBG_PAIRS and f"{d1}{d2}" not in DBG_PAIRS.split(","):
                        continue
                    a1, a2 = d1 - 1, d2 - 1
                    tpl = (d1 == 1)
                    sA, eA = (0, 5) if tpl else (1, 5)
                    U1 = GL[d1 - 1]
                    U2 = GL[d2 - 1]
                    # shifted links
                    if tpl:
                        U2m, u2dt = U2, +1       # U_d2(x+e_T): column view
                    else:
                        U2m, u2dt = mat_shift(U2, a1, +1, sA, eA, "lnk1", cpool), 0
                    U1n = mat_shift(U1, a2, +1, sA, eA, "lnk2", cpool)
                    sU1p = msum(U1, +1, "sU1p", cpool, (sA, eA))   # A.a, E.b
                    sU1nP = msum(U1n, +1, "sU1nP", cpool, (sA, eA))  # B.b, D.a
                    if tpl:
                        sU2p = msum(U2, +1, "sU2p", cpool)       # B.a, A.b
                        sU2mM = msum(U2, -1, "sU2m", cpool)      # E.a, D.b
                        sU2mP, sU2mM2 = sU2p, sU2mM              # column-dt
                    else:
                        sU2p = msum(U2, +1, "sU2p", cpool, (sA, eA))
                        sU2mM = msum(U2, -1, "sU2m", cpool, (sA, eA))
                        sU2mP = msum(U2m, +1, "sU2mP", cpool, (sA, eA))
                        sU2mM2 = msum(U2m, -1, "sU2mM2", cpool, (sA, eA))
                    A = ctile("pA", 9, sA, eA)
                    cmm(A, U1, U2m, sA, eA, bdt=u2dt, asum=sU1p, bsum=sU2mP)
                    B = ctile("pB", 9, sA, eA)
                    cmm(B, U2, U1n, sA, eA, asum=sU2p, bsum=sU1nP)
                    sAm = msum(A, -1, "sAm", cpool)          # L1.a, Q.b
                    sBp = msum(B, +1, "sBp", cpool)          # Q.a, L1.b
                    Q = ctile("pQ", 9, 1, 5)
                    cmm(Q, B, A, 1, 5, bdag=True, asum=sBp, bsum=sAm)  # L4
                    sL1, eL1 = (0, 4) if tpl else (1, 5)
                    L1 = ctile("pL", 9, sL1, eL1)
                    cmm(L1, A, B, sL1, eL1, adag=True, asum=sAm, bsum=sBp)
                    # L1 shift overlaps D/E products; Q-adds deferred
                    if tpl:
                        L1s = mat_shift(L1, a2, -1, 0, 4, "pLs", cpool)
                        l1_ap = L1s.all_ap(0, 4)
                    else:
                        L1ss = mat_shift_diag(L1, a1, a2, "pLs", cpool)
                        l1_ap = L1ss.all_ap(1, 5)
                    D = ctile("pA2", 9, sA, eA)
                    cmm(D, U1n, U2m, sA, eA, bdag=True, bdt=u2dt,
                        asum=sU1nP, bsum=sU2mM2)
                    E = ctile("pB2", 9, sA, eA)
                    cmm(E, U2, U1, sA, eA, adag=True, asum=sU2mM, bsum=sU1p)
                    V.tensor_add(Q.all_ap(1, 5), Q.all_ap(1, 5), l1_ap)
                    sDp = msum(D, +1, "sDp", cpool)          # L2.a, L3.b
                    sEm = msum(E, -1, "sEm", cpool)          # L3.a, L2.b
                    L2 = ctile("pL2", 9, 1, 5)
                    cmm(L2, D, E, 1, 5, bdag=True, asum=sDp, bsum=sEm)
                    L2s = mat_shift(L2, a2, -1, 1, 5, "pL2s", cpool)
                    sL3, eL3 = (0, 4) if tpl else (1, 5)
                    L3 = ctile("pL3", 9, sL3, eL3)
                    cmm(L3, E, D, sL3, eL3, adag=True, asum=sEm, bsum=sDp)
                    V.tensor_add(Q.all_ap(1, 5), Q.all_ap(1, 5),
                                 L2s.all_ap(1, 5))
                    if tpl:
                        V.tensor_add(Q.all_ap(1, 5), Q.all_ap(1, 5),
                                     L3.all_ap(0, 4))
                    else:
                        L3s = mat_shift(L3, a1, -1, 1, 5, "pLs2", cpool)
                        V.tensor_add(Q.all_ap(1, 5), Q.all_ap(1, 5),
                                     L3s.all_ap(1, 5))
                    # ---- G9 = Q - Q^dag ----
                    G9 = ctile("pG9", 9, 1, 5)
                    qv = Q.t.rearrange("p (i j r w) -> p i j r w",
                                       i=3, j=3, r=2, w=Q.W)
                    qT = qv.transpose([0, 2, 1, 3, 4])
                    gv = G9.t.rearrange("p (i j r w) -> p i j r w",
                                        i=3, j=3, r=2, w=G9.W)
                    V.tensor_sub(gv[:, :, :, 0, :], qv[:, :, :, 0, :],
                                 qT[:, :, :, 0, :])
                    V.tensor_add(gv[:, :, :, 1, :], qv[:, :, :, 1, :],
                                 qT[:, :, :, 1, :])
                    # ---- accumulate into chiral W blocks (coef = +-1;
                    #      the 1/16 is applied in the final O accumulate) ----
                    for sp in range(4):
                        wrow, phi_ph = SIGCOL[a1][a2][sp]
                        assert wrow // 2 == sp // 2, (a1, a2, sp, wrow)
                        c = -phi_ph
                        chir, spl, sl = sp // 2, sp % 2, wrow % 2
                        # re(W) += re(c) G9re - im(c) G9im
                        # im(W) += re(c) G9im + im(c) G9re
                        for outr in range(2):
                            if abs(c.imag) < 1e-6:
                                rsrc, coef = outr, c.real
                            else:
                                rsrc, coef = 1 - outr, (-c.imag if outr == 0
                                                        else c.imag)
                            wv = wslot(chir, spl, sl, outr)
                            src_ = G9.gap(0, 1, 9, rsrc, 1, 5)
                            key = (chir, spl, sl, outr)
                            if key not in _winit:
                                _winit.add(key)
                                V.tensor_scalar_mul(wv, src_, float(coef))
                            elif coef > 0:
                                V.tensor_add(wv, wv, src_)
                            else:
                                V.tensor_sub(wv, wv, src_)

        # ---------- apply chiral clover blocks: O += W psi ----------
        if _winit:
            apool_cm = tc.tile_pool(name="ap", bufs=1)
            apool = apool_cm.__enter__()
            R = mk(apool, 24 * W4, "Rap")
            r3 = R.rearrange("p (q w) -> p q w", q=24, w=W4)
            f5 = F.t.rearrange("p (c sp r w) -> p c sp r w",
                               c=3, sp=4, r=2, w=F.W)
            fc0, fc1 = F.cs(1, 5)
            tv = tview4(mtA, 3, 2, W4)     # [p,4,3(j),2(l),w]
            sv = sview4(msA, 3, W4)        # [p,4,3,w]
            for chir in range(2):
                br = f5[:, :, chir * 2:chir * 2 + 2, 0, fc0:fc1]
                bi = f5[:, :, chir * 2:chir * 2 + 2, 1, fc0:fc1]
                for i in range(3):
                    for spl in range(2):
                        ar = wrow_ap(chir, i, spl, 0)
                        ai = wrow_ap(chir, i, spl, 1)
                        V.tensor_mul(tv[:, 0], ar, br)
                        V.tensor_mul(tv[:, 1], ai, bi)
                        V.tensor_mul(tv[:, 2], ar, bi)
                        V.tensor_mul(tv[:, 3], ai, br)
                        t = sv[:, :, 0:2, :]       # [p,4,2,w]
                        V.tensor_add(t, tv[:, :, 0, :, :], tv[:, :, 1, :, :])
                        V.tensor_add(t, t, tv[:, :, 2, :, :])
                        s = sv[:, :, 2, :]         # [p,4,w]
                        V.tensor_add(s, t[:, :, 0, :], t[:, :, 1, :])
                        pl = i * 8 + chir * 4 + spl * 2
                        V.tensor_sub(r3[:, pl:pl + 1, :],
                                     s[:, 0:1, :], s[:, 1:2, :])
                        V.tensor_add(r3[:, pl + 1:pl + 2, :],
                                     s[:, 2:3, :], s[:, 3:4, :])
            # chunked final accumulate + store so DMA overlaps the stt tail
            OC = 24 * TLOC * SL
            for ci in range(4):
                c0, c1 = ci * OC // 4, (ci + 1) * OC // 4
                stt(O.t[:, c0:c1], R[:, c0:c1], 1.0 / 16.0,
                    O.t[:, c0:c1], AL.add)
                eng = nc.sync if ci % 2 == 0 else nc.scalar
                eng.dma_start(out=out_d[:, c0:c1], in_=O.t[:, c0:c1])
            apool_cm.__exit__(None, None, None)
        else:
            nc.sync.dma_start(out=out_d[:, :], in_=O.t[:, :])
        psum_cm.__exit__(None, None, None)
        spool_cm.__exit__(None, None, None)
        main_cm.__exit__(None, None, None)

    nc.compile()
    return nc


# ----------------------------------------------------------------------
# host entry
# ----------------------------------------------------------------------
_CACHE = {}

def _get_nc():
    if "nc" not in _CACHE:
        _CACHE["nc"] = build_program()
    return _CACHE["nc"]


def kernel(field_re, field_im, gauge_re, gauge_im):
    from concourse.bass_utils import run_bass_kernel_spmd
    nc = _get_nc()
    in_maps = [_host_inputs(field_re, field_im, gauge_re, gauge_im, c)
               for c in range(NCORES)]
    br = run_bass_kernel_spmd(nc, in_maps, list(range(NCORES)))
    out = np.empty((T_, Z_, Y_, X_, 3, 4, 2), np.float32)
    for c in range(NCORES):
        out[c * TLOC:(c + 1) * TLOC] = _host_output(br.results[c]["outp"])
    return out


# revision 40
# speedup vs baseline: 2.4433x; 1.0027x over previous
"""Wilson-clover Dirac operator D_WC on Trainium2, 8-core SPMD.

Self-contained: hardcodes LAT=(32,16,16,16), shards the T axis across 8
cores with host-side halo slices (t0-1..t0+4), computes everything
site-locally on-device with DVE elementwise math in an SoA layout:

  plane[p, col]: p = z*8 + (y>>1),  col = plane_idx*W + (s-base)*32
                 + (y&1)*16 + x          (s = local T-slice, W = ns*32)

Shifts: T = column offset (free), X = 2-piece ScalarE copy, Z/Y =
TensorE permutation matmul (stationary 128x128 0/1 matrix) + ScalarE
copyback from PSUM.  All elementwise arithmetic runs on the DVE only
(GPSIMD shares the DVE SBUF port; running both concurrently is a net
loss).  Gauge links for all 4 directions are DMA'd into SBUF once.
"""
import numpy as np

# ----------------------------------------------------------------------
# constants (physics)
# ----------------------------------------------------------------------
MASS, C_SW = 0.5, 1.0
_i = 1j
G1 = np.array([[0,0,0,_i],[0,0,_i,0],[0,-_i,0,0],[-_i,0,0,0]], dtype=np.complex64)
G2 = np.array([[0,0,0,-1],[0,0,1,0],[0,1,0,0],[-1,0,0,0]], dtype=np.complex64)
G3 = np.array([[0,0,_i,0],[0,0,0,-_i],[-_i,0,0,0],[0,_i,0,0]], dtype=np.complex64)
G4 = np.array([[0,0,1,0],[0,0,0,1],[1,0,0,0],[0,1,0,0]], dtype=np.complex64)
GAMMA = [G1, G2, G3, G4]
I4 = np.eye(4, dtype=np.complex64)
SIGMA = [[(0.5j * (GAMMA[m] @ GAMMA[n] - GAMMA[n] @ GAMMA[m])).astype(np.complex64)
          for n in range(4)] for m in range(4)]

T_, Z_, Y_, X_ = 32, 16, 16, 16
NCORES, TLOC, NSH = 8, 4, 6       # halo slices per core
SL = 32                           # cols per slice
USE_BF16 = True                   # bf16 compute tiles (O stays fp32)
import os as _os
DBG_PART = _os.environ.get("KRN_PART", "full")    # full|wilson|clover
DBG_PAIRS = _os.environ.get("KRN_PAIRS", "")      # e.g. "12,34" to keep only
DBG_HOPS = _os.environ.get("KRN_HOPS", "0123")    # wilson dirs to keep

# permutation matrix indices (stationary operands for TensorE shifts)
PZ_P, PZ_M, PY_P, PY_M, PZY_M = 0, 1, 2, 3, 4
NPERM = 5

# ----------------------------------------------------------------------
# spin-structure extraction (numerical)
# ----------------------------------------------------------------------
def _col_struct(M):
    """M has single-nonzero columns: return per-col (row, phase)."""
    out = []
    for s in range(4):
        col = M[:, s]
        r = int(np.argmax(np.abs(col)))
        ph = complex(col[r])
        assert np.sum(np.abs(col) > 1e-6) == 1, (M, s)
        out.append((r, ph))
    return out

def _proj_struct(P):
    """P rank-2 with cols 2,3 = rho * cols q in {0,1}. Returns (q2,rho2,q3,rho3)."""
    res = []
    for s in (2, 3):
        found = None
        for j in (0, 1):
            c, cj = P[:, s], P[:, j]
            nz = np.abs(cj) > 1e-6
            if not nz.any():
                continue
            ratio = c[nz] / cj[nz]
            if np.allclose(ratio, ratio[0], atol=1e-5) and np.allclose(
                    c, ratio[0] * cj, atol=1e-5):
                found = (j, complex(ratio[0]))
                break
        assert found is not None, P
        res.append(found)
    return res

# per direction k: gamma column structure and projector relations
GCOL = [_col_struct(GAMMA[k]) for k in range(4)]           # (p(s), g(s))
PROJ_F = [_proj_struct(GAMMA[k] + I4) for k in range(4)]   # for psi@(G+I)
PROJ_B = [_proj_struct(GAMMA[k] - I4) for k in range(4)]   # for psi@(G-I)
SIGCOL = [[_col_struct(SIGMA[m][n]) if m != n else None for n in range(4)]
          for m in range(4)]

def _phase_parts(ph):
    """phase in {1,-1,i,-i} -> for v = ph*u:
    re(v) = sr*u.[pr]  im(v) = si*u.[pi]   (0=re,1=im planes of u)"""
    if abs(ph - 1) < 1e-5:   return (0, 1.0, 1, 1.0)
    if abs(ph + 1) < 1e-5:   return (0, -1.0, 1, -1.0)
    if abs(ph - 1j) < 1e-5:  return (1, -1.0, 0, 1.0)
    if abs(ph + 1j) < 1e-5:  return (1, 1.0, 0, -1.0)
    raise AssertionError(ph)

# ----------------------------------------------------------------------
# host layout helpers
# ----------------------------------------------------------------------
def _to_planes(vol):
    """vol [ns, Z, Y, X] -> [128, ns*32]"""
    ns = vol.shape[0]
    v = vol.reshape(ns, 16, 8, 2, 16)        # s z yh yl x
    v = np.transpose(v, (1, 2, 0, 3, 4))     # z yh s yl x
    return np.ascontiguousarray(v.reshape(128, ns * 32))

def _from_planes(pl, ns):
    v = pl.reshape(16, 8, ns, 2, 16)
    return np.transpose(v, (2, 0, 1, 3, 4)).reshape(ns, 16, 16, 16)

def _perm_z(d):
    """dst partition m = z*8+yh reads p = ((z+d)%16)*8+yh; P[p, m] = 1."""
    P = np.zeros((128, 128), np.float32)
    for z in range(16):
        for yh in range(8):
            P[((z + d) % 16) * 8 + yh, z * 8 + yh] = 1.0
    return P

def _perm_y(d, dz=0):
    """Y cross-half: dst m = z*8+yh reads p = ((z+dz)%16)*8+((yh+d)%8)."""
    P = np.zeros((128, 128), np.float32)
    for z in range(16):
        for yh in range(8):
            P[((z + dz) % 16) * 8 + ((yh + d) % 8), z * 8 + yh] = 1.0
    return P

def _perm_host():
    P = np.concatenate([_perm_z(+1), _perm_z(-1), _perm_y(+1), _perm_y(-1),
                        _perm_y(-1, dz=-1)], axis=1)  # [128, NPERM*128]
    return P

def _host_inputs(field_re, field_im, gauge_re, gauge_im, core):
    t0 = core * TLOC
    ts = [(t0 - 1 + s) % T_ for s in range(NSH)]
    f = np.stack([field_re[ts], field_im[ts]], axis=0)   # [2,6,Z,Y,X,3,4]
    g = np.stack([gauge_re[:, ts], gauge_im[:, ts]], axis=0)  # [2,4,6,Z,Y,X,3,3]

    # field planes: e=(c*4+sp), plane=e*2+r
    fp = np.empty((24, 128, NSH * SL), np.float32)
    for c in range(3):
        for sp in range(4):
            for r in range(2):
                fp[(c * 4 + sp) * 2 + r] = _to_planes(f[r, :, :, :, :, c, sp])
    fld = np.ascontiguousarray(fp.transpose(1, 0, 2).reshape(128, 24 * NSH * SL))

    # gauge planes: dir k, e=(a*3+b), plane=(k*9+e)*2+r
    gp = np.empty((72, 128, NSH * SL), np.float32)
    for k in range(4):
        for a in range(3):
            for b in range(3):
                for r in range(2):
                    gp[(k * 9 + a * 3 + b) * 2 + r] = _to_planes(
                        g[r, k, :, :, :, :, a, b])
    gg = np.ascontiguousarray(gp.transpose(1, 0, 2).reshape(128, 72 * NSH * SL))
    perm = _perm_host()
    if USE_BF16:
        import ml_dtypes
        fld = fld.astype(ml_dtypes.bfloat16)
        gg = gg.astype(ml_dtypes.bfloat16)
        perm = perm.astype(ml_dtypes.bfloat16)
    return {"fld": fld, "gg": gg, "perm": perm}

def _host_output(outp_flat):
    """device out [128, 24*4*32] -> [TLOC, Z,Y,X, 3,4,2]"""
    pl = outp_flat.reshape(128, 24, TLOC * SL).transpose(1, 0, 2)
    out = np.empty((TLOC, 16, 16, 16, 3, 4, 2), np.float32)
    for c in range(3):
        for sp in range(4):
            for r in range(2):
                out[..., c, sp, r] = _from_planes(pl[(c * 4 + sp) * 2 + r], TLOC)
    return out

# ----------------------------------------------------------------------
# device program
# ----------------------------------------------------------------------
class Grp:
    """Group of planes in one SBUF tile. nent complex entries (re+im planes).
    base = slice index of col 0; ns slices; W = ns*32 cols per plane."""
    def __init__(self, tile, nent, base, ns):
        self.t, self.nent, self.base, self.ns = tile, nent, base, ns
        self.W = ns * SL

    def fl(self):
        return self.t.rearrange("p (q w) -> p q w", q=self.nent * 2, w=self.W)

    def cs(self, s0, s1):
        return ((s0 - self.base) * SL, (s1 - self.base) * SL)

    def pap(self, e, r, s0, s1):
        """single plane AP [128, cols]"""
        c0, c1 = self.cs(s0, s1)
        v = self.fl()[:, (e * 2 + r):(e * 2 + r + 1), c0:c1]
        return v  # [p,1,w]

    def gap(self, e0, estep, n, r, s0, s1):
        """packed-entry AP [p, n, w]: entries e0 + i*estep, fixed r."""
        c0, c1 = self.cs(s0, s1)
        q0 = e0 * 2 + r
        fl = self.fl()
        return fl[:, q0:q0 + 2 * estep * (n - 1) + 1:2 * estep, c0:c1]

    def all_ap(self, s0, s1):
        c0, c1 = self.cs(s0, s1)
        return self.fl()[:, :, c0:c1]


def build_program():
    import concourse.bacc as bacc
    import concourse.mybir as mybir
    from concourse.tile import TileContext
    FP = mybir.dt.float32
    CDT = mybir.dt.bfloat16 if USE_BF16 else FP
    AL = mybir.AluOpType

    nc = bacc.Bacc("TRN2", target_bir_lowering=False, debug=False)
    fld_d = nc.declare_dram_parameter("fld", [128, 24 * NSH * SL], CDT, isOutput=False)
    gg_d = nc.declare_dram_parameter("gg", [128, 72 * NSH * SL], CDT, isOutput=False)
    perm_d = nc.declare_dram_parameter("perm", [128, NPERM * 128], CDT, isOutput=False)
    out_d = nc.declare_dram_parameter("outp", [128, 24 * TLOC * SL], FP, isOutput=True)

    with TileContext(nc) as tc:
        _tc = [0]

        def mk(pool, cols, tag, dt=None):
            _tc[0] += 1
            return pool.tile([128, cols], dt or CDT, tag=tag,
                             name=f"{tag}_{_tc[0]}")

        main_cm = tc.tile_pool(name="main", bufs=1)
        main = main_cm.__enter__()
        spool_cm = tc.tile_pool(name="sp", bufs=1)
        spool = spool_cm.__enter__()
        psum_cm = tc.psum_pool(name="ps", bufs=4)
        psum = psum_cm.__enter__()

        F = Grp(mk(main, 24 * NSH * SL, "F"), 12, 0, NSH)
        O = Grp(mk(main, 24 * TLOC * SL, "O", FP), 12, 1, TLOC)
        PM = mk(main, NPERM * 128, "PM")
        FH = 12 * NSH * SL
        nc.sync.dma_start(out=F.t[:, :FH], in_=fld_d[:, :FH])
        nc.scalar.dma_start(out=F.t[:, FH:], in_=fld_d[:, FH:])
        nc.sync.dma_start(out=PM[:, :], in_=perm_d[:, :])

        # gauge links for all 4 directions, loaded once
        GL = []
        for k in range(4):
            g = Grp(mk(main, 9 * 2 * NSH * SL, f"GL{k}"), 9, 0, NSH)
            eng = nc.sync if k % 2 == 0 else nc.scalar
            eng.dma_start(out=g.t[:, :],
                          in_=gg_d[:, k * 18 * NSH * SL:(k + 1) * 18 * NSH * SL])
            GL.append(g)

        # merged scratch tiles (4 product slots each)
        WMX = 5 * SL
        mtA = mk(spool, 4 * 9 * WMX, "mtA")
        msA = mk(spool, 4 * 3 * WMX, "msA")
        asT = mk(spool, 9 * WMX, "asT")      # Gauss a-sums (ar +- ai)
        bsT = mk(spool, 9 * WMX, "bsT")      # Gauss b-sums (br +- bi)

        def tview4(t, nj, nk, w):
            """[p, 4, nj, nk, w] slots of merged mul scratch"""
            v = t.rearrange("p (h m) -> p h m", h=4)
            return v[:, :, :nj * nk * w].rearrange(
                "p h (j k w) -> p h j k w", j=nj, k=nk, w=w)

        def sview4(t, nj, w):
            v = t.rearrange("p (h m) -> p h m", h=4)
            return v[:, :, :nj * w].rearrange("p h (j w) -> p h j w",
                                              j=nj, w=w)

        V = nc.vector

        def a_pack(A, i, adag, r, s0, s1, dt, nj):
            """[p, nj(bcast), 3, w] for a-values (i,k)."""
            e0, st = (i, 3) if adag else (i * 3, 1)
            ap = A.gap(e0, st, 3, r, s0 + dt, s1 + dt)       # [p,3,w]
            w = ap.shape[2]
            return ap.unsqueeze(1).broadcast_to((128, nj, 3, w))

        def b_pack(B, bdag, r, s0, s1, dt, nj):
            """[p, nj, 3, w] for b-values (k,j)."""
            c0, c1 = B.cs(s0 + dt, s1 + dt)
            w = c1 - c0
            if bdag:  # e = j*3+k
                v = B.t.rearrange("p (j k r w) -> p j k r w", j=3, k=3, r=2, w=B.W)
                return v[:, :, :, r, c0:c1]
            if B.nent == 9:  # e = k*3+j
                v = B.t.rearrange("p (k j r w) -> p k j r w", k=3, j=3, r=2, w=B.W)
                return v[:, :, :, r, c0:c1].transpose([0, 2, 1, 3])
            # halfspinor: e = k*2+j, nj=2
            v = B.t.rearrange("p (k j r w) -> p k j r w", k=3, j=2, r=2, w=B.W)
            return v[:, :, :, r, c0:c1].transpose([0, 2, 1, 3])

        def stt(out, in0, coef, in1, op1=None):
            V.scalar_tensor_tensor(out, in0, float(coef), in1,
                                   AL.mult, op1 or AL.add)

        class SGrp:
            """9 single sum-planes (re +- im of a matrix Grp)."""
            def __init__(self, tile, base, ns):
                self.t, self.base, self.ns = tile, base, ns
                self.W = ns * SL

            def cs(self, s0, s1):
                return ((s0 - self.base) * SL, (s1 - self.base) * SL)

            def fl(self):
                return self.t.rearrange("p (e w) -> p e w", e=9, w=self.W)

            def aview(self, i, adag, s0, s1, dt):
                c0, c1 = self.cs(s0 + dt, s1 + dt)
                e0, st = (i, 3) if adag else (i * 3, 1)
                return self.fl()[:, e0:e0 + 2 * st + 1:st, c0:c1]

            def bview(self, bdag, s0, s1, dt):
                c0, c1 = self.cs(s0 + dt, s1 + dt)
                if bdag:  # e = j*3+k
                    v = self.t.rearrange("p (j k w) -> p j k w",
                                         j=3, k=3, w=self.W)
                    return v[:, :, :, c0:c1]
                v = self.t.rearrange("p (k j w) -> p k j w",
                                     k=3, j=3, w=self.W)
                return v[:, :, :, c0:c1].transpose([0, 2, 1, 3])

        def msum(M, sign, tag, pool, rng=None):
            """SGrp of M.re + sign*M.im over rng (default M's full range)."""
            e0, e1 = rng if rng else (M.base, M.base + M.ns)
            g = SGrp(mk(pool, 9 * (e1 - e0) * SL, tag), e0, e1 - e0)
            mr = M.gap(0, 1, 9, 0, e0, e1)
            mi = M.gap(0, 1, 9, 1, e0, e1)
            if sign > 0:
                V.tensor_add(g.fl(), mr, mi)
            else:
                V.tensor_sub(g.fl(), mr, mi)
            return g

        def cmm(dst, A, B, s0, s1, adag=False, bdag=False, adt=0, bdt=0,
                nj=3, asum=None, bsum=None):
            """dst[i,j] = sum_k aval(i,k)*bval(k,j); dst entries e=i*nj+j.
            3-mult (Gauss) complex product: m1=ar*br, m2=ai*bi,
            m3=(ar+sa*ai)*(br+sb*bi); re=R1-s*R2, im=R3-R1-s*R2, s=sa*sb.
            asum/bsum: precomputed SGrp of (re -+ im) matching the dag mode."""
            assert not (adag and bdag)
            w = (s1 - s0) * SL
            neg = adag or bdag
            br_ = b_pack(B, bdag, 0, s0, s1, bdt, nj)
            bi_ = b_pack(B, bdag, 1, s0, s1, bdt, nj)
            if bsum is not None:
                bs = bsum.bview(bdag, s0, s1, bdt)[:, :nj]
            else:
                bs = bsT.rearrange("p (j k w) -> p j k w", j=3, k=3,
                                   w=WMX)[:, :nj, :, :w]
                if bdag:
                    V.tensor_sub(bs, br_, bi_)
                else:
                    V.tensor_add(bs, br_, bi_)
            if asum is None:
                ar_all = A.gap(0, 1, 9, 0, s0 + adt, s1 + adt)
                ai_all = A.gap(0, 1, 9, 1, s0 + adt, s1 + adt)
                as3 = asT.rearrange("p (e w) -> p e w", e=9, w=WMX)[:, :, :w]
                if adag:
                    V.tensor_sub(as3, ar_all, ai_all)
                else:
                    V.tensor_add(as3, ar_all, ai_all)
            for i in range(3):
                tv = tview4(mtA, nj, 3, w)     # slots 0..2 used
                sv = sview4(msA, nj, w)
                ar = a_pack(A, i, adag, 0, s0, s1, adt, nj)
                ai = a_pack(A, i, adag, 1, s0, s1, adt, nj)
                if asum is not None:
                    asb = asum.aview(i, adag, s0, s1, adt)
                elif adag:
                    asb = as3[:, i:i + 7:3, :]
                else:
                    asb = as3[:, i * 3:i * 3 + 3, :]
                asb = asb.unsqueeze(1).broadcast_to((128, nj, 3, w))
                dre = dst.gap(i * nj, 1, nj, 0, s0, s1)
                dim = dst.gap(i * nj, 1, nj, 1, s0, s1)
                V.tensor_mul(tv[:, 0], ar, br_)
                V.tensor_mul(tv[:, 1], ai, bi_)
                V.tensor_mul(tv[:, 2], asb, bs)
                sv3 = sv[:, 0:3]
                V.tensor_add(sv3, tv[:, 0:3, :, 0, :], tv[:, 0:3, :, 1, :])
                V.tensor_add(sv3, sv3, tv[:, 0:3, :, 2, :])
                if neg:
                    V.tensor_add(dre, sv[:, 0], sv[:, 1])
                else:
                    V.tensor_sub(dre, sv[:, 0], sv[:, 1])
                V.tensor_sub(dim, sv[:, 2], sv[:, 0])
                if neg:
                    V.tensor_add(dim, dim, sv[:, 1])
                else:
                    V.tensor_sub(dim, dim, sv[:, 1])

        # ---------- shift materialization ----------
        def cpy(out, in_):
            nc.scalar.copy(out, in_)

        def perm_ap(idx):
            return PM.rearrange("p (i m) -> p i m", i=NPERM)[:, idx, :]

        def mm_permute(dst_t, src_t, cols, pidx, dhalf=None, shalf=None,
                       xrot=False):
            """dst = P[pidx] applied to src partitions, over [0, cols).
            dhalf/shalf: copy back only dst yl==dhalf 16-col blocks, sourced
            from permuted yl==shalf blocks.
            xrot: additionally rotate X by -1 (dst x reads src x-1)."""
            P = perm_ap(pidx)
            c0 = 0
            while c0 < cols:
                cw = min(512, cols - c0)
                pt = psum.tile([128, cw], FP, tag="pshift",
                               name=f"ps_{_tc[0]}_{c0}")
                _tc[0] += 1
                nc.tensor.matmul(pt[:, :], P, src_t[:, c0:c0 + cw])
                dv = dst_t[:, c0:c0 + cw]
                if dhalf is None and not xrot:
                    cpy(dv, pt[:, :])
                elif dhalf is None and xrot:
                    d3 = dv.rearrange("p (b x) -> p b x", x=16)
                    p3 = pt.rearrange("p (b x) -> p b x", x=16)
                    cpy(d3[:, :, 1:16], p3[:, :, 0:15])
                    cpy(d3[:, :, 0:1], p3[:, :, 15:16])
                else:
                    d4 = dv.rearrange("p (b l x) -> p b l x", l=2, x=16)
                    p4 = pt.rearrange("p (b l x) -> p b l x", l=2, x=16)
                    if not xrot:
                        cpy(d4[:, :, dhalf, :], p4[:, :, shalf, :])
                    else:
                        cpy(d4[:, :, dhalf, 1:16], p4[:, :, shalf, 0:15])
                        cpy(d4[:, :, dhalf, 0:1], p4[:, :, shalf, 15:16])
                c0 += cw

        def lv(t_, lo):
            """yl==lo 16-col blocks of flat tile view"""
            return t_.rearrange("p (m x) -> p m x", x=16)[:, lo::2, :]

        def mat_shift(src, axis, d, s0, s1, tag, pool=None):
            """materialize S(x)=src(x + d*e_axis) over the FULL src range.
            axis 1(Z) 2(Y) 3(X)."""
            g = Grp(mk(pool or main, src.nent * 2 * src.ns * SL, tag),
                    src.nent, src.base, src.ns)
            cols = src.nent * 2 * src.ns * SL
            if axis == 3:   # X: 2-piece ScalarE copy
                def xv(t_):
                    return t_.rearrange("p (m x) -> p m x", x=16)
                dv, sv_ = xv(g.t), xv(src.t)
                if d == +1:
                    cpy(dv[:, :, 0:15], sv_[:, :, 1:16])
                    cpy(dv[:, :, 15:16], sv_[:, :, 0:1])
                else:
                    cpy(dv[:, :, 1:16], sv_[:, :, 0:15])
                    cpy(dv[:, :, 0:1], sv_[:, :, 15:16])
            elif axis == 1:  # Z: pure partition permutation
                mm_permute(g.t, src.t, cols, PZ_P if d == +1 else PZ_M)
            else:           # Y: in-partition half + permuted half
                if d == +1:
                    cpy(lv(g.t, 0), lv(src.t, 1))
                    mm_permute(g.t, src.t, cols, PY_P, dhalf=1, shalf=0)
                else:
                    cpy(lv(g.t, 1), lv(src.t, 0))
                    mm_permute(g.t, src.t, cols, PY_M, dhalf=0, shalf=1)
            return g

        def mat_shift_diag(src, a1, a2, tag, pool=None):
            """S(x) = src(x - e_a1 - e_a2), a1 < a2 spatial axes (1,2,3)."""
            g = Grp(mk(pool or main, src.nent * 2 * src.ns * SL, tag),
                    src.nent, src.base, src.ns)
            cols = src.nent * 2 * src.ns * SL
            if (a1, a2) == (1, 2):    # Z,Y
                mm_permute(g.t, src.t, cols, PZ_M, dhalf=1, shalf=0)
                mm_permute(g.t, src.t, cols, PZY_M, dhalf=0, shalf=1)
            elif (a1, a2) == (1, 3):  # Z,X
                mm_permute(g.t, src.t, cols, PZ_M, xrot=True)
            else:                     # Y,X
                dv = g.t.rearrange("p (m x) -> p m x", x=16)[:, 1::2, :]
                sv_ = src.t.rearrange("p (m x) -> p m x", x=16)[:, 0::2, :]
                cpy(dv[:, :, 1:16], sv_[:, :, 0:15])
                cpy(dv[:, :, 0:1], sv_[:, :, 15:16])
                mm_permute(g.t, src.t, cols, PY_M, dhalf=0, shalf=1, xrot=True)
            return g

        # ---------- mass term: O = (MASS+4) * F ----------
        V.tensor_scalar_mul(O.all_ap(1, 5), F.all_ap(1, 5), float(MASS + 4.0))

        # clover chiral blocks W[(i,sp),(j,s')] = sum_p -(1/16) ph_p(sp)
        #   * G9_p[i,j] with s' = wrow_p(sp);  plane index =
        #   chir*72 + spl*36 + sl*18 + i*6 + j*2 + r
        W4 = 4 * SL
        WCH = mk(main, 144 * W4, "WCH")
        wv3 = WCH.rearrange("p (q w) -> p q w", q=144, w=W4)
        wv6 = WCH.rearrange("p (c s l m w) -> p c s l m w",
                            c=2, s=2, l=2, m=18, w=W4)

        def wslot(chir, spl, sl, r):
            """[p, (i,j)(9), w4] view of one spin-slot of W (3D for stt)"""
            base = chir * 72 + spl * 36 + sl * 18 + r
            return wv3[:, base:base + 17:2, :]

        def wrow_ap(chir, i, spl, r):
            """[p, j(3), sl(2), w4] contraction view of one W row"""
            m0 = i * 6 + r
            return wv6[:, chir, spl, :, m0:m0 + 5:2, :].transpose([0, 2, 1, 3])

        # ---------- Wilson hops ----------
        with tc.tile_pool(name="wp", bufs=2) as wpool:
            for k in (range(4) if DBG_PART in ("full", "wilson") else []):
                if str(k) not in DBG_HOPS:
                    continue
                ax = k  # lattice axis
                Uk = GL[k]
                # ---- forward hop ----
                rng = (0, 4) if k == 0 else (1, 5)
                s0, s1 = rng
                h = Grp(mk(wpool, 12 * (s1 - s0) * SL, "h"),
                        6, s0, s1 - s0)
                # h[c,j] = psi[c,j] + g(j)*psi[c,p(j)]
                for j in range(2):
                    pj, gj = GCOL[k][j]
                    for r in range(2):
                        pr, psgn = (_phase_parts(gj)[0:2] if r == 0
                                    else _phase_parts(gj)[2:4])
                        dst = h.gap(j, 2, 3, r, s0, s1)       # c-packed
                        a0 = F.gap(j, 4, 3, r, s0, s1)        # psi[c,j].r
                        a1 = F.gap(pj, 4, 3, pr, s0, s1)
                        if psgn > 0:
                            V.tensor_add(dst, a1, a0)
                        else:
                            V.tensor_sub(dst, a0, a1)
                phi = Grp(mk(wpool, 12 * (s1 - s0) * SL, "phi"),
                          6, s0, s1 - s0)
                cmm(phi, Uk, h, s0, s1, adag=True, nj=2)
                # shift (-1 along ax) then reconstruct into O
                if k == 0:
                    psh, dt = phi, -1
                else:
                    psh, dt = mat_shift(phi, ax, -1, 1, 5, "psh", wpool), 0
                # out[:, s'] += -0.5 * rho(s') * psh[:, q(s')]
                rec = [(0, 1.0), (1, 1.0), PROJ_F[k][0], PROJ_F[k][1]]
                for sp in range(4):
                    q, rho = rec[sp]
                    for r in range(2):
                        pr, psgn = (_phase_parts(rho)[0:2] if r == 0
                                    else _phase_parts(rho)[2:4])
                        dst = O.gap(sp, 4, 3, r, 1, 5)
                        src_ = psh.gap(q, 2, 3, pr, 1 + dt, 5 + dt)
                        stt(dst, src_, -0.5 * psgn, dst, AL.add)
                # ---- backward hop ----
                rng = (2, 6) if k == 0 else (1, 5)
                s0, s1 = rng
                hb = Grp(mk(wpool, 12 * (s1 - s0) * SL, "h"),
                         6, s0, s1 - s0)
                # hb[c,j] = g(j)*psi[c,p(j)] - psi[c,j]
                for j in range(2):
                    pj, gj = GCOL[k][j]
                    for r in range(2):
                        pr, psgn = (_phase_parts(gj)[0:2] if r == 0
                                    else _phase_parts(gj)[2:4])
                        dst = hb.gap(j, 2, 3, r, s0, s1)
                        a1 = F.gap(pj, 4, 3, pr, s0, s1)
                        a0 = F.gap(j, 4, 3, r, s0, s1)
                        if psgn > 0:
                            V.tensor_sub(dst, a1, a0)
                        else:
                            stt(dst, a1, psgn, a0, AL.subtract)
                if k == 0:
                    hs, dt = hb, +1
                else:
                    hs, dt = mat_shift(hb, ax, +1, 1, 5, "psh", wpool), 0
                gm = Grp(mk(wpool, 12 * 4 * SL, "phi"),
                         6, 1, 4)
                cmm(gm, Uk, hs, 1, 5, nj=2, bdt=dt)
                rec = [(0, 1.0), (1, 1.0), PROJ_B[k][0], PROJ_B[k][1]]
                for sp in range(4):
                    q, rho = rec[sp]
                    for r in range(2):
                        pr, psgn = (_phase_parts(rho)[0:2] if r == 0
                                    else _phase_parts(rho)[2:4])
                        dst = O.gap(sp, 4, 3, r, 1, 5)
                        src_ = gm.gap(q, 2, 3, pr, 1, 5)
                        stt(dst, src_, 0.5 * psgn, dst, AL.add)

        # ---------- clover planes ----------
        _winit = set()
        with tc.tile_pool(name="cp", bufs=1) as cpool:
            def ctile(tag, nent, s0, s1):
                return Grp(mk(cpool, nent * 2 * (s1 - s0) * SL, tag), nent, s0, s1 - s0)

            for d1 in (range(1, 5) if DBG_PART in ("full", "clover") else []):
                for d2 in range(d1 + 1, 5):
                    if DBG_PAIRS and f"{d1}{d2}" not in DBG_PAIRS.split(","):
                        continue
                    a1, a2 = d1 - 1, d2 - 1
                    tpl = (d1 == 1)
                    sA, eA = (0, 5) if tpl else (1, 5)
                    U1 = GL[d1 - 1]
                    U2 = GL[d2 - 1]
                    # shifted links
                    if tpl:
                        U2m, u2dt = U2, +1       # U_d2(x+e_T): column view
                    else:
                        U2m, u2dt = mat_shift(U2, a1, +1, sA, eA, "lnk1", cpool), 0
                    U1n = mat_shift(U1, a2, +1, sA, eA, "lnk2", cpool)
                    sU1p = msum(U1, +1, "sU1p", cpool, (sA, eA))   # A.a, E.b
                    sU1nP = msum(U1n, +1, "sU1nP", cpool, (sA, eA))  # B.b, D.a
                    if tpl:
                        sU2p = msum(U2, +1, "sU2p", cpool)       # B.a, A.b
                        sU2mM = msum(U2, -1, "sU2m", cpool)      # E.a, D.b
                        sU2mP, sU2mM2 = sU2p, sU2mM              # column-dt
                    else:
                        sU2p = msum(U2, +1, "sU2p", cpool, (sA, eA))
                        sU2mM = msum(U2, -1, "sU2m", cpool, (sA, eA))
                        sU2mP = msum(U2m, +1, "sU2mP", cpool, (sA, eA))
                        sU2mM2 = msum(U2m, -1, "sU2mM2", cpool, (sA, eA))
                    A = ctile("pA", 9, sA, eA)
                    cmm(A, U1, U2m, sA, eA, bdt=u2dt, asum=sU1p, bsum=sU2mP)
                    B = ctile("pB", 9, sA, eA)
                    cmm(B, U2, U1n, sA, eA, asum=sU2p, bsum=sU1nP)
                    sAm = msum(A, -1, "sAm", cpool)          # L1.a, Q.b
                    sBp = msum(B, +1, "sBp", cpool)          # Q.a, L1.b
                    Q = ctile("pQ", 9, 1, 5)
                    cmm(Q, B, A, 1, 5, bdag=True, asum=sBp, bsum=sAm)  # L4
                    sL1, eL1 = (0, 4) if tpl else (1, 5)
                    L1 = ctile("pL", 9, sL1, eL1)
                    cmm(L1, A, B, sL1, eL1, adag=True, asum=sAm, bsum=sBp)
                    # L1 shift overlaps D/E products; Q-adds deferred
                    if tpl:
                        L1s = mat_shift(L1, a2, -1, 0, 4, "pLs", cpool)
                        l1_ap = L1s.all_ap(0, 4)
                    else:
                        L1ss = mat_shift_diag(L1, a1, a2, "pLs", cpool)
                        l1_ap = L1ss.all_ap(1, 5)
                    D = ctile("pA2", 9, sA, eA)
                    cmm(D, U1n, U2m, sA, eA, bdag=True, bdt=u2dt,
                        asum=sU1nP, bsum=sU2mM2)
                    E = ctile("pB2", 9, sA, eA)
                    cmm(E, U2, U1, sA, eA, adag=True, asum=sU2mM, bsum=sU1p)
                    V.tensor_add(Q.all_ap(1, 5), Q.all_ap(1, 5), l1_ap)
                    sDp = msum(D, +1, "sDp", cpool)          # L2.a, L3.b
                    sEm = msum(E, -1, "sEm", cpool)          # L3.a, L2.b
                    L2 = ctile("pL2", 9, 1, 5)
                    cmm(L2, D, E, 1, 5, bdag=True, asum=sDp, bsum=sEm)
                    L2s = mat_shift(L2, a2, -1, 1, 5, "pL2s", cpool)
                    sL3, eL3 = (0, 4) if tpl else (1, 5)
                    L3 = ctile("pL3", 9, sL3, eL3)
                    cmm(L3, E, D, sL3, eL3, adag=True, asum=sEm, bsum=sDp)
                    V.tensor_add(Q.all_ap(1, 5), Q.all_ap(1, 5),
                                 L2s.all_ap(1, 5))
                    if tpl:
                        V.tensor_add(Q.all_ap(1, 5), Q.all_ap(1, 5),
                                     L3.all_ap(0, 4))
                    else:
                        L3s = mat_shift(L3, a1, -1, 1, 5, "pLs2", cpool)
                        V.tensor_add(Q.all_ap(1, 5), Q.all_ap(1, 5),
                                     L3s.all_ap(1, 5))
                    # ---- G9 = Q - Q^dag ----
                    G9 = ctile("pG9", 9, 1, 5)
                    qv = Q.t.rearrange("p (i j r w) -> p i j r w",
                                       i=3, j=3, r=2, w=Q.W)
                    qT = qv.transpose([0, 2, 1, 3, 4])
                    gv = G9.t.rearrange("p (i j r w) -> p i j r w",
                                        i=3, j=3, r=2, w=G9.W)
                    V.tensor_sub(gv[:, :, :, 0, :], qv[:, :, :, 0, :],
                                 qT[:, :, :, 0, :])
                    V.tensor_add(gv[:, :, :, 1, :], qv[:, :, :, 1, :],
                                 qT[:, :, :, 1, :])
                    # ---- accumulate into chiral W blocks (coef = +-1;
                    #      the 1/16 is applied in the final O accumulate) ----
                    for sp in range(4):
                        wrow, phi_ph = SIGCOL[a1][a2][sp]
                        assert wrow // 2 == sp // 2, (a1, a2, sp, wrow)
                        c = -phi_ph
                        chir, spl, sl = sp // 2, sp % 2, wrow % 2
                        # re(W) += re(c) G9re - im(c) G9im
                        # im(W) += re(c) G9im + im(c) G9re
                        for outr in range(2):
                            if abs(c.imag) < 1e-6:
                                rsrc, coef = outr, c.real
                            else:
                                rsrc, coef = 1 - outr, (-c.imag if outr == 0
                                                        else c.imag)
                            wv = wslot(chir, spl, sl, outr)
                            src_ = G9.gap(0, 1, 9, rsrc, 1, 5)
                            key = (chir, spl, sl, outr)
                            if key not in _winit:
                                _winit.add(key)
                                V.tensor_scalar_mul(wv, src_, float(coef))
                            elif coef > 0:
                                V.tensor_add(wv, wv, src_)
                            else:
                                V.tensor_sub(wv, wv, src_)

        # ---------- apply chiral clover blocks: O += W psi ----------
        if _winit:
            apool_cm = tc.tile_pool(name="ap", bufs=1)
            apool = apool_cm.__enter__()
            R = mk(apool, 24 * W4, "Rap")
            r3 = R.rearrange("p (q w) -> p q w", q=24, w=W4)
            f5 = F.t.rearrange("p (c sp r w) -> p c sp r w",
                               c=3, sp=4, r=2, w=F.W)
            fc0, fc1 = F.cs(1, 5)
            tv = tview4(mtA, 3, 2, W4)     # [p,4,3(j),2(l),w]
            sv = sview4(msA, 3, W4)        # [p,4,3,w]
            for chir in range(2):
                br = f5[:, :, chir * 2:chir * 2 + 2, 0, fc0:fc1]
                bi = f5[:, :, chir * 2:chir * 2 + 2, 1, fc0:fc1]
                for i in range(3):
                    for spl in range(2):
                        ar = wrow_ap(chir, i, spl, 0)
                        ai = wrow_ap(chir, i, spl, 1)
                        V.tensor_mul(tv[:, 0], ar, br)
                        V.tensor_mul(tv[:, 1], ai, bi)
                        V.tensor_mul(tv[:, 2], ar, bi)
                        V.tensor_mul(tv[:, 3], ai, br)
                        t = sv[:, :, 0:2, :]       # [p,4,2,w]
                        V.tensor_add(t, tv[:, :, 0, :, :], tv[:, :, 1, :, :])
                        V.tensor_add(t, t, tv[:, :, 2, :, :])
                        s = sv[:, :, 2, :]         # [p,4,w]
                        V.tensor_add(s, t[:, :, 0, :], t[:, :, 1, :])
                        pl = i * 8 + chir * 4 + spl * 2
                        V.tensor_sub(r3[:, pl:pl + 1, :],
                                     s[:, 0:1, :], s[:, 1:2, :])
                        V.tensor_add(r3[:, pl + 1:pl + 2, :],
                                     s[:, 2:3, :], s[:, 3:4, :])
            # chunked final accumulate + store so DMA overlaps the stt tail
            OC = 24 * TLOC * SL
            for ci in range(4):
                c0, c1 = ci * OC // 4, (ci + 1) * OC // 4
                stt(O.t[:, c0:c1], R[:, c0:c1], 1.0 / 16.0,
                    O.t[:, c0:c1], AL.add)
                eng = nc.sync if ci % 2 == 0 else nc.scalar
                eng.dma_start(out=out_d[:, c0:c1], in_=O.t[:, c0:c1])
            apool_cm.__exit__(None, None, None)
        else:
            nc.sync.dma_start(out=out_d[:, :], in_=O.t[:, :])
        psum_cm.__exit__(None, None, None)
        spool_cm.__exit__(None, None, None)
        main_cm.__exit__(None, None, None)

    nc.compile()
    return nc


# ----------------------------------------------------------------------
# host entry
# ----------------------------------------------------------------------
_CACHE = {}

def _get_nc():
    if "nc" not in _CACHE:
        _CACHE["nc"] = build_program()
    return _CACHE["nc"]


def kernel(field_re, field_im, gauge_re, gauge_im):
    from concourse.bass_utils import run_bass_kernel_spmd
    nc = _get_nc()
    in_maps = [_host_inputs(field_re, field_im, gauge_re, gauge_im, c)
               for c in range(NCORES)]
    br = run_bass_kernel_spmd(nc, in_maps, list(range(NCORES)))
    out = np.empty((T_, Z_, Y_, X_, 3, 4, 2), np.float32)
    for c in range(NCORES):
        out[c * TLOC:(c + 1) * TLOC] = _host_output(br.results[c]["outp"])
    return out
